# revision 1
# baseline (speedup 1.0000x reference)
"""Trainium2 Bass kernel for a dense transformer block (B=8, S=2048, D=768, H=3072).

Sharding: pure data-parallel over batch -- one batch element per NeuronCore.

All heavy matmuls run as fp8-e4m3 with MatmulPerfMode.DoubleRow (two 128-row
contraction arms per instruction at 0.5 cycles/row -> 4x the fp32r MAC rate).
Error control (absmax gate 2e-2, measured 6.8e-3 in fp64 emulation):
  - attention path (q/k/v/scores/exp/y/o) is naive e4m3: errors average out
    over the 768/2048-long contractions (each source < 3.2e-3 alone).
  - MLP operands carry hi+lo splits: h2 = hi + lo (both e4m3), W = Whi(e4m3)
    + Wlo(e5m2, captures sub-2^-9 residuals without rescaling). Each matmul
    takes three DoubleRow passes: Whi*hi + Whi*lo + Wlo*hi.
  - exp is computed as exp(s/sqrt(d) - 3ln2) = e/8 so the UNNORMALIZED
    attention accumulator sum(e*v) stays below the e4m3 max of 240; the
    softmax 1/Z is applied per-token on the x2 write (via a PE-transposed
    reciprocal column), so o-proj never waits on the normalization chain,
    and bo enters the PSUM group as bo*Z so the 1/Z scaling cancels it.
  - weights are quantized host-side with the LN gammas folded in
    (h @ diag(g) @ W == LN_nogamma(h) @ W'), betas folded into the biases.
  - LN rstd = rsqrt(var+eps) via the quake bit-trick + Newton on DVE integer
    ALU ops, keeping the Sqrt activation table (and its 1.3us reloads
    between softmax exps) off the ACT engine entirely.

Layout (per core): feature-major hT/kT/qT [P, DT, S] fp8; token-major v
[P, ST, D] fp8; x2 kept f32 in SBUF; h2/m stored as [.., 2, ..] hi/lo pairs.
Per-feature biases (bv, bo, bproj) are added inside the PSUM group by a tiny
[1,128]x[1,D] bf16 ones-row matmul instead of an extra vector pass.

Schedule: LN1 runs in groups of 4 tiles (batched rsqrt) one group ahead of
the transpose/v/k consumers; chunk-0 scores+exp are woven into phase 1;
LN2+transposes for chunks 1-3 run inside the PE-bound MLP phase where the
vector engines idle. GPSIMD (Pool) only ever touches SBUF (hw restriction);
all PSUM reads go through DVE/ACT.
"""

import numpy as np

P = 128
S, D, H = 2048, 768, 3072
DT = D // P            # 6 d-tiles
HT = H // P            # 24 h-tiles
ST = S // P            # 16 token tiles
CH = 512               # s1 chunk width
NCH = S // CH          # 4 chunks
TPC = CH // P          # 4 token tiles per chunk
EPS = 1e-5
N_CORES = 8
LN4 = 2.0794415416798357   # 3*ln2; exp bias so e8 = exp(s)/8

WEIGHT_NAMES = [
    "ln1_g", "ln1_b", "ln2_g", "ln2_b",
    "Wq", "bq", "Wk", "bk", "Wv", "bv", "Wo", "bo",
    "Wfc", "bfc", "Wproj", "bproj",
]

_CACHE = {}


def _prep(inputs):
    """Host-side weight quantization + LN/bias folding (pure numpy)."""
    import ml_dtypes
    E4, E5, BF = ml_dtypes.float8_e4m3, ml_dtypes.float8_e5m2, ml_dtypes.bfloat16
    f32 = lambda k: np.asarray(inputs[k], dtype=np.float32)
    g1, b1 = f32("ln1_g"), f32("ln1_b")
    g2, b2 = f32("ln2_g"), f32("ln2_b")
    Wq, Wk, Wv, Wo = f32("Wq"), f32("Wk"), f32("Wv"), f32("Wo")
    Wfc, Wproj = f32("Wfc"), f32("Wproj")
    q8 = lambda a: np.ascontiguousarray(a.astype(E4))
    Wfc_g = g2[:, None] * Wfc
    Wfh = Wfc_g.astype(E4)
    Wph = Wproj.astype(E4)
    return {
        "wq8": q8(g1[:, None] * Wq), "wk8": q8(g1[:, None] * Wk),
        "wv8": q8(g1[:, None] * Wv), "wo8": q8(Wo),
        "bq_": f32("bq") + b1 @ Wq, "bk_": f32("bk") + b1 @ Wk,
        "bo16": np.asarray(
            f32("bo") + (f32("bv") + b1 @ Wv) @ Wo, dtype=BF),
        "wfh": np.ascontiguousarray(Wfh),
        "wfl": np.ascontiguousarray(
            (Wfc_g - Wfh.astype(np.float32)).astype(E5)),
        "wph": np.ascontiguousarray(Wph),
        "wpl": np.ascontiguousarray(
            (Wproj - Wph.astype(np.float32)).astype(E5)),
        "bfc_": f32("bfc") + b2 @ Wfc,
        "bp16": np.asarray(f32("bproj"), dtype=BF),
    }


def _build():
    import concourse.bass as bass
    import concourse.tile as tile
    from concourse import bacc, mybir
    from concourse.masks import make_identity
    from contextlib import ExitStack

    F = mybir.dt.float32
    BF = mybir.dt.bfloat16
    E4 = mybir.dt.float8e4
    E5 = mybir.dt.float8e5
    I32 = mybir.dt.int32
    AF = mybir.ActivationFunctionType
    OP = mybir.AluOpType
    DR = mybir.MatmulPerfMode.DoubleRow

    nc = bacc.Bacc(None, target_bir_lowering=False)

    x_d = nc.dram_tensor("x", [S, D], F, kind="ExternalInput")
    out_d = nc.dram_tensor("out", [S, D], F, kind="ExternalOutput")
    wq8_d = nc.dram_tensor("wq8", [D, D], E4, kind="ExternalInput")
    wk8_d = nc.dram_tensor("wk8", [D, D], E4, kind="ExternalInput")
    wv8_d = nc.dram_tensor("wv8", [D, D], E4, kind="ExternalInput")
    wo8_d = nc.dram_tensor("wo8", [D, D], E4, kind="ExternalInput")
    wfh_d = nc.dram_tensor("wfh", [D, H], E4, kind="ExternalInput")
    wfl_d = nc.dram_tensor("wfl", [D, H], E5, kind="ExternalInput")
    wph_d = nc.dram_tensor("wph", [H, D], E4, kind="ExternalInput")
    wpl_d = nc.dram_tensor("wpl", [H, D], E5, kind="ExternalInput")
    bq_d = nc.dram_tensor("bq_", [D], F, kind="ExternalInput")
    bk_d = nc.dram_tensor("bk_", [D], F, kind="ExternalInput")
    bfc_d = nc.dram_tensor("bfc_", [H], F, kind="ExternalInput")
    bo_d = nc.dram_tensor("bo16", [D], BF, kind="ExternalInput")
    bp_d = nc.dram_tensor("bp16", [D], BF, kind="ExternalInput")

    def bcast_ap(dram_t, n_part=P):
        ap = dram_t.ap()
        return bass.AP(tensor=ap.tensor, offset=ap.offset,
                       ap=[[0, n_part]] + list(ap.ap))

    inv_sqrt_d = 1.0 / float(np.sqrt(np.float32(D)))

    with tile.TileContext(nc) as tc, ExitStack() as ctx:
        singles = ctx.enter_context(tc.tile_pool(name="singles", bufs=1))

        ident16 = singles.tile([P, P], BF)
        make_identity(nc, ident16)
        ones_row = singles.tile([1, P], BF)
        nc.vector.memset(ones_row, 1.0)
        eps_t = singles.tile([P, 1], F)
        nc.vector.memset(eps_t, EPS)
        bo_row = singles.tile([1, D], BF)
        bp_row = singles.tile([1, D], BF)
        zero_t = singles.tile([P, 1], F)
        nc.vector.memset(zero_t, 0.0)
        bq_col = singles.tile([P, DT], F)
        bk_col = singles.tile([P, DT], F)
        bfc_col = singles.tile([P, HT], F)
        ident32 = singles.tile([P, P], F)
        make_identity(nc, ident32)
        ones8 = singles.tile([P, 2, P], E4)
        nc.vector.memset(ones8, 1.0)
        nln4_t = singles.tile([P, 1], F)
        nc.vector.memset(nln4_t, -LN4)

        # persistent activations
        perm = ctx.enter_context(tc.tile_pool(name="perm", bufs=1))
        x2_sb = perm.tile([P, ST, D], F)           # residual stream (48KB/p)
        h2s = perm.tile([P, NCH, DT, 2, CH], E4)   # LN2 out hi/lo (24KB/p)

        wmlp_ctx = ExitStack()
        wfcp = wmlp_ctx.enter_context(tc.tile_pool(name="wfcp", bufs=1))

        qkv_ctx = ExitStack()
        qkvp = qkv_ctx.enter_context(tc.tile_pool(name="qkv", bufs=1))
        k8 = qkvp.tile([P, DT, S], E4)
        q8 = qkvp.tile([P, DT, S], E4)
        v8 = qkvp.tile([P, ST, D], E4)
        wqkv = qkv_ctx.enter_context(tc.tile_pool(name="wqkv", bufs=1))
        wv_t = wqkv.tile([P, DT, D], E4)
        wk_t = wqkv.tile([P, DT, D], E4)
        wq_t = wqkv.tile([P, DT, D], E4)
        wo_t = wqkv.tile([P, DT, D], E4)

        e8_ctx = ExitStack()
        e8p = e8_ctx.enter_context(tc.tile_pool(name="e8p", bufs=2))

        # ------------- Phase 1: LN1 -> hT8; v, k, q (all fp8) -------------
        hT_ctx = ExitStack()
        hTp = hT_ctx.enter_context(tc.tile_pool(name="hT", bufs=1))
        hT8 = hTp.tile([P, DT, S], E4)

        with (
            tc.tile_pool(name="ph1", bufs=3) as ph1,
            tc.tile_pool(name="ps_tr", bufs=2, space="PSUM") as ps_trp,
            tc.tile_pool(name="ps_v", bufs=1, space="PSUM") as ps_vp,
            tc.tile_pool(name="ps_k", bufs=4, space="PSUM") as ps_kp,
        ):
            x_ts = [None] * ST
            h_ts = [None] * ST
            e8_c0 = None

            def ln1_group(g):
                mvg = ph1.tile([P, TPC, 2], F, tag="mvg")
                for i in range(TPC):
                    st = TPC * g + i
                    x_t = ph1.tile([P, D], F, tag="xt", bufs=6)
                    if st % 2:
                        nc.gpsimd.dma_start(
                            out=x_t, in_=x_d.ap()[st * P:(st + 1) * P, :])
                    else:
                        nc.sync.dma_start(x_t,
                                          x_d.ap()[st * P:(st + 1) * P, :])
                    if st == 0:
                        nc.sync.dma_start(
                            wv_t[:],
                            wv8_d.ap().rearrange("(t p) n -> p t n", p=P))
                    if st == 1:
                        nc.sync.dma_start(
                            wk_t[:],
                            wk8_d.ap().rearrange("(t p) n -> p t n", p=P))
                        nc.sync.dma_start(
                            bk_col, bk_d.ap().rearrange("(t p) -> p t", p=P))
                        nc.sync.dma_start(
                            bq_col, bq_d.ap().rearrange("(t p) -> p t", p=P))
                    if st == 2:
                        nc.sync.dma_start(
                            wq_t[:],
                            wq8_d.ap().rearrange("(t p) n -> p t n", p=P))
                    x_ts[st] = x_t
                    stats = ph1.tile([P, 3, 6], F, tag="st")
                    for j in range(3):
                        nc.vector.bn_stats(out=stats[:, j, :],
                                           in_=x_t[:, j * 256:(j + 1) * 256])
                    nc.vector.bn_aggr(out=mvg[:, i, :], in_=stats)
                # batched rsqrt(var+eps): quake bit-trick + 1 Newton (DVE)
                rsg = ph1.tile([P, TPC], F, tag="rsg")
                nc.vector.tensor_scalar(out=rsg, in0=mvg[:, :, 1], scalar1=EPS,
                                        scalar2=None, op0=OP.add)
                rig = ph1.tile([P, TPC], I32, tag="rig")
                nc.vector.tensor_scalar(out=rig, in0=rsg[:].bitcast(I32),
                                        scalar1=1, scalar2=None,
                                        op0=OP.logical_shift_right)
                nc.vector.tensor_scalar(out=rig, in0=rig, scalar1=-1,
                                        scalar2=None, op0=OP.bitwise_xor)
                nc.vector.tensor_scalar(out=rig, in0=rig, scalar1=0x5f3759e0,
                                        scalar2=None, op0=OP.add)
                rng = ph1.tile([P, TPC], F, tag="rng")
                nc.vector.tensor_tensor(out=rng, in0=rig[:].bitcast(F),
                                        in1=rig[:].bitcast(F), op=OP.mult)
                nc.vector.tensor_tensor(out=rng, in0=rng, in1=rsg, op=OP.mult)
                nc.vector.tensor_scalar(out=rng, in0=rng, scalar1=-0.5,
                                        scalar2=1.5, op0=OP.mult, op1=OP.add)
                nc.vector.tensor_tensor(out=rsg, in0=rig[:].bitcast(F),
                                        in1=rng, op=OP.mult)
                for i in range(TPC):
                    st = TPC * g + i
                    h_t = ph1.tile([P, D], BF, tag="ht", bufs=6)
                    nc.gpsimd.tensor_scalar(out=h_t, in0=x_ts[st],
                                            scalar1=mvg[:, i, 0:1],
                                            scalar2=rsg[:, i:i + 1],
                                            op0=OP.subtract, op1=OP.mult)
                    h_ts[st] = h_t

            def consume_group(pg):
                nonlocal e8_c0
                for i in range(TPC):
                    sp = TPC * pg + i
                    h_t = h_ts[sp]
                    ps_tr = ps_trp.tile([P, DT, P], BF, tag="tr")
                    for dt_ in range(DT):
                        nc.tensor.transpose(ps_tr[:, dt_, :],
                                            h_t[:, dt_ * P:(dt_ + 1) * P],
                                            ident16)
                    nc.scalar.activation(
                        out=hT8[:, :, sp * P:(sp + 1) * P],
                        in_=ps_tr, func=AF.Copy, bias=0.0, scale=1.0)
                for i in range(TPC):
                    sv = TPC * pg + i
                    ps_v = ps_vp.tile([P, 1024], F, tag="v")
                    for dc, lo, w in ((0, 0, 512), (1, 512, 256)):
                        for j in range(DT // 2):
                            nc.tensor.matmul(
                                ps_v[:, lo:lo + w],
                                hT8[:, 2 * j:2 * j + 2, sv * P:(sv + 1) * P],
                                wv_t[:, 2 * j:2 * j + 2, lo:lo + w],
                                start=(j == 0), stop=(j == DT // 2 - 1),
                                perf_mode=DR)
                    # bv is folded into bo' host-side: sum(e*(v+bv)) =
                    # sum(e*v) + Z*bv, and the Z*bv@Wo term rides the bo row
                    nc.scalar.activation(out=v8[:, sv, :], in_=ps_v[:, :D],
                                         func=AF.Copy, bias=0.0, scale=1.0)
                kc = pg
                for dtp in range(DT):
                    ps_k = ps_kp.tile([P, CH], F, tag="k")
                    for j in range(DT // 2):
                        nc.tensor.matmul(
                            ps_k,
                            wk_t[:, 2 * j:2 * j + 2, dtp * P:(dtp + 1) * P],
                            hT8[:, 2 * j:2 * j + 2, kc * CH:(kc + 1) * CH],
                            start=(j == 0), stop=(j == DT // 2 - 1),
                            perf_mode=DR)
                    if dtp % 2:
                        nc.scalar.activation(
                            out=k8[:, dtp, kc * CH:(kc + 1) * CH], in_=ps_k,
                            func=AF.Identity, bias=bk_col[:, dtp:dtp + 1],
                            scale=1.0)
                    else:
                        nc.vector.tensor_scalar(
                            out=k8[:, dtp, kc * CH:(kc + 1) * CH], in0=ps_k,
                            scalar1=bk_col[:, dtp:dtp + 1], scalar2=None,
                            op0=OP.add)
                if kc == 0:
                    e8_c0 = e8p.tile([P, ST, CH], E4, tag="e8", name="e8_c0")
                    for dtp in range(DT):
                        ps_q = ps_kp.tile([P, CH], F, tag="k", name="ps_q0")
                        for j in range(DT // 2):
                            nc.tensor.matmul(
                                ps_q,
                                wq_t[:, 2 * j:2 * j + 2,
                                     dtp * P:(dtp + 1) * P],
                                hT8[:, 2 * j:2 * j + 2, 0:CH],
                                start=(j == 0), stop=(j == DT // 2 - 1),
                                perf_mode=DR)
                        if dtp % 2:
                            nc.scalar.activation(
                                out=q8[:, dtp, 0:CH], in_=ps_q,
                                func=AF.Identity,
                                bias=bq_col[:, dtp:dtp + 1], scale=1.0)
                        else:
                            nc.vector.tensor_scalar(
                                out=q8[:, dtp, 0:CH], in0=ps_q,
                                scalar1=bq_col[:, dtp:dtp + 1], scalar2=None,
                                op0=OP.add)
                # chunk-0 scores for this k-chunk's keys
                for st2 in range(TPC * kc, TPC * kc + TPC):
                    ps_s = ps_kp.tile([P, CH], F, tag="k", name="ps_s0")
                    for j in range(DT // 2):
                        nc.tensor.matmul(
                            ps_s,
                            k8[:, 2 * j:2 * j + 2, st2 * P:(st2 + 1) * P],
                            q8[:, 2 * j:2 * j + 2, 0:CH],
                            start=(j == 0), stop=(j == DT // 2 - 1),
                            perf_mode=DR)
                    nc.scalar.activation(out=e8_c0[:, st2, :], in_=ps_s,
                                         func=AF.Exp, scale=inv_sqrt_d,
                                         bias=nln4_t)

            for g in range(TPC + 1):
                if g >= 1:
                    consume_group(g - 1)
                if g < TPC:
                    ln1_group(g)
            # q for chunks 1..3 (frees hT afterwards)
            nc.sync.dma_start(wo_t[:],
                              wo8_d.ap().rearrange("(t p) n -> p t n", p=P))
            nc.sync.dma_start(bo_row, bo_d.ap().unsqueeze(0))
            nc.sync.dma_start(bp_row, bp_d.ap().unsqueeze(0))
            nc.sync.dma_start(bfc_col,
                              bfc_d.ap().rearrange("(t p) -> p t", p=P))
            for sc in range(1, NCH):
                for dtp in range(DT):
                    ps_q = ps_kp.tile([P, CH], F, tag="k")
                    for j in range(DT // 2):
                        nc.tensor.matmul(
                            ps_q,
                            wq_t[:, 2 * j:2 * j + 2, dtp * P:(dtp + 1) * P],
                            hT8[:, 2 * j:2 * j + 2, sc * CH:(sc + 1) * CH],
                            start=(j == 0), stop=(j == DT // 2 - 1),
                            perf_mode=DR)
                    if dtp % 2:
                        nc.scalar.activation(
                            out=q8[:, dtp, sc * CH:(sc + 1) * CH], in_=ps_q,
                            func=AF.Identity, bias=bq_col[:, dtp:dtp + 1],
                            scale=1.0)
                    else:
                        nc.vector.tensor_scalar(
                            out=q8[:, dtp, sc * CH:(sc + 1) * CH], in0=ps_q,
                            scalar1=bq_col[:, dtp:dtp + 1], scalar2=None,
                            op0=OP.add)
        hT_ctx.close()

        def quake_rsqrt(pool, mvs, rss):
            # rsqrt(var+eps): quake bit-trick + 2 Newton steps, all on DVE
            vb = pool.tile([P, TPC], F, tag="vb")
            nc.vector.tensor_scalar(out=vb, in0=mvs[:, :, 1], scalar1=EPS,
                                    scalar2=None, op0=OP.add)
            ib = pool.tile([P, TPC], I32, tag="ib")
            nc.vector.tensor_scalar(out=ib, in0=vb[:].bitcast(I32),
                                    scalar1=1, scalar2=None,
                                    op0=OP.logical_shift_right)
            nc.vector.tensor_scalar(out=ib, in0=ib, scalar1=-1,
                                    scalar2=None, op0=OP.bitwise_xor)
            nc.vector.tensor_scalar(out=ib, in0=ib, scalar1=0x5f3759e0,
                                    scalar2=None, op0=OP.add)
            nc.vector.tensor_copy(out=rss, in_=ib[:].bitcast(F))
            nt = pool.tile([P, TPC], F, tag="nt")
            for _ in range(2):
                nc.vector.tensor_tensor(out=nt, in0=rss, in1=rss, op=OP.mult)
                nc.vector.tensor_tensor(out=nt, in0=nt, in1=vb, op=OP.mult)
                nc.vector.tensor_scalar(out=nt, in0=nt, scalar1=-0.5,
                                        scalar2=1.5, op0=OP.mult, op1=OP.add)
                nc.vector.tensor_tensor(out=rss, in0=rss, in1=nt, op=OP.mult)

        # ------------- Phase 2: attention + LN2 (per chunk) -------------
        with (
            tc.tile_pool(name="ph3", bufs=2) as ph3,
            tc.tile_pool(name="h2p", bufs=6) as h2p,
            tc.tile_pool(name="ytp", bufs=2) as ytp,
            tc.tile_pool(name="ps_sc", bufs=2, space="PSUM") as ps_scp,
            tc.tile_pool(name="ps_y", bufs=6, space="PSUM") as ps_yp,
        ):
            wfh_t = None
            h2_prev = None   # chunk sc-1's h2_t tiles; transposed during sc
            e8_list = [None] * NCH

            def score_exp(sc_, st2, e8_t):
                ps_s = ps_scp.tile([P, CH], F, tag="sc", name="ps_se")
                for j in range(DT // 2):
                    nc.tensor.matmul(
                        ps_s,
                        k8[:, 2 * j:2 * j + 2, st2 * P:(st2 + 1) * P],
                        q8[:, 2 * j:2 * j + 2, sc_ * CH:(sc_ + 1) * CH],
                        start=(j == 0), stop=(j == DT // 2 - 1),
                        perf_mode=DR)
                nc.scalar.activation(out=e8_t[:, st2, :], in_=ps_s,
                                     func=AF.Exp, scale=inv_sqrt_d,
                                     bias=nln4_t)

            def emit_transposes(pc, h2_ts):
                for sp in range(TPC):
                    h2_t = h2_ts[sp]
                    ps_tr = ps_yp.tile([P, DT, P], BF, tag="y",
                                       name="ps_tr2")
                    for dt_ in range(DT):
                        nc.tensor.transpose(
                            ps_tr[:, dt_, :],
                            h2_t[:, dt_ * P:(dt_ + 1) * P], ident16)
                    hi = h2s[:, pc, :, 0, sp * P:(sp + 1) * P]
                    nc.scalar.activation(out=hi, in_=ps_tr, func=AF.Copy,
                                         bias=0.0, scale=1.0)
                    nc.vector.tensor_tensor(
                        out=h2s[:, pc, :, 1, sp * P:(sp + 1) * P],
                        in0=ps_tr, in1=hi, op=OP.subtract)

            e8_list[0] = e8_c0
            for sc in range(NCH):
                e8 = e8_list[sc]
                ps_ys = [ps_yp.tile([P, CH], F, tag="y", name=f"ps_y{i}")
                         for i in range(DT)]
                for st2 in range(ST + 2):
                    # st2 0-1 of chunks 1..3 were pre-warmed by the previous
                    # chunk's tail so the first yT pair never waits on exp
                    if 2 <= st2 < ST and sc > 0:
                        score_exp(sc, st2, e8)
                    if st2 >= 2 and st2 % 2 == 0:
                        pr = st2 // 2 - 1
                        t0 = 2 * pr
                        for dtp in range(DT):
                            nc.tensor.matmul(
                                ps_ys[dtp],
                                v8[:, t0:t0 + 2, dtp * P:(dtp + 1) * P],
                                e8[:, t0:t0 + 2, :],
                                start=(pr == 0), stop=(pr == ST // 2 - 1),
                                perf_mode=DR)
                # pre-warm next chunk's exp pipeline
                if sc + 1 < NCH:
                    e8_list[sc + 1] = e8p.tile([P, ST, CH], E4, tag="e8", name="e8n")
                    for st2 in (0, 1):
                        score_exp(sc + 1, st2, e8_list[sc + 1])
                # yT stays unnormalized (values < 240 thanks to the e/8
                # scaling); 1/Z is applied per-token on the x2 write instead,
                # so o-proj never waits on the rz chain. Copies start right
                # at the pair-7 stop and drain during the Z matmuls.
                yT8 = ytp.tile([P, DT, CH], E4, tag="yt")
                for dtp in range(DT):
                    if dtp % 2:
                        nc.scalar.activation(out=yT8[:, dtp], in_=ps_ys[dtp],
                                             func=AF.Copy, bias=0.0, scale=1.0)
                    else:
                        nc.vector.tensor_copy(out=yT8[:, dtp], in_=ps_ys[dtp])
                # Z after the exps, in the scores slot rotation
                ps_z = ps_scp.tile([P, CH], F, tag="sc", name="ps_z")
                for pr in range(ST // 2):
                    nc.tensor.matmul(ps_z, ones8, e8[:, 2 * pr:2 * pr + 2, :],
                                     start=(pr == 0), stop=(pr == ST // 2 - 1),
                                     perf_mode=DR)
                z_row = ph3.tile([1, CH], BF, tag="zrow")
                nc.vector.tensor_copy(out=z_row, in_=ps_z[0:1, :])
                rz = ph3.tile([P, CH], F, tag="rz")
                nc.vector.reciprocal(out=rz, in_=ps_z)
                # transpose rz into per-token columns for the x2 scaling
                ps_rzt = ps_yp.tile([P, TPC, P], F, tag="y", name="ps_rzt")
                for su in range(TPC):
                    nc.tensor.transpose(ps_rzt[:, su, :],
                                        rz[:, su * P:(su + 1) * P], ident32)
                rz_cols = ph3.tile([P, TPC], F, tag="rzc")
                nc.vector.tensor_copy(out=rz_cols, in_=ps_rzt[:, :, 0:1])
                h2_ts = [None] * TPC
                mvs = ph3.tile([P, TPC, 2], F, tag="mvs")
                rss = ph3.tile([P, TPC], F, tag="rss")
                for su in range(TPC):
                    st = sc * TPC + su
                    x_t = ph3.tile([P, D], F, tag="xt3")
                    nc.sync.dma_start(x_t, x_d.ap()[st * P:(st + 1) * P, :])
                    ps_o0 = ps_scp.tile([P, CH], F, tag="sc", name="ps_o0")
                    ps_o1 = ps_yp.tile([P, CH], F, tag="y", name="ps_o1")
                    for ps_o, lo, w in ((ps_o0, 0, 512), (ps_o1, 512, 256)):
                        for j in range(DT // 2):
                            nc.tensor.matmul(
                                ps_o[:, :w],
                                yT8[:, 2 * j:2 * j + 2, su * P:(su + 1) * P],
                                wo_t[:, 2 * j:2 * j + 2, lo:lo + w],
                                start=(j == 0), stop=False, perf_mode=DR)
                        # bo enters as bo*Z so the 1/Z scaling cancels it out
                        nc.tensor.matmul(ps_o[:, :w],
                                         z_row[:, su * P:(su + 1) * P],
                                         bo_row[:, lo:lo + w],
                                         start=False, stop=True)
                    nc.vector.tensor_scalar(out=x2_sb[:, st, :512],
                                            in0=ps_o0,
                                            scalar1=rz_cols[:, su:su + 1],
                                            scalar2=None, op0=OP.mult)
                    nc.gpsimd.tensor_tensor(out=x2_sb[:, st, :512],
                                            in0=x2_sb[:, st, :512],
                                            in1=x_t[:, :512], op=OP.add)
                    nc.vector.tensor_scalar(out=x2_sb[:, st, 512:],
                                            in0=ps_o1[:, :256],
                                            scalar1=rz_cols[:, su:su + 1],
                                            scalar2=None, op0=OP.mult)
                    nc.gpsimd.tensor_tensor(out=x2_sb[:, st, 512:],
                                            in0=x2_sb[:, st, 512:],
                                            in1=x_t[:, 512:], op=OP.add)
                    if sc == 0:
                        # LN2 stats for chunk 0 only; later chunks' LN2 runs
                        # inside the PE-bound MLP phase where engines idle
                        stats = ph3.tile([P, 3, 6], F, tag="st3")
                        for i in range(3):
                            nc.vector.bn_stats(out=stats[:, i, :],
                                               in_=x2_sb[:, st,
                                                         i * 256:(i + 1) * 256])
                        nc.vector.bn_aggr(out=mvs[:, su, :], in_=stats)
                if sc == 0:
                    quake_rsqrt(ph3, mvs, rss)
                    for su in range(TPC):
                        st = sc * TPC + su
                        h2_t = h2p.tile([P, D], BF, tag="h2")
                        nc.gpsimd.tensor_scalar(out=h2_t, in0=x2_sb[:, st, :],
                                                scalar1=mvs[:, su, 0:1],
                                                scalar2=rss[:, su:su + 1],
                                                op0=OP.subtract, op1=OP.mult)
                        h2_ts[su] = h2_t
                    h2_prev = h2_ts
                if sc == 1:
                    emit_transposes(0, h2_prev)
                # prefetch MLP fc hi-weights while attention runs
                if sc == 0:
                    wfh_t = wfcp.tile([P, DT, H], E4)
                    nc.sync.dma_start(
                        wfh_t[:], wfh_d.ap().rearrange("(t p) n -> p t n", p=P))
        e8_ctx.close()
        qkv_ctx.close()

        # ------------- Phase 3: MLP (per chunk) -------------
        wprp = wmlp_ctx.enter_context(tc.tile_pool(name="wprp", bufs=1))
        wfl_t = wprp.tile([P, DT, H], E5)
        for pc in range(3):
            lo, hi = pc * (H // 3), (pc + 1) * (H // 3)
            nc.sync.dma_start(
                wfl_t[:, :, lo:hi],
                wfl_d.ap()[:, lo:hi].rearrange("(t p) n -> p t n", p=P))
        wph_t = wprp.tile([P, HT, D], E4)
        nc.sync.dma_start(wph_t[:],
                          wph_d.ap().rearrange("(t p) n -> p t n", p=P))
        wpl_t = wprp.tile([P, HT, D], E5)
        nc.sync.dma_start(wpl_t[:],
                          wpl_d.ap().rearrange("(t p) n -> p t n", p=P))
        with (
            tc.tile_pool(name="ph5", bufs=3) as ph5,
            tc.tile_pool(name="msp", bufs=1) as msp,
            tc.tile_pool(name="ps_u", bufs=3, space="PSUM") as ps_up,
            tc.tile_pool(name="ps_tr3", bufs=1, space="PSUM") as ps_tr3p,
            tc.tile_pool(name="ps_o2", bufs=2, space="PSUM") as ps_o2p,
        ):
            ms = msp.tile([P, HT, 2, CH], E4)

            def mlp_ln2(pc):
                # LN2 + transposes + hi/lo split for chunk pc, overlapped
                # with the PE-bound fc/proj stream
                mvs3 = ph5.tile([P, TPC, 2], F, tag="mvs3")
                rss3 = ph5.tile([P, TPC], F, tag="rss3")
                for su in range(TPC):
                    st = pc * TPC + su
                    stats = ph5.tile([P, 3, 6], F, tag="st5")
                    for i in range(3):
                        nc.vector.bn_stats(out=stats[:, i, :],
                                           in_=x2_sb[:, st,
                                                     i * 256:(i + 1) * 256])
                    nc.vector.bn_aggr(out=mvs3[:, su, :], in_=stats)
                quake_rsqrt(ph5, mvs3, rss3)
                for su in range(TPC):
                    st = pc * TPC + su
                    h2_t = ph5.tile([P, D], BF, tag="h2m", bufs=4)
                    nc.vector.tensor_scalar(out=h2_t, in0=x2_sb[:, st, :],
                                            scalar1=mvs3[:, su, 0:1],
                                            scalar2=rss3[:, su:su + 1],
                                            op0=OP.subtract, op1=OP.mult)
                    ps_tr = ps_tr3p.tile([P, DT, P], BF, tag="tr3")
                    for dt_ in range(DT):
                        nc.tensor.transpose(ps_tr[:, dt_, :],
                                            h2_t[:, dt_ * P:(dt_ + 1) * P],
                                            ident16)
                    hi = h2s[:, pc, :, 0, su * P:(su + 1) * P]
                    nc.scalar.activation(out=hi, in_=ps_tr, func=AF.Copy,
                                         bias=0.0, scale=1.0)
                    nc.vector.tensor_tensor(
                        out=h2s[:, pc, :, 1, su * P:(su + 1) * P],
                        in0=ps_tr, in1=hi, op=OP.subtract)

            for sc in range(NCH):
                hs = h2s[:, sc]
                for ht in range(HT):
                    ps_u = ps_up.tile([P, CH], F, tag="u")
                    hsl = ht * P
                    for j in range(DT // 2):
                        nc.tensor.matmul(
                            ps_u, wfh_t[:, 2 * j:2 * j + 2, hsl:hsl + P],
                            hs[:, 2 * j:2 * j + 2, 0, :],
                            start=(j == 0), stop=False, perf_mode=DR)
                    for j in range(DT // 2):
                        nc.tensor.matmul(
                            ps_u, wfh_t[:, 2 * j:2 * j + 2, hsl:hsl + P],
                            hs[:, 2 * j:2 * j + 2, 1, :],
                            start=False, stop=False, perf_mode=DR)
                    for j in range(DT // 2):
                        nc.tensor.matmul(
                            ps_u, wfl_t[:, 2 * j:2 * j + 2, hsl:hsl + P],
                            hs[:, 2 * j:2 * j + 2, 0, :],
                            start=False, stop=(j == DT // 2 - 1), perf_mode=DR)
                    m16 = ph5.tile([P, CH], BF, tag="m16")
                    nc.scalar.activation(out=m16, in_=ps_u, func=AF.Gelu,
                                         bias=bfc_col[:, ht:ht + 1], scale=1.0)
                    nc.gpsimd.tensor_copy(out=ms[:, ht, 0, :], in_=m16)
                    nc.gpsimd.tensor_tensor(out=ms[:, ht, 1, :], in0=m16,
                                            in1=ms[:, ht, 0, :],
                                            op=OP.subtract)
                if sc + 1 < NCH:
                    mlp_ln2(sc + 1)
                for su in range(TPC):
                    st = sc * TPC + su
                    ps_o2 = ps_o2p.tile([P, 1024], F, tag="o2")
                    for lo, w in ((0, 512), (512, 256)):
                        for arm in range(3):   # Whi*hi, Whi*lo, Wlo*hi
                            wt = wph_t if arm < 2 else wpl_t
                            mslot = 0 if arm != 1 else 1
                            for j in range(HT // 2):
                                nc.tensor.matmul(
                                    ps_o2[:, lo:lo + w],
                                    ms[:, 2 * j:2 * j + 2, mslot,
                                       su * P:(su + 1) * P],
                                    wt[:, 2 * j:2 * j + 2, lo:lo + w],
                                    start=(arm == 0 and j == 0), stop=False,
                                    perf_mode=DR)
                        nc.tensor.matmul(ps_o2[:, lo:lo + w], ones_row,
                                         bp_row[:, lo:lo + w],
                                         start=False, stop=True)
                    o_t = ph5.tile([P, D], F, tag="ot")
                    nc.vector.tensor_tensor(out=o_t[:, :512],
                                            in0=ps_o2[:, :512],
                                            in1=x2_sb[:, st, :512], op=OP.add)
                    nc.vector.tensor_tensor(out=o_t[:, 512:],
                                            in0=ps_o2[:, 512:768],
                                            in1=x2_sb[:, st, 512:], op=OP.add)
                    nc.sync.dma_start(out_d.ap()[st * P:(st + 1) * P, :], o_t)
        wmlp_ctx.close()

    return nc


def _get_nc():
    if "nc" not in _CACHE:
        nc = _build()
        nc.compile()
        _CACHE["nc"] = nc
    return _CACHE["nc"]


TRACE = False


def kernel(**inputs):
    from concourse.bass_utils import run_bass_kernel_spmd

    nc = _get_nc()
    x = np.asarray(inputs["x"], dtype=np.float32)
    base = _prep(inputs)
    in_maps = [dict(base, x=np.ascontiguousarray(x[b])) for b in range(N_CORES)]
    res = run_bass_kernel_spmd(nc, in_maps, core_ids=list(range(N_CORES)),
                               trace=TRACE)
    _CACHE["last_res"] = res
    return np.stack([res.results[b]["out"] for b in range(N_CORES)], axis=0)



# revision 10
# speedup vs baseline: 1.1777x; 1.1777x over previous
"""Trainium2 Bass kernel for a dense transformer block (B=8, S=2048, D=768, H=3072).

Sharding: pure data-parallel over batch -- one batch element per NeuronCore.

All heavy matmuls are fp8-e4m3 MatmulPerfMode.DoubleRow (K=256/instruction at
0.5 cycles/row). Cost model: matmul time = out_free_size * cycles_per_row, so
total PE time ~ (#contraction passes) x (output width); arms on the MLP
matmuls are the dominant cost.

Numerics (rel_absmax gate 2e-2; emulated 1.5e-2, baseline was 6.1e-3):
  - weights are pre-scaled by powers of two (SW=16 for the D-side mats,
    SP=32 for Wproj) so their entries sit in e4m3's normal range instead of
    half-denormal; the inverse scales ride existing activation scale operands
    (q/k/v/gelu) or the rz-extraction matmul (o-proj) for free.
  - attention path (q/k/v/scores/exp/y/o) is plain e4m3.
  - fc keeps 3 arms: Wfh*(hi+lo) + Wfl*hi (h2 split hi/lo e4m3; Wfl e5m2
    residual). proj runs 2 arms: Wph*(mhi+mlo) -- the Wpl residual arm is
    dropped (WPL_J can partially restore it).
  - exp is computed as exp(s/sqrt(d) - 3ln2) = e/8 so the unnormalized
    attention accumulator stays below e4m3's 240 max; 1/Z (and the 1/SW
    unscale) is applied per-token on the x2 write via a K=1 fp32 matmul that
    extracts rz columns (replaces the old fp32 transposes).
  - when all matmul biases fold to zero (true for this reference: betas and
    biases are zeros) the bias matmuls are omitted entirely at build time.

Schedule: a 3-stage software pipeline over the 4 token chunks, fused across
the old attention/MLP phase boundary so the MLP's PE-dense stream fills the
stalls of the attention dependency chains:
  slot c emits: attnV/Z/o-proj/LN2 of chunk c-1, fc+proj of chunk c-2, and
  scores+exp of chunk c (last, so the ACT engine does gelus first and exps
  at the end -- Exp and Gelu live in different activation-table sets, so this
  ordering costs only 2 table loads per slot).
x2 and h2s are rolling 2-chunk buffers; ms single-chunk. PSUM: 3 banks for
scores/Z/LN2-transposes, 2 for attnV/o-proj, 2 for fc/proj, 1 for rz.
"""

import numpy as np

P = 128
S, D, H = 2048, 768, 3072
DT = D // P            # 6 d-tiles
HT = H // P            # 24 h-tiles
ST = S // P            # 16 token tiles
CH = 512               # chunk width (tokens)
NCH = S // CH          # 4 chunks
TPC = CH // P          # 4 token tiles per chunk
EPS = 1e-5
N_CORES = 8
LN4 = 2.0794415416798357   # 3*ln2; exp bias so e8 = exp(s)/8
SW = 16.0                  # scale for D-side weight mats (sigma ~0.036)
SP = 32.0                  # scale for Wproj (sigma ~0.018)
WPL_J = 0                  # 0..12: partial Wpl residual passes (accuracy knob)

WEIGHT_NAMES = [
    "ln1_g", "ln1_b", "ln2_g", "ln2_b",
    "Wq", "bq", "Wk", "bk", "Wv", "bv", "Wo", "bo",
    "Wfc", "bfc", "Wproj", "bproj",
]

_CACHE = {}


def _prep(inputs):
    """Host-side weight quantization + LN/bias folding (pure numpy)."""
    import ml_dtypes
    E4, E5, BF = ml_dtypes.float8_e4m3, ml_dtypes.float8_e5m2, ml_dtypes.bfloat16
    f32 = lambda k: np.asarray(inputs[k], dtype=np.float32)
    g1, b1 = f32("ln1_g"), f32("ln1_b")
    g2, b2 = f32("ln2_g"), f32("ln2_b")
    Wq, Wk, Wv, Wo = f32("Wq"), f32("Wk"), f32("Wv"), f32("Wo")
    Wfc, Wproj = f32("Wfc"), f32("Wproj")
    q8 = lambda a: np.ascontiguousarray(a.astype(E4))
    Wfc_g = g2[:, None] * Wfc * SW
    wfh = Wfc_g.astype(E4)
    wph_f = Wproj * SP
    wph = wph_f.astype(E4)
    bo_f = f32("bo") + (f32("bv") + b1 @ Wv) @ Wo
    bp_f = f32("bproj")
    out = {
        "wq8": q8(SW * g1[:, None] * Wq), "wk8": q8(SW * g1[:, None] * Wk),
        "wv8": q8(SW * g1[:, None] * Wv), "wo8": q8(SW * Wo),
        "bq_": f32("bq") + b1 @ Wq, "bk_": f32("bk") + b1 @ Wk,
        "wfh": np.ascontiguousarray(wfh),
        "wfl": np.ascontiguousarray(
            (Wfc_g - wfh.astype(np.float32)).astype(E5)),
        "wph": np.ascontiguousarray(wph),
        "wpl": np.ascontiguousarray(
            (wph_f - wph.astype(np.float32)).astype(E5)),
        "bfc_": f32("bfc") + b2 @ Wfc,
        "bo16": np.asarray(SW * bo_f, dtype=BF),
        "bp16": np.asarray(SP * bp_f, dtype=BF),
    }
    out["_has_bias"] = bool(np.any(bo_f != 0.0) or np.any(bp_f != 0.0))
    return out


def _build(has_bias, wpl_j):
    import concourse.bass as bass
    import concourse.tile as tile
    from concourse import bacc, mybir
    from concourse.masks import make_identity
    from contextlib import ExitStack

    F = mybir.dt.float32
    BF = mybir.dt.bfloat16
    E4 = mybir.dt.float8e4
    E5 = mybir.dt.float8e5
    I32 = mybir.dt.int32
    AF = mybir.ActivationFunctionType
    OP = mybir.AluOpType
    DR = mybir.MatmulPerfMode.DoubleRow

    nc = bacc.Bacc(None, target_bir_lowering=False)

    x_d = nc.dram_tensor("x", [S, D], F, kind="ExternalInput")
    out_d = nc.dram_tensor("out", [S, D], F, kind="ExternalOutput")
    wq8_d = nc.dram_tensor("wq8", [D, D], E4, kind="ExternalInput")
    wk8_d = nc.dram_tensor("wk8", [D, D], E4, kind="ExternalInput")
    wv8_d = nc.dram_tensor("wv8", [D, D], E4, kind="ExternalInput")
    wo8_d = nc.dram_tensor("wo8", [D, D], E4, kind="ExternalInput")
    wfh_d = nc.dram_tensor("wfh", [D, H], E4, kind="ExternalInput")
    wfl_d = nc.dram_tensor("wfl", [D, H], E5, kind="ExternalInput")
    wph_d = nc.dram_tensor("wph", [H, D], E4, kind="ExternalInput")
    bq_d = nc.dram_tensor("bq_", [D], F, kind="ExternalInput")
    bk_d = nc.dram_tensor("bk_", [D], F, kind="ExternalInput")
    bfc_d = nc.dram_tensor("bfc_", [H], F, kind="ExternalInput")
    if wpl_j:
        wpl_d = nc.dram_tensor("wpl", [H, D], E5, kind="ExternalInput")
    if has_bias:
        bo_d = nc.dram_tensor("bo16", [D], BF, kind="ExternalInput")
        bp_d = nc.dram_tensor("bp16", [D], BF, kind="ExternalInput")

    inv_sqrt_d = 1.0 / float(np.sqrt(np.float32(D)))

    with tile.TileContext(nc) as tc, ExitStack() as ctx:
        singles = ctx.enter_context(tc.tile_pool(name="singles", bufs=1))

        ident16 = singles.tile([P, P], BF)
        make_identity(nc, ident16)
        nln4_t = singles.tile([P, 1], F)
        nc.vector.memset(nln4_t, -LN4)
        invsw = singles.tile([1, 1], F)
        nc.vector.memset(invsw, 1.0 / SW)
        ones8 = singles.tile([P, 2, P], E4)
        nc.vector.memset(ones8, 1.0)
        bq_col = singles.tile([P, DT], F)
        bk_col = singles.tile([P, DT], F)
        bfc_col = singles.tile([P, HT], F)
        if has_bias:
            ones_row = singles.tile([1, P], BF)
            nc.vector.memset(ones_row, 1.0)
            bo_row = singles.tile([1, D], BF)
            bp_row = singles.tile([1, D], BF)

        # persistent activations
        perm = ctx.enter_context(tc.tile_pool(name="perm", bufs=1))
        x2_sb = perm.tile([P, 2 * TPC, D], F)       # rolling 2-chunk residual
        h2s = perm.tile([P, 2, DT, 2, CH], E4)      # rolling LN2 out hi/lo
        ms = perm.tile([P, HT, 2, CH], E4)          # gelu out hi/lo (1 chunk)

        wfhp = ctx.enter_context(tc.tile_pool(name="wfhp", bufs=1))
        wfh_t = wfhp.tile([P, DT, H], E4)
        wflp = ctx.enter_context(tc.tile_pool(name="wflp", bufs=1))
        wfl_t = wflp.tile([P, DT, H], E5)

        qkv_ctx = ExitStack()
        qkvp = qkv_ctx.enter_context(tc.tile_pool(name="qkv", bufs=1))
        k8 = qkvp.tile([P, DT, S], E4)
        q8 = qkvp.tile([P, DT, S], E4)
        v8 = qkvp.tile([P, ST, D], E4)
        wo_t = qkvp.tile([P, DT, D], E4)

        e8_ctx = ExitStack()
        e8p = e8_ctx.enter_context(tc.tile_pool(name="e8p", bufs=2))
        e8_list = [None] * NCH

        wqkv_ctx = ExitStack()
        wqkv = wqkv_ctx.enter_context(tc.tile_pool(name="wqkv", bufs=1))
        wv_t = wqkv.tile([P, DT, D], E4)
        wk_t = wqkv.tile([P, DT, D], E4)
        wq_t = wqkv.tile([P, DT, D], E4)

        hT_ctx = ExitStack()
        hTp = hT_ctx.enter_context(tc.tile_pool(name="hT", bufs=1))
        hT8 = hTp.tile([P, DT, S], E4)

        # ------------- Phase 1: LN1 -> hT8; v, k, q (all fp8) -------------
        with (
            tc.tile_pool(name="ph1", bufs=3) as ph1,
            tc.tile_pool(name="ps_tr", bufs=2, space="PSUM") as ps_trp,
            tc.tile_pool(name="ps_k", bufs=3, space="PSUM") as ps_kp,
            tc.tile_pool(name="ps_v", bufs=3, space="PSUM") as ps_vp,
        ):
            x_ts = [None] * ST
            h_ts = [None] * ST
            Q0 = [nc.sync, nc.gpsimd, nc.scalar, nc.gpsimd]

            def ln1_group(g):
                mvg = ph1.tile([P, TPC, 2], F, tag="mvg")
                for i in range(TPC):
                    st = TPC * g + i
                    x_t = ph1.tile([P, D], F, tag="xt", bufs=6)
                    if g == 0:
                        # spread the first group across 4 queues so tile 3
                        # lands ~4.4us in, not ~9us
                        Q0[i].dma_start(out=x_t,
                                        in_=x_d.ap()[st * P:(st + 1) * P, :])
                    elif st % 2:
                        nc.gpsimd.dma_start(
                            out=x_t, in_=x_d.ap()[st * P:(st + 1) * P, :])
                    else:
                        nc.sync.dma_start(x_t,
                                          x_d.ap()[st * P:(st + 1) * P, :])
                    # weight prefetches ride the sync queue behind the x tiles
                    if st == 1:
                        nc.sync.dma_start(
                            wv_t[:],
                            wv8_d.ap().rearrange("(t p) n -> p t n", p=P))
                    if st == 2:
                        nc.sync.dma_start(
                            wk_t[:],
                            wk8_d.ap().rearrange("(t p) n -> p t n", p=P))
                        nc.sync.dma_start(
                            bk_col, bk_d.ap().rearrange("(t p) -> p t", p=P))
                        nc.sync.dma_start(
                            bq_col, bq_d.ap().rearrange("(t p) -> p t", p=P))
                    if st == 3:
                        nc.sync.dma_start(
                            wq_t[:],
                            wq8_d.ap().rearrange("(t p) n -> p t n", p=P))
                    if st == 5:
                        nc.sync.dma_start(
                            wo_t[:],
                            wo8_d.ap().rearrange("(t p) n -> p t n", p=P))
                        nc.sync.dma_start(
                            bfc_col, bfc_d.ap().rearrange("(t p) -> p t", p=P))
                    if st == 7:
                        nc.sync.dma_start(
                            wfh_t[:],
                            wfh_d.ap().rearrange("(t p) n -> p t n", p=P))
                    if st == 11:
                        nc.sync.dma_start(
                            wfl_t[:],
                            wfl_d.ap().rearrange("(t p) n -> p t n", p=P))
                    x_ts[st] = x_t
                    stats = ph1.tile([P, 3, 6], F, tag="st")
                    for j in range(3):
                        nc.vector.bn_stats(out=stats[:, j, :],
                                           in_=x_t[:, j * 256:(j + 1) * 256])
                    nc.vector.bn_aggr(out=mvg[:, i, :], in_=stats)
                # batched rsqrt(var+eps): quake bit-trick + 1 Newton (DVE)
                rsg = ph1.tile([P, TPC], F, tag="rsg")
                nc.vector.tensor_scalar(out=rsg, in0=mvg[:, :, 1], scalar1=EPS,
                                        scalar2=None, op0=OP.add)
                rig = ph1.tile([P, TPC], I32, tag="rig")
                nc.vector.tensor_scalar(out=rig, in0=rsg[:].bitcast(I32),
                                        scalar1=1, scalar2=None,
                                        op0=OP.logical_shift_right)
                nc.vector.tensor_scalar(out=rig, in0=rig, scalar1=-1,
                                        scalar2=None, op0=OP.bitwise_xor)
                nc.vector.tensor_scalar(out=rig, in0=rig, scalar1=0x5f3759e0,
                                        scalar2=None, op0=OP.add)
                rng = ph1.tile([P, TPC], F, tag="rng")
                nc.vector.tensor_tensor(out=rng, in0=rig[:].bitcast(F),
                                        in1=rig[:].bitcast(F), op=OP.mult)
                nc.vector.tensor_tensor(out=rng, in0=rng, in1=rsg, op=OP.mult)
                nc.vector.tensor_scalar(out=rng, in0=rng, scalar1=-0.5,
                                        scalar2=1.5, op0=OP.mult, op1=OP.add)
                nc.vector.tensor_tensor(out=rsg, in0=rig[:].bitcast(F),
                                        in1=rng, op=OP.mult)
                for i in range(TPC):
                    st = TPC * g + i
                    h_t = ph1.tile([P, D], BF, tag="ht", bufs=6)
                    nc.gpsimd.tensor_scalar(out=h_t, in0=x_ts[st],
                                            scalar1=mvg[:, i, 0:1],
                                            scalar2=rsg[:, i:i + 1],
                                            op0=OP.subtract, op1=OP.mult)
                    h_ts[st] = h_t

            def consume_group(pg):
                for i in range(TPC):
                    sp = TPC * pg + i
                    h_t = h_ts[sp]
                    ps_tr = ps_trp.tile([P, DT, P], BF, tag="tr")
                    for dt_ in range(DT):
                        nc.tensor.transpose(ps_tr[:, dt_, :],
                                            h_t[:, dt_ * P:(dt_ + 1) * P],
                                            ident16)
                    nc.scalar.activation(
                        out=hT8[:, :, sp * P:(sp + 1) * P],
                        in_=ps_tr, func=AF.Copy, bias=0.0, scale=1.0)
                for i in range(TPC):
                    sv = TPC * pg + i
                    for lo, w, eng in ((0, 512, 0), (512, 256, 1)):
                        ps_v = ps_vp.tile([P, CH], F, tag="v")
                        for j in range(DT // 2):
                            nc.tensor.matmul(
                                ps_v[:, :w],
                                hT8[:, 2 * j:2 * j + 2, sv * P:(sv + 1) * P],
                                wv_t[:, 2 * j:2 * j + 2, lo:lo + w],
                                start=(j == 0), stop=(j == DT // 2 - 1),
                                perf_mode=DR)
                        # bv folds into bo' host-side (bo16)
                        if eng:
                            nc.scalar.activation(
                                out=v8[:, sv, lo:lo + w], in_=ps_v[:, :w],
                                func=AF.Identity, bias=0.0, scale=1.0 / SW)
                        else:
                            nc.vector.tensor_scalar(
                                out=v8[:, sv, lo:lo + w], in0=ps_v[:, :w],
                                scalar1=1.0 / SW, scalar2=None, op0=OP.mult)
                kc = pg
                for dtp in range(DT):
                    ps_k = ps_kp.tile([P, CH], F, tag="k")
                    for j in range(DT // 2):
                        nc.tensor.matmul(
                            ps_k,
                            wk_t[:, 2 * j:2 * j + 2, dtp * P:(dtp + 1) * P],
                            hT8[:, 2 * j:2 * j + 2, kc * CH:(kc + 1) * CH],
                            start=(j == 0), stop=(j == DT // 2 - 1),
                            perf_mode=DR)
                    if dtp % 2:
                        nc.scalar.activation(
                            out=k8[:, dtp, kc * CH:(kc + 1) * CH], in_=ps_k,
                            func=AF.Identity, bias=bk_col[:, dtp:dtp + 1],
                            scale=1.0 / SW)
                    else:
                        nc.vector.tensor_scalar(
                            out=k8[:, dtp, kc * CH:(kc + 1) * CH], in0=ps_k,
                            scalar1=1.0 / SW, scalar2=bk_col[:, dtp:dtp + 1],
                            op0=OP.mult, op1=OP.add)
                if kc == 0:
                    e8_list[0] = e8p.tile([P, ST, CH], E4, tag="e8",
                                          name="e8_c0")
                    for dtp in range(DT):
                        ps_q = ps_kp.tile([P, CH], F, tag="k", name="ps_q0")
                        for j in range(DT // 2):
                            nc.tensor.matmul(
                                ps_q,
                                wq_t[:, 2 * j:2 * j + 2,
                                     dtp * P:(dtp + 1) * P],
                                hT8[:, 2 * j:2 * j + 2, 0:CH],
                                start=(j == 0), stop=(j == DT // 2 - 1),
                                perf_mode=DR)
                        if dtp % 2:
                            nc.scalar.activation(
                                out=q8[:, dtp, 0:CH], in_=ps_q,
                                func=AF.Identity,
                                bias=bq_col[:, dtp:dtp + 1], scale=1.0 / SW)
                        else:
                            nc.vector.tensor_scalar(
                                out=q8[:, dtp, 0:CH], in0=ps_q,
                                scalar1=1.0 / SW,
                                scalar2=bq_col[:, dtp:dtp + 1],
                                op0=OP.mult, op1=OP.add)
                # chunk-0 scores for this k-chunk's keys
                for st2 in range(TPC * kc, TPC * kc + TPC):
                    ps_s = ps_kp.tile([P, CH], F, tag="k", name="ps_s0")
                    for j in range(DT // 2):
                        nc.tensor.matmul(
                            ps_s,
                            k8[:, 2 * j:2 * j + 2, st2 * P:(st2 + 1) * P],
                            q8[:, 2 * j:2 * j + 2, 0:CH],
                            start=(j == 0), stop=(j == DT // 2 - 1),
                            perf_mode=DR)
                    nc.scalar.activation(out=e8_list[0][:, st2, :], in_=ps_s,
                                         func=AF.Exp, scale=inv_sqrt_d,
                                         bias=nln4_t)

            for g in range(TPC + 1):
                if g >= 1:
                    consume_group(g - 1)
                if g < TPC:
                    ln1_group(g)
            # q for chunks 1..3 (frees hT afterwards)
            for sc in range(1, NCH):
                for dtp in range(DT):
                    ps_q = ps_kp.tile([P, CH], F, tag="k")
                    for j in range(DT // 2):
                        nc.tensor.matmul(
                            ps_q,
                            wq_t[:, 2 * j:2 * j + 2, dtp * P:(dtp + 1) * P],
                            hT8[:, 2 * j:2 * j + 2, sc * CH:(sc + 1) * CH],
                            start=(j == 0), stop=(j == DT // 2 - 1),
                            perf_mode=DR)
                    if dtp % 2:
                        nc.scalar.activation(
                            out=q8[:, dtp, sc * CH:(sc + 1) * CH], in_=ps_q,
                            func=AF.Identity, bias=bq_col[:, dtp:dtp + 1],
                            scale=1.0 / SW)
                    else:
                        nc.vector.tensor_scalar(
                            out=q8[:, dtp, sc * CH:(sc + 1) * CH], in0=ps_q,
                            scalar1=1.0 / SW, scalar2=bq_col[:, dtp:dtp + 1],
                            op0=OP.mult, op1=OP.add)
        hT_ctx.close()
        wqkv_ctx.close()

        # MLP proj weights arrive into the space hT8/wq/wk/wv vacated
        wphp_ctx = ExitStack()
        wphp = wphp_ctx.enter_context(tc.tile_pool(name="wphp", bufs=1))
        wph_t = wphp.tile([P, HT, D], E4)
        nc.sync.dma_start(wph_t[:],
                          wph_d.ap().rearrange("(t p) n -> p t n", p=P))
        if wpl_j:
            wpl_t = wphp.tile([P, 2 * wpl_j, D], E5)
            nc.sync.dma_start(
                wpl_t[:],
                wpl_d.ap()[0:2 * wpl_j * P, :].rearrange(
                    "(t p) n -> p t n", p=P))
        if has_bias:
            nc.sync.dma_start(bo_row, bo_d.ap().unsqueeze(0))
            nc.sync.dma_start(bp_row, bp_d.ap().unsqueeze(0))

        def quake_rsqrt(pool, mvs, rss):
            # rsqrt(var+eps): quake bit-trick + 2 Newton steps, all on DVE
            vb = pool.tile([P, TPC], F, tag="vb")
            nc.vector.tensor_scalar(out=vb, in0=mvs[:, :, 1], scalar1=EPS,
                                    scalar2=None, op0=OP.add)
            ib = pool.tile([P, TPC], I32, tag="ib")
            nc.vector.tensor_scalar(out=ib, in0=vb[:].bitcast(I32),
                                    scalar1=1, scalar2=None,
                                    op0=OP.logical_shift_right)
            nc.vector.tensor_scalar(out=ib, in0=ib, scalar1=-1,
                                    scalar2=None, op0=OP.bitwise_xor)
            nc.vector.tensor_scalar(out=ib, in0=ib, scalar1=0x5f3759e0,
                                    scalar2=None, op0=OP.add)
            nc.vector.tensor_copy(out=rss, in_=ib[:].bitcast(F))
            nt = pool.tile([P, TPC], F, tag="nt")
            for _ in range(2):
                nc.vector.tensor_tensor(out=nt, in0=rss, in1=rss, op=OP.mult)
                nc.vector.tensor_tensor(out=nt, in0=nt, in1=vb, op=OP.mult)
                nc.vector.tensor_scalar(out=nt, in0=nt, scalar1=-0.5,
                                        scalar2=1.5, op0=OP.mult, op1=OP.add)
                nc.vector.tensor_tensor(out=rss, in0=rss, in1=nt, op=OP.mult)

        # ------------- Fused pipeline: slots c = 1..5 -------------
        # slot c: back(c-1) attnV/Z/o/LN2 + mlp(c-2) fc/proj + front(c) scores
        with (
            tc.tile_pool(name="pp", bufs=2) as pp,
            tc.tile_pool(name="ps_sc", bufs=3, space="PSUM") as ps_scp,
            tc.tile_pool(name="ps_av", bufs=2, space="PSUM") as ps_avp,
            tc.tile_pool(name="ps_fp", bufs=2, space="PSUM") as ps_fpp,
            tc.tile_pool(name="ps_rz", bufs=1, space="PSUM") as ps_rzp,
        ):
            for c in range(1, 6):
                b = c - 1 if c - 1 <= 3 else None      # back chunk
                m = c - 2 if 0 <= c - 2 <= 3 else None  # mlp chunk
                have_front = c <= 3

                if have_front:
                    e8_list[c] = e8p.tile([P, ST, CH], E4, tag="e8",
                                          name=f"e8_{c}")

                # ---- mlp(m) quanta ----
                fcq = []
                if m is not None:
                    hs = h2s[:, m % 2]

                    def fc_ht(ht, hs=hs):
                        ps = ps_fpp.tile([P, CH], F, tag="fp", name="u")
                        for j in range(DT // 2):
                            nc.tensor.matmul(
                                ps, wfh_t[:, 2 * j:2 * j + 2,
                                          ht * P:(ht + 1) * P],
                                hs[:, 2 * j:2 * j + 2, 0, :],
                                start=(j == 0), stop=False, perf_mode=DR)
                        for j in range(DT // 2):
                            nc.tensor.matmul(
                                ps, wfh_t[:, 2 * j:2 * j + 2,
                                          ht * P:(ht + 1) * P],
                                hs[:, 2 * j:2 * j + 2, 1, :],
                                start=False, stop=False, perf_mode=DR)
                        for j in range(DT // 2):
                            nc.tensor.matmul(
                                ps, wfl_t[:, 2 * j:2 * j + 2,
                                          ht * P:(ht + 1) * P],
                                hs[:, 2 * j:2 * j + 2, 0, :],
                                start=False, stop=(j == DT // 2 - 1),
                                perf_mode=DR)
                        m16 = pp.tile([P, CH], BF, tag="m16", bufs=3)
                        nc.scalar.activation(out=m16, in_=ps, func=AF.Gelu,
                                             bias=bfc_col[:, ht:ht + 1],
                                             scale=1.0 / SW)
                        nc.gpsimd.tensor_copy(out=ms[:, ht, 0, :], in_=m16)
                        nc.gpsimd.tensor_tensor(out=ms[:, ht, 1, :], in0=m16,
                                                in1=ms[:, ht, 0, :],
                                                op=OP.subtract)

                    fcq = [(lambda ht=ht: fc_ht(ht)) for ht in range(HT)]

                fci = [0]

                def emit_fc(n):
                    for _ in range(n):
                        if fci[0] < len(fcq):
                            fcq[fci[0]]()
                            fci[0] += 1

                def proj_su(su, seg, o_ts={}):
                    lo, w = (0, 512) if seg == 0 else (512, 256)
                    ps = ps_fpp.tile([P, CH], F, tag="fp", name="pj")
                    last = ('b' if has_bias else
                            ('l' if wpl_j else 'a'))
                    for arm in range(2):
                        for j in range(HT // 2):
                            isl = (last == 'a' and arm == 1
                                   and j == HT // 2 - 1)
                            nc.tensor.matmul(
                                ps[:, :w],
                                ms[:, 2 * j:2 * j + 2, arm,
                                   su * P:(su + 1) * P],
                                wph_t[:, 2 * j:2 * j + 2, lo:lo + w],
                                start=(arm == 0 and j == 0), stop=isl,
                                perf_mode=DR)
                    if wpl_j:
                        for j in range(wpl_j):
                            isl = (last == 'l' and j == wpl_j - 1)
                            nc.tensor.matmul(
                                ps[:, :w],
                                ms[:, 2 * j:2 * j + 2, 0,
                                   su * P:(su + 1) * P],
                                wpl_t[:, 2 * j:2 * j + 2, lo:lo + w],
                                start=False, stop=isl, perf_mode=DR)
                    if has_bias:
                        nc.tensor.matmul(ps[:, :w], ones_row,
                                         bp_row[:, lo:lo + w],
                                         start=False, stop=True)
                    st_sl = (m % 2) * TPC + su
                    if seg == 0:
                        o_ts[su] = pp.tile([P, D], F, tag="ot", bufs=2,
                                           name="o_t")
                    o_t = o_ts[su]
                    nc.scalar.activation(out=o_t[:, lo:lo + w],
                                         in_=ps[:, :w], func=AF.Identity,
                                         bias=0.0, scale=1.0 / SP)
                    nc.gpsimd.tensor_tensor(out=o_t[:, lo:lo + w],
                                            in0=o_t[:, lo:lo + w],
                                            in1=x2_sb[:, st_sl, lo:lo + w],
                                            op=OP.add)
                    if seg == 1:
                        st = m * TPC + su
                        nc.sync.dma_start(
                            out_d.ap()[st * P:(st + 1) * P, :], o_t)

                # ---- back(b) helpers ----
                if b is not None:
                    e8b = e8_list[b]
                    xb_ts = []
                    for su in range(TPC):
                        st = b * TPC + su
                        x_t = pp.tile([P, D], F, tag="xb", bufs=4)
                        nc.sync.dma_start(x_t,
                                          x_d.ap()[st * P:(st + 1) * P, :])
                        xb_ts.append(x_t)
                    yT8 = pp.tile([P, DT, CH], E4, tag="yt", bufs=2)
                    mvs = pp.tile([P, TPC, 2], F, tag="mvs")
                    rss = pp.tile([P, TPC], F, tag="rss")
                    rz_cols = pp.tile([P, TPC], F, tag="rzc")

                    def back_attnv(dtp):
                        ps = ps_avp.tile([P, CH], F, tag="av", name="av")
                        for pr in range(ST // 2):
                            nc.tensor.matmul(
                                ps,
                                v8[:, 2 * pr:2 * pr + 2,
                                   dtp * P:(dtp + 1) * P],
                                e8b[:, 2 * pr:2 * pr + 2, :],
                                start=(pr == 0), stop=(pr == ST // 2 - 1),
                                perf_mode=DR)
                        if dtp % 2:
                            nc.scalar.activation(out=yT8[:, dtp, :], in_=ps,
                                                 func=AF.Copy, bias=0.0,
                                                 scale=1.0)
                        else:
                            nc.vector.tensor_copy(out=yT8[:, dtp, :], in_=ps)

                    def back_z_rz():
                        ps_z = ps_scp.tile([P, CH], F, tag="sc", name="z")
                        for pr in range(ST // 2):
                            nc.tensor.matmul(ps_z, ones8,
                                             e8b[:, 2 * pr:2 * pr + 2, :],
                                             start=(pr == 0),
                                             stop=(pr == ST // 2 - 1),
                                             perf_mode=DR)
                        if has_bias:
                            z_row = pp.tile([1, CH], BF, tag="zrow")
                            nc.vector.tensor_copy(out=z_row, in_=ps_z[0:1, :])
                        else:
                            z_row = None
                        rz = pp.tile([P, CH], F, tag="rz", bufs=1)
                        nc.vector.reciprocal(out=rz, in_=ps_z)
                        ps_rz = ps_rzp.tile([P, TPC], F, tag="rz")
                        for su in range(TPC):
                            # K=1 fp32 matmul: broadcast rz row -> per-token
                            # column, pre-divided by SW (invsw operand)
                            nc.tensor.matmul(
                                ps_rz[:, su:su + 1],
                                rz[0:1, su * P:(su + 1) * P],
                                invsw, start=True, stop=True)
                        nc.vector.tensor_copy(out=rz_cols, in_=ps_rz)
                        return z_row

                    def back_o(su, z_row):
                        st_sl = (b % 2) * TPC + su
                        for lo, w in ((0, 512), (512, 256)):
                            ps = ps_avp.tile([P, CH], F, tag="av", name="o")
                            for j in range(DT // 2):
                                nc.tensor.matmul(
                                    ps[:, :w],
                                    yT8[:, 2 * j:2 * j + 2,
                                        su * P:(su + 1) * P],
                                    wo_t[:, 2 * j:2 * j + 2, lo:lo + w],
                                    start=(j == 0),
                                    stop=(j == DT // 2 - 1 and
                                          not has_bias),
                                    perf_mode=DR)
                            if has_bias:
                                # bo rides as bo*Z*SW; the rz/SW scaling
                                # cancels it back to +bo
                                nc.tensor.matmul(ps[:, :w],
                                                 z_row[:, su * P:(su + 1) * P],
                                                 bo_row[:, lo:lo + w],
                                                 start=False, stop=True)
                            nc.vector.tensor_scalar(
                                out=x2_sb[:, st_sl, lo:lo + w],
                                in0=ps[:, :w],
                                scalar1=rz_cols[:, su:su + 1],
                                scalar2=None, op0=OP.mult)
                            nc.gpsimd.tensor_tensor(
                                out=x2_sb[:, st_sl, lo:lo + w],
                                in0=x2_sb[:, st_sl, lo:lo + w],
                                in1=xb_ts[su][:, lo:lo + w], op=OP.add)
                        stats = pp.tile([P, 3, 6], F, tag="st3")
                        for i in range(3):
                            nc.vector.bn_stats(
                                out=stats[:, i, :],
                                in_=x2_sb[:, st_sl, i * 256:(i + 1) * 256])
                        nc.vector.bn_aggr(out=mvs[:, su, :], in_=stats)

                    def back_ln2(su):
                        st_sl = (b % 2) * TPC + su
                        h2_t = pp.tile([P, D], BF, tag="h2", bufs=3)
                        nc.vector.tensor_scalar(out=h2_t,
                                                in0=x2_sb[:, st_sl, :],
                                                scalar1=mvs[:, su, 0:1],
                                                scalar2=rss[:, su:su + 1],
                                                op0=OP.subtract, op1=OP.mult)
                        ps_tr = ps_scp.tile([P, DT, P], BF, tag="sc",
                                            name="tr2")
                        for dt_ in range(DT):
                            nc.tensor.transpose(
                                ps_tr[:, dt_, :],
                                h2_t[:, dt_ * P:(dt_ + 1) * P], ident16)
                        hi = h2s[:, b % 2, :, 0, su * P:(su + 1) * P]
                        nc.scalar.activation(out=hi, in_=ps_tr, func=AF.Copy,
                                             bias=0.0, scale=1.0)
                        nc.vector.tensor_tensor(
                            out=h2s[:, b % 2, :, 1, su * P:(su + 1) * P],
                            in0=ps_tr, in1=hi, op=OP.subtract)

                def front_score(st2):
                    ps = ps_scp.tile([P, CH], F, tag="sc", name="s")
                    for j in range(DT // 2):
                        nc.tensor.matmul(
                            ps,
                            k8[:, 2 * j:2 * j + 2, st2 * P:(st2 + 1) * P],
                            q8[:, 2 * j:2 * j + 2, c * CH:(c + 1) * CH],
                            start=(j == 0), stop=(j == DT // 2 - 1),
                            perf_mode=DR)
                    nc.scalar.activation(out=e8_list[c][:, st2, :], in_=ps,
                                         func=AF.Exp, scale=inv_sqrt_d,
                                         bias=nln4_t)

                # ---- slot emission ----
                if b is not None:
                    # stage 1: attnV woven with fc
                    for dtp in range(DT):
                        back_attnv(dtp)
                        emit_fc(1)
                    emit_fc(2)
                    # stage 2: Z + rz, more fc
                    z_row = back_z_rz()
                    emit_fc(4)
                    # stage 3: o-proj + x2 + stats, more fc
                    for su in range(TPC):
                        back_o(su, z_row)
                        emit_fc(2)
                    # stage 4: LN2 apply + transposes + h2s
                    quake_rsqrt(pp, mvs, rss)
                    for su in range(TPC):
                        back_ln2(su)
                        emit_fc(1)
                emit_fc(HT)  # any leftovers (and the c=5 no-back slot)
                # stage 5: proj woven with next chunk's scores+exps
                sci = 0
                if m is not None:
                    for su in range(TPC):
                        for seg in range(2):
                            proj_su(su, seg)
                            if have_front:
                                for _ in range(2):
                                    if sci < ST:
                                        front_score(sci)
                                        sci += 1
                if have_front:
                    while sci < ST:
                        front_score(sci)
                        sci += 1
        wphp_ctx.close()
        e8_ctx.close()
        qkv_ctx.close()

    return nc


def _get_nc():
    key = _CACHE.get("key")
    if "nc" not in _CACHE or key != (_CACHE.get("has_bias"), WPL_J):
        has_bias = _CACHE.get("has_bias", False)
        nc = _build(has_bias, WPL_J)
        nc.compile()
        _CACHE["nc"] = nc
        _CACHE["key"] = (has_bias, WPL_J)
    return _CACHE["nc"]


TRACE = False


def kernel(**inputs):
    from concourse.bass_utils import run_bass_kernel_spmd

    x = np.asarray(inputs["x"], dtype=np.float32)
    base = _prep(inputs)
    _CACHE["has_bias"] = base.pop("_has_bias")
    nc = _get_nc()
    names = {"wq8", "wk8", "wv8", "wo8", "wfh", "wfl", "wph",
             "bq_", "bk_", "bfc_"}
    if WPL_J:
        names.add("wpl")
    if _CACHE["has_bias"]:
        names.add("bo16")
        names.add("bp16")
    ship = {k: v for k, v in base.items() if k in names}
    in_maps = [dict(ship, x=np.ascontiguousarray(x[bb]))
               for bb in range(N_CORES)]
    res = run_bass_kernel_spmd(nc, in_maps, core_ids=list(range(N_CORES)),
                               trace=TRACE)
    _CACHE["last_res"] = res
    return np.stack([res.results[bb]["out"] for bb in range(N_CORES)], axis=0)


# revision 16
# speedup vs baseline: 1.1995x; 1.0185x over previous
"""Trainium2 Bass kernel for a dense transformer block (B=8, S=2048, D=768, H=3072).

Sharding: pure data-parallel over batch -- one batch element per NeuronCore.

All heavy matmuls are fp8-e4m3 MatmulPerfMode.DoubleRow (K=256/instruction at
0.5 cycles/row). Cost model: matmul time = out_free_size * cycles_per_row, so
total PE time ~ (#contraction passes) x (output width); arms on the MLP
matmuls are the dominant cost.

Numerics (rel_absmax gate 2e-2; emulated 1.5e-2, baseline was 6.1e-3):
  - weights are pre-scaled by powers of two (SW=16 for the D-side mats,
    SP=32 for Wproj) so their entries sit in e4m3's normal range instead of
    half-denormal; the inverse scales ride existing activation scale operands
    (q/k/v/gelu) or the rz-extraction matmul (o-proj) for free.
  - attention path (q/k/v/scores/exp/y/o) is plain e4m3.
  - fc keeps 3 arms: Wfh*(hi+lo) + Wfl*hi (h2 split hi/lo e4m3; Wfl e5m2
    residual). proj runs 2 arms: Wph*(mhi+mlo) -- the Wpl residual arm is
    dropped (WPL_J can partially restore it).
  - exp is computed as exp(s/sqrt(d) - 3ln2) = e/8 so the unnormalized
    attention accumulator stays below e4m3's 240 max; 1/Z (and the 1/SW
    unscale) is applied per-token on the x2 write via a K=1 fp32 matmul that
    extracts rz columns (replaces the old fp32 transposes).
  - when all matmul biases fold to zero (true for this reference: betas and
    biases are zeros) the bias matmuls are omitted entirely at build time.

Schedule: a 3-stage software pipeline over the 4 token chunks, fused across
the old attention/MLP phase boundary so the MLP's PE-dense stream fills the
stalls of the attention dependency chains:
  slot c emits: attnV/Z/o-proj/LN2 of chunk c-1, fc+proj of chunk c-2, and
  scores+exp of chunk c (last, so the ACT engine does gelus first and exps
  at the end -- Exp and Gelu live in different activation-table sets, so this
  ordering costs only 2 table loads per slot).
x2 and h2s are rolling 2-chunk buffers; ms single-chunk. PSUM: 3 banks for
scores/Z/LN2-transposes, 2 for attnV/o-proj, 2 for fc/proj, 1 for rz.
"""

import numpy as np

P = 128
S, D, H = 2048, 768, 3072
DT = D // P            # 6 d-tiles
HT = H // P            # 24 h-tiles
ST = S // P            # 16 token tiles
CH = 512               # chunk width (tokens)
NCH = S // CH          # 4 chunks
TPC = CH // P          # 4 token tiles per chunk
EPS = 1e-5
N_CORES = 8
LN4 = 2.0794415416798357   # 3*ln2; exp bias so e8 = exp(s)/8
SW = 16.0                  # scale for D-side weight mats (sigma ~0.036)
SP = 32.0                  # scale for Wproj (sigma ~0.018)
WPL_J = 0                  # 0..12: partial Wpl residual passes (accuracy knob)

WEIGHT_NAMES = [
    "ln1_g", "ln1_b", "ln2_g", "ln2_b",
    "Wq", "bq", "Wk", "bk", "Wv", "bv", "Wo", "bo",
    "Wfc", "bfc", "Wproj", "bproj",
]

_CACHE = {}


def _prep(inputs):
    """Host-side weight quantization + LN/bias folding (pure numpy)."""
    import ml_dtypes
    E4, E5, BF = ml_dtypes.float8_e4m3, ml_dtypes.float8_e5m2, ml_dtypes.bfloat16
    f32 = lambda k: np.asarray(inputs[k], dtype=np.float32)
    g1, b1 = f32("ln1_g"), f32("ln1_b")
    g2, b2 = f32("ln2_g"), f32("ln2_b")
    Wq, Wk, Wv, Wo = f32("Wq"), f32("Wk"), f32("Wv"), f32("Wo")
    Wfc, Wproj = f32("Wfc"), f32("Wproj")
    q8 = lambda a: np.ascontiguousarray(a.astype(E4))
    Wfc_g = g2[:, None] * Wfc * SW
    wfh = Wfc_g.astype(E4)
    wph_f = Wproj * SP
    wph = wph_f.astype(E4)
    bo_f = f32("bo") + (f32("bv") + b1 @ Wv) @ Wo
    bp_f = f32("bproj")
    out = {
        "wq8": q8(SW * g1[:, None] * Wq), "wk8": q8(SW * g1[:, None] * Wk),
        "wv8": q8(SW * g1[:, None] * Wv), "wo8": q8(SW * Wo),
        "bq_": f32("bq") + b1 @ Wq, "bk_": f32("bk") + b1 @ Wk,
        "wfh": np.ascontiguousarray(wfh),
        "wfl": np.ascontiguousarray(
            (Wfc_g - wfh.astype(np.float32)).astype(E5)),
        "wph": np.ascontiguousarray(wph),
        "wpl": np.ascontiguousarray(
            (wph_f - wph.astype(np.float32)).astype(E5)),
        "bfc_": f32("bfc") + b2 @ Wfc,
        "bo16": np.asarray(SW * bo_f, dtype=BF),
        "bp16": np.asarray(SP * bp_f, dtype=BF),
    }
    out["_has_bias"] = bool(np.any(bo_f != 0.0) or np.any(bp_f != 0.0))
    return out


def _build(has_bias, wpl_j):
    import concourse.bass as bass
    import concourse.tile as tile
    from concourse import bacc, mybir
    from concourse.masks import make_identity
    from contextlib import ExitStack

    F = mybir.dt.float32
    BF = mybir.dt.bfloat16
    E4 = mybir.dt.float8e4
    E5 = mybir.dt.float8e5
    I32 = mybir.dt.int32
    AF = mybir.ActivationFunctionType
    OP = mybir.AluOpType
    DR = mybir.MatmulPerfMode.DoubleRow

    nc = bacc.Bacc(None, target_bir_lowering=False)

    x_d = nc.dram_tensor("x", [S, D], F, kind="ExternalInput")
    out_d = nc.dram_tensor("out", [S, D], F, kind="ExternalOutput")
    wq8_d = nc.dram_tensor("wq8", [D, D], E4, kind="ExternalInput")
    wk8_d = nc.dram_tensor("wk8", [D, D], E4, kind="ExternalInput")
    wv8_d = nc.dram_tensor("wv8", [D, D], E4, kind="ExternalInput")
    wo8_d = nc.dram_tensor("wo8", [D, D], E4, kind="ExternalInput")
    wfh_d = nc.dram_tensor("wfh", [D, H], E4, kind="ExternalInput")
    wfl_d = nc.dram_tensor("wfl", [D, H], E5, kind="ExternalInput")
    wph_d = nc.dram_tensor("wph", [H, D], E4, kind="ExternalInput")
    bq_d = nc.dram_tensor("bq_", [D], F, kind="ExternalInput")
    bk_d = nc.dram_tensor("bk_", [D], F, kind="ExternalInput")
    bfc_d = nc.dram_tensor("bfc_", [H], F, kind="ExternalInput")
    if wpl_j:
        wpl_d = nc.dram_tensor("wpl", [H, D], E5, kind="ExternalInput")
    if has_bias:
        bo_d = nc.dram_tensor("bo16", [D], BF, kind="ExternalInput")
        bp_d = nc.dram_tensor("bp16", [D], BF, kind="ExternalInput")

    inv_sqrt_d = 1.0 / float(np.sqrt(np.float32(D)))

    with tile.TileContext(nc) as tc, ExitStack() as ctx:
        singles = ctx.enter_context(tc.tile_pool(name="singles", bufs=1))

        ident16 = singles.tile([P, P], BF)
        make_identity(nc, ident16)
        nln4_t = singles.tile([P, 1], F)
        nc.vector.memset(nln4_t, -LN4)
        invsw = singles.tile([1, 1], F)
        nc.vector.memset(invsw, 1.0 / SW)
        ones8 = singles.tile([P, 2, P], E4)
        nc.vector.memset(ones8, 1.0)
        bq_col = singles.tile([P, DT], F)
        bk_col = singles.tile([P, DT], F)
        bfc_col = singles.tile([P, HT], F)
        if has_bias:
            ones_row = singles.tile([1, P], BF)
            nc.vector.memset(ones_row, 1.0)
            bo_row = singles.tile([1, D], BF)
            bp_row = singles.tile([1, D], BF)

        # persistent activations
        perm = ctx.enter_context(tc.tile_pool(name="perm", bufs=1))
        x2_sb = perm.tile([P, 2 * TPC, D], F)       # rolling 2-chunk residual
        h2s = perm.tile([P, 2, DT, 2, CH], E4)      # rolling LN2 out hi/lo
        ms = perm.tile([P, HT, 2, CH], E4)          # gelu out hi/lo (1 chunk)

        wfhp = ctx.enter_context(tc.tile_pool(name="wfhp", bufs=1))
        wfh_t = wfhp.tile([P, DT, H], E4)
        wflp = ctx.enter_context(tc.tile_pool(name="wflp", bufs=1))
        wfl_t = wflp.tile([P, DT, H], E5)

        qkv_ctx = ExitStack()
        qkvp = qkv_ctx.enter_context(tc.tile_pool(name="qkv", bufs=1))
        k8 = qkvp.tile([P, DT, S], E4)
        q8 = qkvp.tile([P, DT, S], E4)
        v8 = qkvp.tile([P, ST, D], E4)
        wo_t = qkvp.tile([P, DT, D], E4)

        e8_ctx = ExitStack()
        e8p = e8_ctx.enter_context(tc.tile_pool(name="e8p", bufs=2))
        e8_list = [None] * NCH

        wqkv_ctx = ExitStack()
        wqkv = wqkv_ctx.enter_context(tc.tile_pool(name="wqkv", bufs=1))
        wv_t = wqkv.tile([P, DT, D], E4)
        wk_t = wqkv.tile([P, DT, D], E4)
        wq_t = wqkv.tile([P, DT, D], E4)

        hT_ctx = ExitStack()
        hTp = hT_ctx.enter_context(tc.tile_pool(name="hT", bufs=1))
        hT8 = hTp.tile([P, DT, S], E4)

        # ------------- Phase 1: LN1 -> hT8; v, k, q (all fp8) -------------
        with (
            tc.tile_pool(name="ph1", bufs=3) as ph1,
            tc.tile_pool(name="ps_tr", bufs=2, space="PSUM") as ps_trp,
            tc.tile_pool(name="ps_k", bufs=3, space="PSUM") as ps_kp,
            tc.tile_pool(name="ps_v", bufs=3, space="PSUM") as ps_vp,
        ):
            x_ts = [None] * ST
            h_ts = [None] * ST
            Q0 = {0: nc.sync, 1: nc.gpsimd, 2: nc.scalar, 3: nc.gpsimd}

            def ln1_group(g0, n):
                mvg = ph1.tile([P, TPC, 2], F, tag="mvg")
                for i in range(n):
                    st = g0 + i
                    x_t = ph1.tile([P, D], F, tag="xt", bufs=6)
                    q = Q0.get(st, nc.gpsimd if st % 2 else nc.sync)
                    q.dma_start(out=x_t, in_=x_d.ap()[st * P:(st + 1) * P, :])
                    # weight prefetches: wv/wk/wq must all be issued by the
                    # end of group 1 (consume_chunk(0) follows it)
                    if st == 2:
                        nc.sync.dma_start(
                            wv_t[:],
                            wv8_d.ap().rearrange("(t p) n -> p t n", p=P))
                        nc.scalar.dma_start(
                            wk_t[:],
                            wk8_d.ap().rearrange("(t p) n -> p t n", p=P))
                    if st == 3:
                        nc.sync.dma_start(
                            wq_t[:],
                            wq8_d.ap().rearrange("(t p) n -> p t n", p=P))
                        nc.scalar.dma_start(
                            bk_col, bk_d.ap().rearrange("(t p) -> p t", p=P))
                        nc.scalar.dma_start(
                            bq_col, bq_d.ap().rearrange("(t p) -> p t", p=P))
                    if st == 9:
                        nc.sync.dma_start(
                            wo_t[:],
                            wo8_d.ap().rearrange("(t p) n -> p t n", p=P))
                        nc.sync.dma_start(
                            bfc_col, bfc_d.ap().rearrange("(t p) -> p t", p=P))
                    x_ts[st] = x_t
                    stats = ph1.tile([P, 3, 6], F, tag="st")
                    for j in range(3):
                        nc.vector.bn_stats(out=stats[:, j, :],
                                           in_=x_t[:, j * 256:(j + 1) * 256])
                    nc.vector.bn_aggr(out=mvg[:, i, :], in_=stats)
                # batched rsqrt(var+eps): quake bit-trick + 1 Newton (DVE)
                rsg = ph1.tile([P, TPC], F, tag="rsg")
                nc.vector.tensor_scalar(out=rsg[:, :n], in0=mvg[:, :n, 1],
                                        scalar1=EPS, scalar2=None, op0=OP.add)
                rig = ph1.tile([P, TPC], I32, tag="rig")
                nc.vector.tensor_scalar(out=rig[:, :n],
                                        in0=rsg[:, :n].bitcast(I32),
                                        scalar1=1, scalar2=None,
                                        op0=OP.logical_shift_right)
                nc.vector.tensor_scalar(out=rig[:, :n], in0=rig[:, :n],
                                        scalar1=-1,
                                        scalar2=None, op0=OP.bitwise_xor)
                nc.vector.tensor_scalar(out=rig[:, :n], in0=rig[:, :n],
                                        scalar1=0x5f3759e0,
                                        scalar2=None, op0=OP.add)
                rng = ph1.tile([P, TPC], F, tag="rng")
                nc.vector.tensor_tensor(out=rng[:, :n],
                                        in0=rig[:, :n].bitcast(F),
                                        in1=rig[:, :n].bitcast(F), op=OP.mult)
                nc.vector.tensor_tensor(out=rng[:, :n], in0=rng[:, :n],
                                        in1=rsg[:, :n], op=OP.mult)
                nc.vector.tensor_scalar(out=rng[:, :n], in0=rng[:, :n],
                                        scalar1=-0.5,
                                        scalar2=1.5, op0=OP.mult, op1=OP.add)
                nc.vector.tensor_tensor(out=rsg[:, :n],
                                        in0=rig[:, :n].bitcast(F),
                                        in1=rng[:, :n], op=OP.mult)
                for i in range(n):
                    st = g0 + i
                    h_t = ph1.tile([P, D], BF, tag="ht", bufs=6)
                    nc.gpsimd.tensor_scalar(out=h_t, in0=x_ts[st],
                                            scalar1=mvg[:, i, 0:1],
                                            scalar2=rsg[:, i:i + 1],
                                            op0=OP.subtract, op1=OP.mult)
                    h_ts[st] = h_t

            def consume_trv(g0, n):
                for i in range(n):
                    sp = g0 + i
                    h_t = h_ts[sp]
                    ps_tr = ps_trp.tile([P, DT, P], BF, tag="tr")
                    for dt_ in range(DT):
                        nc.tensor.transpose(ps_tr[:, dt_, :],
                                            h_t[:, dt_ * P:(dt_ + 1) * P],
                                            ident16)
                    nc.scalar.activation(
                        out=hT8[:, :, sp * P:(sp + 1) * P],
                        in_=ps_tr, func=AF.Copy, bias=0.0, scale=1.0)

            def consume_chunk(kc):
                for i in range(TPC):
                    sv = TPC * kc + i
                    for lo, w, eng in ((0, 512, 0), (512, 256, 1)):
                        ps_v = ps_vp.tile([P, CH], F, tag="v")
                        for j in range(DT // 2):
                            nc.tensor.matmul(
                                ps_v[:, :w],
                                hT8[:, 2 * j:2 * j + 2, sv * P:(sv + 1) * P],
                                wv_t[:, 2 * j:2 * j + 2, lo:lo + w],
                                start=(j == 0), stop=(j == DT // 2 - 1),
                                perf_mode=DR)
                        # bv folds into bo' host-side (bo16)
                        if eng:
                            nc.scalar.activation(
                                out=v8[:, sv, lo:lo + w], in_=ps_v[:, :w],
                                func=AF.Identity, bias=0.0, scale=1.0 / SW)
                        else:
                            nc.vector.tensor_scalar(
                                out=v8[:, sv, lo:lo + w], in0=ps_v[:, :w],
                                scalar1=1.0 / SW, scalar2=None, op0=OP.mult)
                for dtp in range(DT):
                    ps_k = ps_kp.tile([P, CH], F, tag="k")
                    for j in range(DT // 2):
                        nc.tensor.matmul(
                            ps_k,
                            wk_t[:, 2 * j:2 * j + 2, dtp * P:(dtp + 1) * P],
                            hT8[:, 2 * j:2 * j + 2, kc * CH:(kc + 1) * CH],
                            start=(j == 0), stop=(j == DT // 2 - 1),
                            perf_mode=DR)
                    if dtp % 2:
                        nc.scalar.activation(
                            out=k8[:, dtp, kc * CH:(kc + 1) * CH], in_=ps_k,
                            func=AF.Identity, bias=bk_col[:, dtp:dtp + 1],
                            scale=1.0 / SW)
                    else:
                        nc.vector.tensor_scalar(
                            out=k8[:, dtp, kc * CH:(kc + 1) * CH], in0=ps_k,
                            scalar1=1.0 / SW, scalar2=bk_col[:, dtp:dtp + 1],
                            op0=OP.mult, op1=OP.add)
                if kc == 0:
                    e8_list[0] = e8p.tile([P, ST, CH], E4, tag="e8",
                                          name="e8_c0")
                    for dtp in range(DT):
                        ps_q = ps_kp.tile([P, CH], F, tag="k", name="ps_q0")
                        for j in range(DT // 2):
                            nc.tensor.matmul(
                                ps_q,
                                wq_t[:, 2 * j:2 * j + 2,
                                     dtp * P:(dtp + 1) * P],
                                hT8[:, 2 * j:2 * j + 2, 0:CH],
                                start=(j == 0), stop=(j == DT // 2 - 1),
                                perf_mode=DR)
                        if dtp % 2:
                            nc.scalar.activation(
                                out=q8[:, dtp, 0:CH], in_=ps_q,
                                func=AF.Identity,
                                bias=bq_col[:, dtp:dtp + 1], scale=1.0 / SW)
                        else:
                            nc.vector.tensor_scalar(
                                out=q8[:, dtp, 0:CH], in0=ps_q,
                                scalar1=1.0 / SW,
                                scalar2=bq_col[:, dtp:dtp + 1],
                                op0=OP.mult, op1=OP.add)
                # chunk-0 scores for this k-chunk's keys
                for st2 in range(TPC * kc, TPC * kc + TPC):
                    ps_s = ps_kp.tile([P, CH], F, tag="k", name="ps_s0")
                    for j in range(DT // 2):
                        nc.tensor.matmul(
                            ps_s,
                            k8[:, 2 * j:2 * j + 2, st2 * P:(st2 + 1) * P],
                            q8[:, 2 * j:2 * j + 2, 0:CH],
                            start=(j == 0), stop=(j == DT // 2 - 1),
                            perf_mode=DR)
                    nc.scalar.activation(out=e8_list[0][:, st2, :], in_=ps_s,
                                         func=AF.Exp, scale=inv_sqrt_d,
                                         bias=nln4_t)

            groups = [(0, 2), (2, 2), (4, 4), (8, 4), (12, 4)]
            done_chunks = 0
            for gi in range(len(groups) + 1):
                if gi >= 1:
                    g0, n = groups[gi - 1]
                    consume_trv(g0, n)
                    full = (g0 + n) // TPC
                    while done_chunks < full:
                        consume_chunk(done_chunks)
                        done_chunks += 1
                if gi < len(groups):
                    ln1_group(*groups[gi])
            # q for chunks 1..3 (frees hT afterwards)
            for sc in range(1, NCH):
                for dtp in range(DT):
                    ps_q = ps_kp.tile([P, CH], F, tag="k")
                    for j in range(DT // 2):
                        nc.tensor.matmul(
                            ps_q,
                            wq_t[:, 2 * j:2 * j + 2, dtp * P:(dtp + 1) * P],
                            hT8[:, 2 * j:2 * j + 2, sc * CH:(sc + 1) * CH],
                            start=(j == 0), stop=(j == DT // 2 - 1),
                            perf_mode=DR)
                    if dtp % 2:
                        nc.scalar.activation(
                            out=q8[:, dtp, sc * CH:(sc + 1) * CH], in_=ps_q,
                            func=AF.Identity, bias=bq_col[:, dtp:dtp + 1],
                            scale=1.0 / SW)
                    else:
                        nc.vector.tensor_scalar(
                            out=q8[:, dtp, sc * CH:(sc + 1) * CH], in0=ps_q,
                            scalar1=1.0 / SW, scalar2=bq_col[:, dtp:dtp + 1],
                            op0=OP.mult, op1=OP.add)
        hT_ctx.close()
        wqkv_ctx.close()

        # MLP fc weights land during slot 1 (the DMA device is saturated with
        # x tiles + qkv weights during phase 1; here it idles)
        nc.sync.dma_start(wfh_t[:],
                          wfh_d.ap().rearrange("(t p) n -> p t n", p=P))
        nc.sync.dma_start(wfl_t[:],
                          wfl_d.ap().rearrange("(t p) n -> p t n", p=P))

        # MLP proj weights arrive into the space hT8/wq/wk/wv vacated
        wphp_ctx = ExitStack()
        wphp = wphp_ctx.enter_context(tc.tile_pool(name="wphp", bufs=1))
        wph_t = wphp.tile([P, HT, D], E4)
        nc.sync.dma_start(wph_t[:],
                          wph_d.ap().rearrange("(t p) n -> p t n", p=P))
        if wpl_j:
            wpl_t = wphp.tile([P, 2 * wpl_j, D], E5)
            nc.sync.dma_start(
                wpl_t[:],
                wpl_d.ap()[0:2 * wpl_j * P, :].rearrange(
                    "(t p) n -> p t n", p=P))
        if has_bias:
            nc.sync.dma_start(bo_row, bo_d.ap().unsqueeze(0))
            nc.sync.dma_start(bp_row, bp_d.ap().unsqueeze(0))

        def quake_rsqrt(pool, mvs, rss):
            # rsqrt(var+eps): quake bit-trick + 2 Newton steps, all on DVE
            vb = pool.tile([P, TPC], F, tag="vb")
            nc.vector.tensor_scalar(out=vb, in0=mvs[:, :, 1], scalar1=EPS,
                                    scalar2=None, op0=OP.add)
            ib = pool.tile([P, TPC], I32, tag="ib")
            nc.vector.tensor_scalar(out=ib, in0=vb[:].bitcast(I32),
                                    scalar1=1, scalar2=None,
                                    op0=OP.logical_shift_right)
            nc.vector.tensor_scalar(out=ib, in0=ib, scalar1=-1,
                                    scalar2=None, op0=OP.bitwise_xor)
            nc.vector.tensor_scalar(out=ib, in0=ib, scalar1=0x5f3759e0,
                                    scalar2=None, op0=OP.add)
            nc.vector.tensor_copy(out=rss, in_=ib[:].bitcast(F))
            nt = pool.tile([P, TPC], F, tag="nt")
            for _ in range(2):
                nc.vector.tensor_tensor(out=nt, in0=rss, in1=rss, op=OP.mult)
                nc.vector.tensor_tensor(out=nt, in0=nt, in1=vb, op=OP.mult)
                nc.vector.tensor_scalar(out=nt, in0=nt, scalar1=-0.5,
                                        scalar2=1.5, op0=OP.mult, op1=OP.add)
                nc.vector.tensor_tensor(out=rss, in0=rss, in1=nt, op=OP.mult)

        # ------------- Fused pipeline: slots c = 1..5 -------------
        # slot c: back(c-1) attnV/Z/o/LN2 + mlp(c-2) fc/proj + front(c) scores
        with (
            tc.tile_pool(name="pp", bufs=2) as pp,
            tc.tile_pool(name="ps_sc", bufs=3, space="PSUM") as ps_scp,
            tc.tile_pool(name="ps_av", bufs=2, space="PSUM") as ps_avp,
            tc.tile_pool(name="ps_fp", bufs=2, space="PSUM") as ps_fpp,
            tc.tile_pool(name="ps_rz", bufs=1, space="PSUM") as ps_rzp,
        ):
            for c in range(1, 6):
                b = c - 1 if c - 1 <= 3 else None      # back chunk
                m = c - 2 if 0 <= c - 2 <= 3 else None  # mlp chunk
                have_front = c <= 3

                if have_front:
                    e8_list[c] = e8p.tile([P, ST, CH], E4, tag="e8",
                                          name=f"e8_{c}")

                # ---- mlp(m) quanta ----
                fcq = []
                if m is not None:
                    hs = h2s[:, m % 2]

                    def fc_ht(ht, hs=hs):
                        ps = ps_fpp.tile([P, CH], F, tag="fp", name="u")
                        for j in range(DT // 2):
                            nc.tensor.matmul(
                                ps, wfh_t[:, 2 * j:2 * j + 2,
                                          ht * P:(ht + 1) * P],
                                hs[:, 2 * j:2 * j + 2, 0, :],
                                start=(j == 0), stop=False, perf_mode=DR)
                        for j in range(DT // 2):
                            nc.tensor.matmul(
                                ps, wfh_t[:, 2 * j:2 * j + 2,
                                          ht * P:(ht + 1) * P],
                                hs[:, 2 * j:2 * j + 2, 1, :],
                                start=False, stop=False, perf_mode=DR)
                        for j in range(DT // 2):
                            nc.tensor.matmul(
                                ps, wfl_t[:, 2 * j:2 * j + 2,
                                          ht * P:(ht + 1) * P],
                                hs[:, 2 * j:2 * j + 2, 0, :],
                                start=False, stop=(j == DT // 2 - 1),
                                perf_mode=DR)
                        m16 = pp.tile([P, CH], BF, tag="m16", bufs=3)
                        nc.scalar.activation(out=m16, in_=ps, func=AF.Gelu,
                                             bias=bfc_col[:, ht:ht + 1],
                                             scale=1.0 / SW)
                        nc.gpsimd.tensor_copy(out=ms[:, ht, 0, :], in_=m16)
                        nc.gpsimd.tensor_tensor(out=ms[:, ht, 1, :], in0=m16,
                                                in1=ms[:, ht, 0, :],
                                                op=OP.subtract)

                    fcq = [(lambda ht=ht: fc_ht(ht)) for ht in range(HT)]

                fci = [0]

                def emit_fc(n):
                    for _ in range(n):
                        if fci[0] < len(fcq):
                            fcq[fci[0]]()
                            fci[0] += 1

                def proj_su(su, seg, o_ts={}):
                    lo, w = (0, 512) if seg == 0 else (512, 256)
                    ps = ps_fpp.tile([P, CH], F, tag="fp", name="pj")
                    last = ('b' if has_bias else
                            ('l' if wpl_j else 'a'))
                    for arm in range(2):
                        for j in range(HT // 2):
                            isl = (last == 'a' and arm == 1
                                   and j == HT // 2 - 1)
                            nc.tensor.matmul(
                                ps[:, :w],
                                ms[:, 2 * j:2 * j + 2, arm,
                                   su * P:(su + 1) * P],
                                wph_t[:, 2 * j:2 * j + 2, lo:lo + w],
                                start=(arm == 0 and j == 0), stop=isl,
                                perf_mode=DR)
                    if wpl_j:
                        for j in range(wpl_j):
                            isl = (last == 'l' and j == wpl_j - 1)
                            nc.tensor.matmul(
                                ps[:, :w],
                                ms[:, 2 * j:2 * j + 2, 0,
                                   su * P:(su + 1) * P],
                                wpl_t[:, 2 * j:2 * j + 2, lo:lo + w],
                                start=False, stop=isl, perf_mode=DR)
                    if has_bias:
                        nc.tensor.matmul(ps[:, :w], ones_row,
                                         bp_row[:, lo:lo + w],
                                         start=False, stop=True)
                    st_sl = (m % 2) * TPC + su
                    if seg == 0:
                        o_ts[su] = pp.tile([P, D], F, tag="ot", bufs=2,
                                           name="o_t")
                    o_t = o_ts[su]
                    nc.scalar.activation(out=o_t[:, lo:lo + w],
                                         in_=ps[:, :w], func=AF.Identity,
                                         bias=0.0, scale=1.0 / SP)
                    nc.gpsimd.tensor_tensor(out=o_t[:, lo:lo + w],
                                            in0=o_t[:, lo:lo + w],
                                            in1=x2_sb[:, st_sl, lo:lo + w],
                                            op=OP.add)
                    if seg == 1:
                        st = m * TPC + su
                        nc.sync.dma_start(
                            out_d.ap()[st * P:(st + 1) * P, :], o_t)

                # ---- back(b) helpers ----
                if b is not None:
                    e8b = e8_list[b]
                    xb_ts = []
                    for su in range(TPC):
                        st = b * TPC + su
                        x_t = pp.tile([P, D], F, tag="xb", bufs=4)
                        nc.sync.dma_start(x_t,
                                          x_d.ap()[st * P:(st + 1) * P, :])
                        xb_ts.append(x_t)
                    yT8 = pp.tile([P, DT, CH], E4, tag="yt", bufs=2)
                    mvs = pp.tile([P, TPC, 2], F, tag="mvs")
                    rss = pp.tile([P, TPC], F, tag="rss")
                    rz_cols = pp.tile([P, TPC], F, tag="rzc")

                    def back_attnv(dtp):
                        ps = ps_avp.tile([P, CH], F, tag="av", name="av")
                        for pr in range(ST // 2):
                            nc.tensor.matmul(
                                ps,
                                v8[:, 2 * pr:2 * pr + 2,
                                   dtp * P:(dtp + 1) * P],
                                e8b[:, 2 * pr:2 * pr + 2, :],
                                start=(pr == 0), stop=(pr == ST // 2 - 1),
                                perf_mode=DR)
                        if dtp % 2:
                            nc.scalar.activation(out=yT8[:, dtp, :], in_=ps,
                                                 func=AF.Copy, bias=0.0,
                                                 scale=1.0)
                        else:
                            nc.vector.tensor_copy(out=yT8[:, dtp, :], in_=ps)

                    def back_z_rz():
                        ps_z = ps_scp.tile([P, CH], F, tag="sc", name="z")
                        for pr in range(ST // 2):
                            nc.tensor.matmul(ps_z, ones8,
                                             e8b[:, 2 * pr:2 * pr + 2, :],
                                             start=(pr == 0),
                                             stop=(pr == ST // 2 - 1),
                                             perf_mode=DR)
                        if has_bias:
                            z_row = pp.tile([1, CH], BF, tag="zrow")
                            nc.vector.tensor_copy(out=z_row, in_=ps_z[0:1, :])
                        else:
                            z_row = None
                        rz = pp.tile([P, CH], F, tag="rz", bufs=1)
                        nc.vector.reciprocal(out=rz, in_=ps_z)
                        ps_rz = ps_rzp.tile([P, TPC], F, tag="rz")
                        for su in range(TPC):
                            # K=1 fp32 matmul: broadcast rz row -> per-token
                            # column, pre-divided by SW (invsw operand)
                            nc.tensor.matmul(
                                ps_rz[:, su:su + 1],
                                rz[0:1, su * P:(su + 1) * P],
                                invsw, start=True, stop=True)
                        nc.vector.tensor_copy(out=rz_cols, in_=ps_rz)
                        return z_row

                    def back_o(su, z_row):
                        st_sl = (b % 2) * TPC + su
                        for lo, w in ((0, 512), (512, 256)):
                            ps = ps_avp.tile([P, CH], F, tag="av", name="o")
                            for j in range(DT // 2):
                                nc.tensor.matmul(
                                    ps[:, :w],
                                    yT8[:, 2 * j:2 * j + 2,
                                        su * P:(su + 1) * P],
                                    wo_t[:, 2 * j:2 * j + 2, lo:lo + w],
                                    start=(j == 0),
                                    stop=(j == DT // 2 - 1 and
                                          not has_bias),
                                    perf_mode=DR)
                            if has_bias:
                                # bo rides as bo*Z*SW; the rz/SW scaling
                                # cancels it back to +bo
                                nc.tensor.matmul(ps[:, :w],
                                                 z_row[:, su * P:(su + 1) * P],
                                                 bo_row[:, lo:lo + w],
                                                 start=False, stop=True)
                            nc.vector.tensor_scalar(
                                out=x2_sb[:, st_sl, lo:lo + w],
                                in0=ps[:, :w],
                                scalar1=rz_cols[:, su:su + 1],
                                scalar2=None, op0=OP.mult)
                            nc.gpsimd.tensor_tensor(
                                out=x2_sb[:, st_sl, lo:lo + w],
                                in0=x2_sb[:, st_sl, lo:lo + w],
                                in1=xb_ts[su][:, lo:lo + w], op=OP.add)
                        stats = pp.tile([P, 3, 6], F, tag="st3")
                        for i in range(3):
                            nc.vector.bn_stats(
                                out=stats[:, i, :],
                                in_=x2_sb[:, st_sl, i * 256:(i + 1) * 256])
                        nc.vector.bn_aggr(out=mvs[:, su, :], in_=stats)

                    def back_ln2(su):
                        st_sl = (b % 2) * TPC + su
                        h2_t = pp.tile([P, D], BF, tag="h2", bufs=3)
                        nc.vector.tensor_scalar(out=h2_t,
                                                in0=x2_sb[:, st_sl, :],
                                                scalar1=mvs[:, su, 0:1],
                                                scalar2=rss[:, su:su + 1],
                                                op0=OP.subtract, op1=OP.mult)
                        ps_tr = ps_scp.tile([P, DT, P], BF, tag="sc",
                                            name="tr2")
                        for dt_ in range(DT):
                            nc.tensor.transpose(
                                ps_tr[:, dt_, :],
                                h2_t[:, dt_ * P:(dt_ + 1) * P], ident16)
                        hi = h2s[:, b % 2, :, 0, su * P:(su + 1) * P]
                        nc.scalar.activation(out=hi, in_=ps_tr, func=AF.Copy,
                                             bias=0.0, scale=1.0)
                        nc.vector.tensor_tensor(
                            out=h2s[:, b % 2, :, 1, su * P:(su + 1) * P],
                            in0=ps_tr, in1=hi, op=OP.subtract)

                def front_score(st2):
                    ps = ps_scp.tile([P, CH], F, tag="sc", name="s")
                    for j in range(DT // 2):
                        nc.tensor.matmul(
                            ps,
                            k8[:, 2 * j:2 * j + 2, st2 * P:(st2 + 1) * P],
                            q8[:, 2 * j:2 * j + 2, c * CH:(c + 1) * CH],
                            start=(j == 0), stop=(j == DT // 2 - 1),
                            perf_mode=DR)
                    nc.scalar.activation(out=e8_list[c][:, st2, :], in_=ps,
                                         func=AF.Exp, scale=inv_sqrt_d,
                                         bias=nln4_t)

                # ---- slot emission ----
                sci = [0]

                def emit_sc(n):
                    for _ in range(n):
                        if have_front and sci[0] < ST:
                            front_score(sci[0])
                            sci[0] += 1

                # in MLP-less slots the scores weave early so their exps
                # (ACT) finish before the next slot's gelus queue behind them
                early_sc = (m is None)
                if b is not None:
                    # stage 1: attnV woven with fc
                    for dtp in range(DT):
                        if early_sc:
                            emit_sc(2)
                        back_attnv(dtp)
                        emit_fc(1)
                    emit_fc(2)
                    # stage 2: Z + rz, more fc
                    if early_sc:
                        emit_sc(2)
                    z_row = back_z_rz()
                    emit_fc(4)
                    # stage 3: o-proj + x2 + stats, more fc
                    for su in range(TPC):
                        if early_sc:
                            emit_sc(1)
                        back_o(su, z_row)
                        emit_fc(2)
                    # stage 4: LN2 apply + transposes + h2s
                    quake_rsqrt(pp, mvs, rss)
                    for su in range(TPC):
                        back_ln2(su)
                        emit_fc(1)
                emit_fc(HT)  # any leftovers (and the c=5 no-back slot)
                # stage 5: proj woven with next chunk's scores+exps
                if m is not None:
                    for su in range(TPC):
                        for seg in range(2):
                            proj_su(su, seg)
                            emit_sc(2)
                emit_sc(ST)
        wphp_ctx.close()
        e8_ctx.close()
        qkv_ctx.close()

    return nc


def _get_nc():
    key = _CACHE.get("key")
    if "nc" not in _CACHE or key != (_CACHE.get("has_bias"), WPL_J):
        has_bias = _CACHE.get("has_bias", False)
        nc = _build(has_bias, WPL_J)
        nc.compile()
        _CACHE["nc"] = nc
        _CACHE["key"] = (has_bias, WPL_J)
    return _CACHE["nc"]


TRACE = False


def kernel(**inputs):
    from concourse.bass_utils import run_bass_kernel_spmd

    x = np.asarray(inputs["x"], dtype=np.float32)
    base = _prep(inputs)
    _CACHE["has_bias"] = base.pop("_has_bias")
    nc = _get_nc()
    names = {"wq8", "wk8", "wv8", "wo8", "wfh", "wfl", "wph",
             "bq_", "bk_", "bfc_"}
    if WPL_J:
        names.add("wpl")
    if _CACHE["has_bias"]:
        names.add("bo16")
        names.add("bp16")
    ship = {k: v for k, v in base.items() if k in names}
    in_maps = [dict(ship, x=np.ascontiguousarray(x[bb]))
               for bb in range(N_CORES)]
    res = run_bass_kernel_spmd(nc, in_maps, core_ids=list(range(N_CORES)),
                               trace=TRACE)
    _CACHE["last_res"] = res
    return np.stack([res.results[bb]["out"] for bb in range(N_CORES)], axis=0)


# revision 26
# speedup vs baseline: 1.2124x; 1.0107x over previous
"""Trainium2 Bass kernel for a dense transformer block (B=8, S=2048, D=768, H=3072).

Sharding: pure data-parallel over batch -- one batch element per NeuronCore.

All heavy matmuls are fp8-e4m3 MatmulPerfMode.DoubleRow (K=256/instruction at
0.5 cycles/row). Cost model: matmul time = out_free_size * cycles_per_row, so
total PE time ~ (#contraction passes) x (output width); arms on the MLP
matmuls are the dominant cost.

Numerics (rel_absmax gate 2e-2; emulated 1.5e-2, baseline was 6.1e-3):
  - weights are pre-scaled by powers of two (SW=16 for the D-side mats,
    SP=32 for Wproj) so their entries sit in e4m3's normal range instead of
    half-denormal; the inverse scales ride existing activation scale operands
    (q/k/v/gelu) or the rz-extraction matmul (o-proj) for free.
  - attention path (q/k/v/scores/exp/y/o) is plain e4m3.
  - fc keeps 3 arms: Wfh*(hi+lo) + Wfl*hi (h2 split hi/lo e4m3; Wfl e5m2
    residual). proj runs 2 arms: Wph*(mhi+mlo) -- the Wpl residual arm is
    dropped (WPL_J can partially restore it).
  - exp is computed as exp(s/sqrt(d) - 3ln2) = e/8 so the unnormalized
    attention accumulator stays below e4m3's 240 max; 1/Z (and the 1/SW
    unscale) is applied per-token on the x2 write via a K=1 fp32 matmul that
    extracts rz columns (replaces the old fp32 transposes).
  - when all matmul biases fold to zero (true for this reference: betas and
    biases are zeros) the bias matmuls are omitted entirely at build time.

Schedule: a 3-stage software pipeline over the 4 token chunks, fused across
the old attention/MLP phase boundary so the MLP's PE-dense stream fills the
stalls of the attention dependency chains:
  slot c emits: attnV/Z/o-proj/LN2 of chunk c-1, fc+proj of chunk c-2, and
  scores+exp of chunk c (last, so the ACT engine does gelus first and exps
  at the end -- Exp and Gelu live in different activation-table sets, so this
  ordering costs only 2 table loads per slot).
x2 and h2s are rolling 2-chunk buffers; ms single-chunk. PSUM: 3 banks for
scores/Z/LN2-transposes, 2 for attnV/o-proj, 2 for fc/proj, 1 for rz.
"""

import numpy as np

P = 128
S, D, H = 2048, 768, 3072
DT = D // P            # 6 d-tiles
HT = H // P            # 24 h-tiles
ST = S // P            # 16 token tiles
CH = 512               # chunk width (tokens)
NCH = S // CH          # 4 chunks
TPC = CH // P          # 4 token tiles per chunk
EPS = 1e-5
N_CORES = 8
LN4 = 2.0794415416798357   # 3*ln2; exp bias so e8 = exp(s)/8
SW = 16.0                  # scale for D-side weight mats (sigma ~0.036)
SP = 32.0                  # scale for Wproj (sigma ~0.018)
WPL_J = 0                  # 0..12: partial Wpl residual passes (accuracy knob)

WEIGHT_NAMES = [
    "ln1_g", "ln1_b", "ln2_g", "ln2_b",
    "Wq", "bq", "Wk", "bk", "Wv", "bv", "Wo", "bo",
    "Wfc", "bfc", "Wproj", "bproj",
]

_CACHE = {}


def _prep(inputs):
    """Host-side weight quantization + LN/bias folding (pure numpy)."""
    import ml_dtypes
    E4, E5, BF = ml_dtypes.float8_e4m3, ml_dtypes.float8_e5m2, ml_dtypes.bfloat16
    f32 = lambda k: np.asarray(inputs[k], dtype=np.float32)
    g1, b1 = f32("ln1_g"), f32("ln1_b")
    g2, b2 = f32("ln2_g"), f32("ln2_b")
    Wq, Wk, Wv, Wo = f32("Wq"), f32("Wk"), f32("Wv"), f32("Wo")
    Wfc, Wproj = f32("Wfc"), f32("Wproj")
    q8 = lambda a: np.ascontiguousarray(a.astype(E4))
    Wfc_g = g2[:, None] * Wfc * SW
    wfh = Wfc_g.astype(E4)
    wph_f = Wproj * SP
    wph = wph_f.astype(E4)
    bo_f = f32("bo") + (f32("bv") + b1 @ Wv) @ Wo
    bp_f = f32("bproj")
    out = {
        "wq8": q8(SW * g1[:, None] * Wq), "wk8": q8(SW * g1[:, None] * Wk),
        "wv8": q8(SW * g1[:, None] * Wv), "wo8": q8(SW * Wo),
        "bq_": f32("bq") + b1 @ Wq, "bk_": f32("bk") + b1 @ Wk,
        "wfh": np.ascontiguousarray(wfh),
        "wfl": np.ascontiguousarray(
            (Wfc_g - wfh.astype(np.float32)).astype(E5)),
        "wph": np.ascontiguousarray(wph),
        "wpl": np.ascontiguousarray(
            (wph_f - wph.astype(np.float32)).astype(E5)),
        "bfc_": f32("bfc") + b2 @ Wfc,
        "bo16": np.asarray(SW * bo_f, dtype=BF),
        "bp16": np.asarray(SP * bp_f, dtype=BF),
    }
    out["_has_bias"] = bool(np.any(bo_f != 0.0) or np.any(bp_f != 0.0))
    return out


def _build(has_bias, wpl_j):
    import concourse.bass as bass
    import concourse.tile as tile
    from concourse import bacc, mybir
    from concourse.masks import make_identity
    from contextlib import ExitStack

    F = mybir.dt.float32
    BF = mybir.dt.bfloat16
    E4 = mybir.dt.float8e4
    E5 = mybir.dt.float8e5
    I32 = mybir.dt.int32
    AF = mybir.ActivationFunctionType
    OP = mybir.AluOpType
    DR = mybir.MatmulPerfMode.DoubleRow

    nc = bacc.Bacc(None, target_bir_lowering=False)

    x_d = nc.dram_tensor("x", [S, D], F, kind="ExternalInput")
    out_d = nc.dram_tensor("out", [S, D], F, kind="ExternalOutput")
    wq8_d = nc.dram_tensor("wq8", [D, D], E4, kind="ExternalInput")
    wk8_d = nc.dram_tensor("wk8", [D, D], E4, kind="ExternalInput")
    wv8_d = nc.dram_tensor("wv8", [D, D], E4, kind="ExternalInput")
    wo8_d = nc.dram_tensor("wo8", [D, D], E4, kind="ExternalInput")
    wfh_d = nc.dram_tensor("wfh", [D, H], E4, kind="ExternalInput")
    wfl_d = nc.dram_tensor("wfl", [D, H], E5, kind="ExternalInput")
    wph_d = nc.dram_tensor("wph", [H, D], E4, kind="ExternalInput")
    bq_d = nc.dram_tensor("bq_", [D], F, kind="ExternalInput")
    bk_d = nc.dram_tensor("bk_", [D], F, kind="ExternalInput")
    bfc_d = nc.dram_tensor("bfc_", [H], F, kind="ExternalInput")
    if wpl_j:
        wpl_d = nc.dram_tensor("wpl", [H, D], E5, kind="ExternalInput")
    if has_bias:
        bo_d = nc.dram_tensor("bo16", [D], BF, kind="ExternalInput")
        bp_d = nc.dram_tensor("bp16", [D], BF, kind="ExternalInput")

    inv_sqrt_d = 1.0 / float(np.sqrt(np.float32(D)))

    with tile.TileContext(nc) as tc, ExitStack() as ctx:
        singles = ctx.enter_context(tc.tile_pool(name="singles", bufs=1))

        ident16 = singles.tile([P, P], BF)
        make_identity(nc, ident16)
        warm = singles.tile([1, 1], F)
        nc.vector.memset(warm, 0.0)
        nln4_t = singles.tile([P, 1], F)
        nc.vector.memset(nln4_t, -LN4)
        invsw = singles.tile([1, 1], F)
        nc.vector.memset(invsw, 1.0 / SW)
        ones8 = singles.tile([P, 2, P], E4)
        nc.vector.memset(ones8, 1.0)
        bq_col = singles.tile([P, DT], F)
        bk_col = singles.tile([P, DT], F)
        bfc_col = singles.tile([P, HT], F)
        if has_bias:
            ones_row = singles.tile([1, P], BF)
            nc.vector.memset(ones_row, 1.0)
            bo_row = singles.tile([1, D], BF)
            bp_row = singles.tile([1, D], BF)

        # persistent activations
        perm = ctx.enter_context(tc.tile_pool(name="perm", bufs=1))
        x2_sb = perm.tile([P, 2 * TPC, D], F)       # rolling 2-chunk residual
        h2s = perm.tile([P, 2, DT, 2, CH], E4)      # rolling LN2 out hi/lo
        ms = perm.tile([P, HT, 2, CH], E4)          # gelu out hi/lo (1 chunk)

        wfhp = ctx.enter_context(tc.tile_pool(name="wfhp", bufs=1))
        wfh_t = wfhp.tile([P, DT, H], E4)
        wflp = ctx.enter_context(tc.tile_pool(name="wflp", bufs=1))
        wfl_t = wflp.tile([P, DT, H], E5)

        qkv_ctx = ExitStack()
        qkvp = qkv_ctx.enter_context(tc.tile_pool(name="qkv", bufs=1))
        k8 = qkvp.tile([P, DT, S], E4)
        q8 = qkvp.tile([P, DT, S], E4)
        v8 = qkvp.tile([P, ST, D], E4)
        wo_t = qkvp.tile([P, DT, D], E4)

        e8_ctx = ExitStack()
        e8p = e8_ctx.enter_context(tc.tile_pool(name="e8p", bufs=2))
        e8_list = [None] * NCH

        wqkv_ctx = ExitStack()
        wqkv = wqkv_ctx.enter_context(tc.tile_pool(name="wqkv", bufs=1))
        wv_t = wqkv.tile([P, DT, D], E4)
        wk_t = wqkv.tile([P, DT, D], E4)
        wq_t = wqkv.tile([P, DT, D], E4)

        hT_ctx = ExitStack()
        hTp = hT_ctx.enter_context(tc.tile_pool(name="hT", bufs=1))
        hT8 = hTp.tile([P, DT, S], E4)

        # ------------- Phase 1: LN1 -> hT8; v, k, q (all fp8) -------------
        with (
            tc.tile_pool(name="ph1", bufs=3) as ph1,
            tc.tile_pool(name="ps_tr", bufs=2, space="PSUM") as ps_trp,
            tc.tile_pool(name="ps_k", bufs=3, space="PSUM") as ps_kp,
            tc.tile_pool(name="ps_v", bufs=3, space="PSUM") as ps_vp,
        ):
            x_ts = [None] * ST
            h_ts = [None] * ST
            # trigger the exp-set act table load at t=0, off the critical path
            nc.scalar.activation(out=warm, in_=warm, func=AF.Exp,
                                 bias=0.0, scale=1.0)
            # qkv weights ride the otherwise-idle ACT DMA queue; x tiles
            # alternate SP/Pool so nothing queues behind a weight transfer
            nc.scalar.dma_start(
                wv_t[:], wv8_d.ap().rearrange("(t p) n -> p t n", p=P))
            nc.scalar.dma_start(
                wk_t[:], wk8_d.ap().rearrange("(t p) n -> p t n", p=P))

            def ln1_group(g0, n):
                mvg = ph1.tile([P, TPC, 2], F, tag="mvg")
                for i in range(n):
                    st = g0 + i
                    x_t = ph1.tile([P, D], F, tag="xt", bufs=6)
                    q = nc.gpsimd if st % 2 else nc.sync
                    q.dma_start(out=x_t, in_=x_d.ap()[st * P:(st + 1) * P, :])
                    if st == 2:
                        nc.sync.dma_start(
                            wq_t[:],
                            wq8_d.ap().rearrange("(t p) n -> p t n", p=P))
                    if st == 3:
                        nc.gpsimd.dma_start(
                            bk_col, bk_d.ap().rearrange("(t p) -> p t", p=P))
                        nc.gpsimd.dma_start(
                            bq_col, bq_d.ap().rearrange("(t p) -> p t", p=P))
                    if st == 9:
                        nc.sync.dma_start(
                            wo_t[:],
                            wo8_d.ap().rearrange("(t p) n -> p t n", p=P))
                        nc.sync.dma_start(
                            bfc_col, bfc_d.ap().rearrange("(t p) -> p t", p=P))
                    x_ts[st] = x_t
                    stats = ph1.tile([P, 3, 6], F, tag="st")
                    for j in range(3):
                        nc.vector.bn_stats(out=stats[:, j, :],
                                           in_=x_t[:, j * 256:(j + 1) * 256])
                    nc.vector.bn_aggr(out=mvg[:, i, :], in_=stats)
                # batched rsqrt(var+eps): quake bit-trick + 1 Newton (DVE)
                rsg = ph1.tile([P, TPC], F, tag="rsg")
                nc.vector.tensor_scalar(out=rsg[:, :n], in0=mvg[:, :n, 1],
                                        scalar1=EPS, scalar2=None, op0=OP.add)
                rig = ph1.tile([P, TPC], I32, tag="rig")
                nc.vector.tensor_scalar(out=rig[:, :n],
                                        in0=rsg[:, :n].bitcast(I32),
                                        scalar1=1, scalar2=None,
                                        op0=OP.logical_shift_right)
                nc.vector.tensor_scalar(out=rig[:, :n], in0=rig[:, :n],
                                        scalar1=-1,
                                        scalar2=None, op0=OP.bitwise_xor)
                nc.vector.tensor_scalar(out=rig[:, :n], in0=rig[:, :n],
                                        scalar1=0x5f3759e0,
                                        scalar2=None, op0=OP.add)
                rng = ph1.tile([P, TPC], F, tag="rng")
                nc.vector.tensor_tensor(out=rng[:, :n],
                                        in0=rig[:, :n].bitcast(F),
                                        in1=rig[:, :n].bitcast(F), op=OP.mult)
                nc.vector.tensor_tensor(out=rng[:, :n], in0=rng[:, :n],
                                        in1=rsg[:, :n], op=OP.mult)
                nc.vector.tensor_scalar(out=rng[:, :n], in0=rng[:, :n],
                                        scalar1=-0.5,
                                        scalar2=1.5, op0=OP.mult, op1=OP.add)
                nc.vector.tensor_tensor(out=rsg[:, :n],
                                        in0=rig[:, :n].bitcast(F),
                                        in1=rng[:, :n], op=OP.mult)
                for i in range(n):
                    st = g0 + i
                    h_t = ph1.tile([P, D], BF, tag="ht", bufs=6)
                    nc.gpsimd.tensor_scalar(out=h_t, in0=x_ts[st],
                                            scalar1=mvg[:, i, 0:1],
                                            scalar2=rsg[:, i:i + 1],
                                            op0=OP.subtract, op1=OP.mult)
                    h_ts[st] = h_t

            def consume_trv(g0, n):
                for i in range(n):
                    sp = g0 + i
                    h_t = h_ts[sp]
                    ps_tr = ps_trp.tile([P, DT, P], BF, tag="tr")
                    for dt_ in range(DT):
                        nc.tensor.transpose(ps_tr[:, dt_, :],
                                            h_t[:, dt_ * P:(dt_ + 1) * P],
                                            ident16)
                    nc.scalar.activation(
                        out=hT8[:, :, sp * P:(sp + 1) * P],
                        in_=ps_tr, func=AF.Copy, bias=0.0, scale=1.0)
                for i in range(n):
                    sv = g0 + i
                    for lo, w, eng in ((0, 512, 0), (512, 256, 1)):
                        ps_v = ps_vp.tile([P, CH], F, tag="v")
                        for j in range(DT // 2):
                            nc.tensor.matmul(
                                ps_v[:, :w],
                                hT8[:, 2 * j:2 * j + 2, sv * P:(sv + 1) * P],
                                wv_t[:, 2 * j:2 * j + 2, lo:lo + w],
                                start=(j == 0), stop=(j == DT // 2 - 1),
                                perf_mode=DR)
                        # bv folds into bo' host-side (bo16)
                        if eng:
                            nc.scalar.activation(
                                out=v8[:, sv, lo:lo + w], in_=ps_v[:, :w],
                                func=AF.Identity, bias=0.0, scale=1.0 / SW)
                        else:
                            nc.vector.tensor_scalar(
                                out=v8[:, sv, lo:lo + w], in0=ps_v[:, :w],
                                scalar1=1.0 / SW, scalar2=None, op0=OP.mult)

            def consume_chunk(kc):
                for dtp in range(DT):
                    ps_k = ps_kp.tile([P, CH], F, tag="k")
                    for j in range(DT // 2):
                        nc.tensor.matmul(
                            ps_k,
                            wk_t[:, 2 * j:2 * j + 2, dtp * P:(dtp + 1) * P],
                            hT8[:, 2 * j:2 * j + 2, kc * CH:(kc + 1) * CH],
                            start=(j == 0), stop=(j == DT // 2 - 1),
                            perf_mode=DR)
                    if dtp % 2:
                        nc.scalar.activation(
                            out=k8[:, dtp, kc * CH:(kc + 1) * CH], in_=ps_k,
                            func=AF.Identity, bias=bk_col[:, dtp:dtp + 1],
                            scale=1.0 / SW)
                    else:
                        nc.vector.tensor_scalar(
                            out=k8[:, dtp, kc * CH:(kc + 1) * CH], in0=ps_k,
                            scalar1=1.0 / SW, scalar2=bk_col[:, dtp:dtp + 1],
                            op0=OP.mult, op1=OP.add)
                if kc == 0:
                    e8_list[0] = e8p.tile([P, ST, CH], E4, tag="e8",
                                          name="e8_c0")
                    for dtp in range(DT):
                        ps_q = ps_kp.tile([P, CH], F, tag="k", name="ps_q0")
                        for j in range(DT // 2):
                            nc.tensor.matmul(
                                ps_q,
                                wq_t[:, 2 * j:2 * j + 2,
                                     dtp * P:(dtp + 1) * P],
                                hT8[:, 2 * j:2 * j + 2, 0:CH],
                                start=(j == 0), stop=(j == DT // 2 - 1),
                                perf_mode=DR)
                        if dtp % 2:
                            nc.scalar.activation(
                                out=q8[:, dtp, 0:CH], in_=ps_q,
                                func=AF.Identity,
                                bias=bq_col[:, dtp:dtp + 1], scale=1.0 / SW)
                        else:
                            nc.vector.tensor_scalar(
                                out=q8[:, dtp, 0:CH], in0=ps_q,
                                scalar1=1.0 / SW,
                                scalar2=bq_col[:, dtp:dtp + 1],
                                op0=OP.mult, op1=OP.add)
                # chunk-0 scores for this k-chunk's keys
                for st2 in range(TPC * kc, TPC * kc + TPC):
                    ps_s = ps_kp.tile([P, CH], F, tag="k", name="ps_s0")
                    for j in range(DT // 2):
                        nc.tensor.matmul(
                            ps_s,
                            k8[:, 2 * j:2 * j + 2, st2 * P:(st2 + 1) * P],
                            q8[:, 2 * j:2 * j + 2, 0:CH],
                            start=(j == 0), stop=(j == DT // 2 - 1),
                            perf_mode=DR)
                    nc.scalar.activation(out=e8_list[0][:, st2, :], in_=ps_s,
                                         func=AF.Exp, scale=inv_sqrt_d,
                                         bias=nln4_t)

            groups = [(g0, 2) for g0 in range(0, ST, 2)]
            done_chunks = 0
            for gi in range(len(groups) + 1):
                if gi >= 1:
                    g0, n = groups[gi - 1]
                    consume_trv(g0, n)
                    full = (g0 + n) // TPC
                    while done_chunks < full:
                        consume_chunk(done_chunks)
                        done_chunks += 1
                if gi < len(groups):
                    ln1_group(*groups[gi])
            # q for chunks 1..3 (frees hT afterwards)
            for sc in range(1, NCH):
                for dtp in range(DT):
                    ps_q = ps_kp.tile([P, CH], F, tag="k")
                    for j in range(DT // 2):
                        nc.tensor.matmul(
                            ps_q,
                            wq_t[:, 2 * j:2 * j + 2, dtp * P:(dtp + 1) * P],
                            hT8[:, 2 * j:2 * j + 2, sc * CH:(sc + 1) * CH],
                            start=(j == 0), stop=(j == DT // 2 - 1),
                            perf_mode=DR)
                    if dtp % 2:
                        nc.scalar.activation(
                            out=q8[:, dtp, sc * CH:(sc + 1) * CH], in_=ps_q,
                            func=AF.Identity, bias=bq_col[:, dtp:dtp + 1],
                            scale=1.0 / SW)
                    else:
                        nc.vector.tensor_scalar(
                            out=q8[:, dtp, sc * CH:(sc + 1) * CH], in0=ps_q,
                            scalar1=1.0 / SW, scalar2=bq_col[:, dtp:dtp + 1],
                            op0=OP.mult, op1=OP.add)
        hT_ctx.close()
        wqkv_ctx.close()

        # MLP fc weights land during slot 1 (the DMA device is saturated with
        # x tiles + qkv weights during phase 1; here it idles)
        nc.sync.dma_start(wfh_t[:],
                          wfh_d.ap().rearrange("(t p) n -> p t n", p=P))
        nc.sync.dma_start(wfl_t[:],
                          wfl_d.ap().rearrange("(t p) n -> p t n", p=P))

        # MLP proj weights arrive into the space hT8/wq/wk/wv vacated
        wphp_ctx = ExitStack()
        wphp = wphp_ctx.enter_context(tc.tile_pool(name="wphp", bufs=1))
        wph_t = wphp.tile([P, HT, D], E4)
        nc.sync.dma_start(wph_t[:],
                          wph_d.ap().rearrange("(t p) n -> p t n", p=P))
        if wpl_j:
            wpl_t = wphp.tile([P, 2 * wpl_j, D], E5)
            nc.sync.dma_start(
                wpl_t[:],
                wpl_d.ap()[0:2 * wpl_j * P, :].rearrange(
                    "(t p) n -> p t n", p=P))
        if has_bias:
            nc.sync.dma_start(bo_row, bo_d.ap().unsqueeze(0))
            nc.sync.dma_start(bp_row, bp_d.ap().unsqueeze(0))

        def quake_rsqrt(pool, mvs, rss):
            # rsqrt(var+eps): quake bit-trick + 2 Newton steps, all on DVE
            vb = pool.tile([P, TPC], F, tag="vb")
            nc.vector.tensor_scalar(out=vb, in0=mvs[:, :, 1], scalar1=EPS,
                                    scalar2=None, op0=OP.add)
            ib = pool.tile([P, TPC], I32, tag="ib")
            nc.vector.tensor_scalar(out=ib, in0=vb[:].bitcast(I32),
                                    scalar1=1, scalar2=None,
                                    op0=OP.logical_shift_right)
            nc.vector.tensor_scalar(out=ib, in0=ib, scalar1=-1,
                                    scalar2=None, op0=OP.bitwise_xor)
            nc.vector.tensor_scalar(out=ib, in0=ib, scalar1=0x5f3759e0,
                                    scalar2=None, op0=OP.add)
            nc.vector.tensor_copy(out=rss, in_=ib[:].bitcast(F))
            nt = pool.tile([P, TPC], F, tag="nt")
            for _ in range(2):
                nc.vector.tensor_tensor(out=nt, in0=rss, in1=rss, op=OP.mult)
                nc.vector.tensor_tensor(out=nt, in0=nt, in1=vb, op=OP.mult)
                nc.vector.tensor_scalar(out=nt, in0=nt, scalar1=-0.5,
                                        scalar2=1.5, op0=OP.mult, op1=OP.add)
                nc.vector.tensor_tensor(out=rss, in0=rss, in1=nt, op=OP.mult)

        # ------------- Fused pipeline: slots c = 1..5 -------------
        # slot c: back(c-1) attnV/Z/o/LN2 + mlp(c-2) fc/proj + front(c) scores
        with (
            tc.tile_pool(name="pp", bufs=2) as pp,
            tc.tile_pool(name="ps_sc", bufs=2, space="PSUM") as ps_scp,
            tc.tile_pool(name="ps_av", bufs=2, space="PSUM") as ps_avp,
            tc.tile_pool(name="ps_fp", bufs=3, space="PSUM") as ps_fpp,
            tc.tile_pool(name="ps_rz", bufs=1, space="PSUM") as ps_rzp,
        ):
            for c in range(1, 6):
                b = c - 1 if c - 1 <= 3 else None      # back chunk
                m = c - 2 if 0 <= c - 2 <= 3 else None  # mlp chunk
                have_front = c <= 3

                if have_front:
                    e8_list[c] = e8p.tile([P, ST, CH], E4, tag="e8",
                                          name=f"e8_{c}")

                # ---- mlp(m) quanta ----
                fcq = []
                if m is not None:
                    hs = h2s[:, m % 2]

                    def fc_ht(ht, hs=hs):
                        ps = ps_fpp.tile([P, CH], F, tag="fp", name="u")
                        for j in range(DT // 2):
                            nc.tensor.matmul(
                                ps, wfh_t[:, 2 * j:2 * j + 2,
                                          ht * P:(ht + 1) * P],
                                hs[:, 2 * j:2 * j + 2, 0, :],
                                start=(j == 0), stop=False, perf_mode=DR)
                        for j in range(DT // 2):
                            nc.tensor.matmul(
                                ps, wfh_t[:, 2 * j:2 * j + 2,
                                          ht * P:(ht + 1) * P],
                                hs[:, 2 * j:2 * j + 2, 1, :],
                                start=False, stop=False, perf_mode=DR)
                        for j in range(DT // 2):
                            nc.tensor.matmul(
                                ps, wfl_t[:, 2 * j:2 * j + 2,
                                          ht * P:(ht + 1) * P],
                                hs[:, 2 * j:2 * j + 2, 0, :],
                                start=False, stop=(j == DT // 2 - 1),
                                perf_mode=DR)
                        m16 = pp.tile([P, CH], BF, tag="m16", bufs=3)
                        nc.scalar.activation(out=m16, in_=ps, func=AF.Gelu,
                                             bias=bfc_col[:, ht:ht + 1],
                                             scale=1.0 / SW)
                        nc.gpsimd.tensor_copy(out=ms[:, ht, 0, :], in_=m16)
                        nc.gpsimd.tensor_tensor(out=ms[:, ht, 1, :], in0=m16,
                                                in1=ms[:, ht, 0, :],
                                                op=OP.subtract)

                    fcq = [(lambda ht=ht: fc_ht(ht)) for ht in range(HT)]

                fci = [0]

                def emit_fc(n):
                    for _ in range(n):
                        if fci[0] < len(fcq):
                            fcq[fci[0]]()
                            fci[0] += 1

                def proj_su(su, seg, o_ts={}):
                    lo, w = (0, 512) if seg == 0 else (512, 256)
                    ps = ps_fpp.tile([P, CH], F, tag="fp", name="pj")
                    last = ('b' if has_bias else
                            ('l' if wpl_j else 'a'))
                    for arm in range(2):
                        for j in range(HT // 2):
                            isl = (last == 'a' and arm == 1
                                   and j == HT // 2 - 1)
                            nc.tensor.matmul(
                                ps[:, :w],
                                ms[:, 2 * j:2 * j + 2, arm,
                                   su * P:(su + 1) * P],
                                wph_t[:, 2 * j:2 * j + 2, lo:lo + w],
                                start=(arm == 0 and j == 0), stop=isl,
                                perf_mode=DR)
                    if wpl_j:
                        for j in range(wpl_j):
                            isl = (last == 'l' and j == wpl_j - 1)
                            nc.tensor.matmul(
                                ps[:, :w],
                                ms[:, 2 * j:2 * j + 2, 0,
                                   su * P:(su + 1) * P],
                                wpl_t[:, 2 * j:2 * j + 2, lo:lo + w],
                                start=False, stop=isl, perf_mode=DR)
                    if has_bias:
                        nc.tensor.matmul(ps[:, :w], ones_row,
                                         bp_row[:, lo:lo + w],
                                         start=False, stop=True)
                    st_sl = (m % 2) * TPC + su
                    if seg == 0:
                        o_ts[su] = pp.tile([P, D], F, tag="ot", bufs=2,
                                           name="o_t")
                    o_t = o_ts[su]
                    nc.scalar.activation(out=o_t[:, lo:lo + w],
                                         in_=ps[:, :w], func=AF.Identity,
                                         bias=0.0, scale=1.0 / SP)
                    nc.gpsimd.tensor_tensor(out=o_t[:, lo:lo + w],
                                            in0=o_t[:, lo:lo + w],
                                            in1=x2_sb[:, st_sl, lo:lo + w],
                                            op=OP.add)
                    # per-segment DMA shortens the end-of-kernel drain
                    st = m * TPC + su
                    nc.sync.dma_start(
                        out_d.ap()[st * P:(st + 1) * P, lo:lo + w],
                        o_t[:, lo:lo + w])

                # ---- back(b) helpers ----
                if b is not None:
                    e8b = e8_list[b]
                    xb_ts = []
                    for su in range(TPC):
                        st = b * TPC + su
                        x_t = pp.tile([P, D], F, tag="xb", bufs=4)
                        nc.sync.dma_start(x_t,
                                          x_d.ap()[st * P:(st + 1) * P, :])
                        xb_ts.append(x_t)
                    yT8 = pp.tile([P, DT, CH], E4, tag="yt", bufs=2)
                    mvs = pp.tile([P, TPC, 2], F, tag="mvs")
                    rss = pp.tile([P, TPC], F, tag="rss")
                    rz_cols = pp.tile([P, TPC], F, tag="rzc")

                    def back_attnv(dtp):
                        ps = ps_avp.tile([P, CH], F, tag="av", name="av")
                        for pr in range(ST // 2):
                            nc.tensor.matmul(
                                ps,
                                v8[:, 2 * pr:2 * pr + 2,
                                   dtp * P:(dtp + 1) * P],
                                e8b[:, 2 * pr:2 * pr + 2, :],
                                start=(pr == 0), stop=(pr == ST // 2 - 1),
                                perf_mode=DR)
                        if dtp % 2:
                            nc.scalar.activation(out=yT8[:, dtp, :], in_=ps,
                                                 func=AF.Copy, bias=0.0,
                                                 scale=1.0)
                        else:
                            nc.vector.tensor_copy(out=yT8[:, dtp, :], in_=ps)

                    def back_z_rz():
                        ps_z = ps_scp.tile([P, CH], F, tag="sc", name="z")
                        for pr in range(ST // 2):
                            nc.tensor.matmul(ps_z, ones8,
                                             e8b[:, 2 * pr:2 * pr + 2, :],
                                             start=(pr == 0),
                                             stop=(pr == ST // 2 - 1),
                                             perf_mode=DR)
                        if has_bias:
                            z_row = pp.tile([1, CH], BF, tag="zrow")
                            nc.vector.tensor_copy(out=z_row, in_=ps_z[0:1, :])
                        else:
                            z_row = None
                        rz = pp.tile([P, CH], F, tag="rz", bufs=1)
                        nc.vector.reciprocal(out=rz, in_=ps_z)
                        ps_rz = ps_rzp.tile([P, TPC], F, tag="rz")
                        for su in range(TPC):
                            # K=1 fp32 matmul: broadcast rz row -> per-token
                            # column, pre-divided by SW (invsw operand)
                            nc.tensor.matmul(
                                ps_rz[:, su:su + 1],
                                rz[0:1, su * P:(su + 1) * P],
                                invsw, start=True, stop=True)
                        nc.vector.tensor_copy(out=rz_cols, in_=ps_rz)
                        return z_row

                    def back_o(su, z_row):
                        st_sl = (b % 2) * TPC + su
                        for lo, w in ((0, 512), (512, 256)):
                            ps = ps_avp.tile([P, CH], F, tag="av", name="o")
                            for j in range(DT // 2):
                                nc.tensor.matmul(
                                    ps[:, :w],
                                    yT8[:, 2 * j:2 * j + 2,
                                        su * P:(su + 1) * P],
                                    wo_t[:, 2 * j:2 * j + 2, lo:lo + w],
                                    start=(j == 0),
                                    stop=(j == DT // 2 - 1 and
                                          not has_bias),
                                    perf_mode=DR)
                            if has_bias:
                                # bo rides as bo*Z*SW; the rz/SW scaling
                                # cancels it back to +bo
                                nc.tensor.matmul(ps[:, :w],
                                                 z_row[:, su * P:(su + 1) * P],
                                                 bo_row[:, lo:lo + w],
                                                 start=False, stop=True)
                            # whole x2 chain on DVE: mult, residual add, then
                            # stats follow in the same FIFO (no cross-engine
                            # sem latency)
                            nc.vector.tensor_scalar(
                                out=x2_sb[:, st_sl, lo:lo + w],
                                in0=ps[:, :w],
                                scalar1=rz_cols[:, su:su + 1],
                                scalar2=None, op0=OP.mult)
                            nc.vector.tensor_tensor(
                                out=x2_sb[:, st_sl, lo:lo + w],
                                in0=x2_sb[:, st_sl, lo:lo + w],
                                in1=xb_ts[su][:, lo:lo + w], op=OP.add)
                        stats = pp.tile([P, 3, 6], F, tag="st3")
                        for i in range(3):
                            nc.vector.bn_stats(
                                out=stats[:, i, :],
                                in_=x2_sb[:, st_sl, i * 256:(i + 1) * 256])
                        nc.vector.bn_aggr(out=mvs[:, su, :], in_=stats)

                    def back_ln2(su):
                        st_sl = (b % 2) * TPC + su
                        h2_t = pp.tile([P, D], BF, tag="h2", bufs=3)
                        nc.vector.tensor_scalar(out=h2_t,
                                                in0=x2_sb[:, st_sl, :],
                                                scalar1=mvs[:, su, 0:1],
                                                scalar2=rss[:, su:su + 1],
                                                op0=OP.subtract, op1=OP.mult)
                        ps_tr = ps_scp.tile([P, DT, P], BF, tag="sc",
                                            name="tr2")
                        for dt_ in range(DT):
                            nc.tensor.transpose(
                                ps_tr[:, dt_, :],
                                h2_t[:, dt_ * P:(dt_ + 1) * P], ident16)
                        hi = h2s[:, b % 2, :, 0, su * P:(su + 1) * P]
                        nc.scalar.activation(out=hi, in_=ps_tr, func=AF.Copy,
                                             bias=0.0, scale=1.0)
                        nc.vector.tensor_tensor(
                            out=h2s[:, b % 2, :, 1, su * P:(su + 1) * P],
                            in0=ps_tr, in1=hi, op=OP.subtract)

                def front_score(st2):
                    ps = ps_scp.tile([P, CH], F, tag="sc", name="s")
                    for j in range(DT // 2):
                        nc.tensor.matmul(
                            ps,
                            k8[:, 2 * j:2 * j + 2, st2 * P:(st2 + 1) * P],
                            q8[:, 2 * j:2 * j + 2, c * CH:(c + 1) * CH],
                            start=(j == 0), stop=(j == DT // 2 - 1),
                            perf_mode=DR)
                    nc.scalar.activation(out=e8_list[c][:, st2, :], in_=ps,
                                         func=AF.Exp, scale=inv_sqrt_d,
                                         bias=nln4_t)

                # ---- slot emission ----
                sci = [0]

                def emit_sc(n):
                    for _ in range(n):
                        if have_front and sci[0] < ST:
                            front_score(sci[0])
                            sci[0] += 1

                # in MLP-less slots the scores weave early so their exps
                # (ACT) finish before the next slot's gelus queue behind them
                early_sc = (m is None)
                if m is not None:
                    # prefetch the gelu act-table while the previous slot's
                    # exps drain, so gelu(0) doesn't stall the fc stream
                    nc.scalar.activation(out=warm, in_=warm, func=AF.Gelu,
                                         bias=0.0, scale=1.0)
                if b is not None:
                    # stage 1: attnV woven with fc
                    for dtp in range(DT):
                        if early_sc:
                            emit_sc(2)
                        back_attnv(dtp)
                        emit_fc(1)
                    emit_fc(2)
                    # stage 2: Z + rz, more fc
                    if early_sc:
                        emit_sc(2)
                    z_row = back_z_rz()
                    emit_fc(4)
                    # stage 3: o-proj + x2 + stats, more fc
                    for su in range(TPC):
                        if early_sc:
                            emit_sc(1)
                        back_o(su, z_row)
                        emit_fc(2)
                    # stage 4: LN2 apply + transposes + h2s
                    quake_rsqrt(pp, mvs, rss)
                    for su in range(TPC):
                        back_ln2(su)
                        emit_fc(1)
                emit_fc(HT)  # any leftovers (and the c=5 no-back slot)
                # stage 5: proj woven with next chunk's scores+exps
                if m is not None:
                    for su in range(TPC):
                        for seg in range(2):
                            proj_su(su, seg)
                            emit_sc(2)
                emit_sc(ST)
        wphp_ctx.close()
        e8_ctx.close()
        qkv_ctx.close()

    return nc


def _get_nc():
    key = _CACHE.get("key")
    if "nc" not in _CACHE or key != (_CACHE.get("has_bias"), WPL_J):
        has_bias = _CACHE.get("has_bias", False)
        nc = _build(has_bias, WPL_J)
        nc.compile()
        _CACHE["nc"] = nc
        _CACHE["key"] = (has_bias, WPL_J)
    return _CACHE["nc"]


TRACE = False


def kernel(**inputs):
    from concourse.bass_utils import run_bass_kernel_spmd

    x = np.asarray(inputs["x"], dtype=np.float32)
    base = _prep(inputs)
    _CACHE["has_bias"] = base.pop("_has_bias")
    nc = _get_nc()
    names = {"wq8", "wk8", "wv8", "wo8", "wfh", "wfl", "wph",
             "bq_", "bk_", "bfc_"}
    if WPL_J:
        names.add("wpl")
    if _CACHE["has_bias"]:
        names.add("bo16")
        names.add("bp16")
    ship = {k: v for k, v in base.items() if k in names}
    in_maps = [dict(ship, x=np.ascontiguousarray(x[bb]))
               for bb in range(N_CORES)]
    res = run_bass_kernel_spmd(nc, in_maps, core_ids=list(range(N_CORES)),
                               trace=TRACE)
    _CACHE["last_res"] = res
    return np.stack([res.results[bb]["out"] for bb in range(N_CORES)], axis=0)


# revision 44
# speedup vs baseline: 1.2343x; 1.0181x over previous
"""Trainium2 Bass kernel for a dense transformer block (B=8, S=2048, D=768, H=3072).

Sharding: pure data-parallel over batch -- one batch element per NeuronCore.

All heavy matmuls are fp8-e4m3 MatmulPerfMode.DoubleRow (K=256/instruction at
0.5 cycles/row). Cost model: matmul time = out_free_size * cycles_per_row, so
total PE time ~ (#contraction passes) x (output width); arms on the MLP
matmuls are the dominant cost.

Numerics (rel_absmax gate 2e-2; emulated 1.5e-2, baseline was 6.1e-3):
  - weights are pre-scaled by powers of two (SW=16 for the D-side mats,
    SP=32 for Wproj) so their entries sit in e4m3's normal range instead of
    half-denormal; the inverse scales ride existing activation scale operands
    (q/k/v/gelu) or the rz-extraction matmul (o-proj) for free.
  - attention path (q/k/v/scores/exp/y/o) is plain e4m3.
  - fc keeps 3 arms: Wfh*(hi+lo) + Wfl*hi (h2 split hi/lo e4m3; Wfl e5m2
    residual). proj runs 2 arms: Wph*(mhi+mlo) -- the Wpl residual arm is
    dropped (WPL_J can partially restore it).
  - exp is computed as exp(s/sqrt(d) - 3ln2) = e/8 so the unnormalized
    attention accumulator stays below e4m3's 240 max; 1/Z (and the 1/SW
    unscale) is applied per-token on the x2 write via a K=1 fp32 matmul that
    extracts rz columns (replaces the old fp32 transposes).
  - when all matmul biases fold to zero (true for this reference: betas and
    biases are zeros) the bias matmuls are omitted entirely at build time.

Schedule: a 3-stage software pipeline over the 4 token chunks, fused across
the old attention/MLP phase boundary so the MLP's PE-dense stream fills the
stalls of the attention dependency chains:
  slot c emits: attnV/Z/o-proj/LN2 of chunk c-1, fc+proj of chunk c-2, and
  scores+exp of chunk c (last, so the ACT engine does gelus first and exps
  at the end -- Exp and Gelu live in different activation-table sets, so this
  ordering costs only 2 table loads per slot).
x2 and h2s are rolling 2-chunk buffers; ms single-chunk. PSUM: 3 banks for
scores/Z/LN2-transposes, 2 for attnV/o-proj, 2 for fc/proj, 1 for rz.
"""

import numpy as np

P = 128
S, D, H = 2048, 768, 3072
DT = D // P            # 6 d-tiles
HT = H // P            # 24 h-tiles
ST = S // P            # 16 token tiles
CH = 512               # chunk width (tokens)
NCH = S // CH          # 4 chunks
TPC = CH // P          # 4 token tiles per chunk
EPS = 1e-5
N_CORES = 8
LN4 = 2.0794415416798357   # 3*ln2; exp bias so e8 = exp(s)/8
SW = 16.0                  # scale for D-side weight mats (sigma ~0.036)
SP = 32.0                  # scale for Wproj (sigma ~0.018)
WPL_J = 0                  # 0..12: partial Wpl residual passes (accuracy knob)

WEIGHT_NAMES = [
    "ln1_g", "ln1_b", "ln2_g", "ln2_b",
    "Wq", "bq", "Wk", "bk", "Wv", "bv", "Wo", "bo",
    "Wfc", "bfc", "Wproj", "bproj",
]

_CACHE = {}


def _prep(inputs):
    """Host-side weight quantization + LN/bias folding (pure numpy)."""
    import ml_dtypes
    E4, E5, BF = ml_dtypes.float8_e4m3, ml_dtypes.float8_e5m2, ml_dtypes.bfloat16
    f32 = lambda k: np.asarray(inputs[k], dtype=np.float32)
    g1, b1 = f32("ln1_g"), f32("ln1_b")
    g2, b2 = f32("ln2_g"), f32("ln2_b")
    Wq, Wk, Wv, Wo = f32("Wq"), f32("Wk"), f32("Wv"), f32("Wo")
    Wfc, Wproj = f32("Wfc"), f32("Wproj")
    q8 = lambda a: np.ascontiguousarray(a.astype(E4))
    Wfc_g = g2[:, None] * Wfc * SW
    wfh = Wfc_g.astype(E4)
    wph_f = Wproj * SP
    wph = wph_f.astype(E4)
    bo_f = f32("bo") + (f32("bv") + b1 @ Wv) @ Wo
    bp_f = f32("bproj")
    bq_ = f32("bq") + b1 @ Wq
    bk_ = f32("bk") + b1 @ Wk
    # scores = (h g1 Wq)(h g1 Wk)^T = h M h^T with M precomputed host-side:
    # the whole k tensor never materializes on device.
    M = g1[:, None] * (Wq @ Wk.T) * g1[None, :]
    # per-query bias shift q.bk cancels in softmax; per-key shift bq.k is
    # the rank-1 term h.(g1 Wk bq) riding the exp bias; bq.bk is a constant
    rk = g1 * (Wk @ bq_)
    out = {
        "wm8": q8(SW * M),
        "wv8": q8(SW * g1[:, None] * Wv), "wo8": q8(SW * Wo),
        "rk8": q8(SW * rk),
        "sb": np.full((P,), float(bq_ @ bk_) / np.sqrt(np.float32(D)) - LN4,
                      dtype=np.float32),
        "wfh": np.ascontiguousarray(wfh),
        "wfl": np.ascontiguousarray(
            (Wfc_g - wfh.astype(np.float32)).astype(E5)),
        "wph": np.ascontiguousarray(wph),
        "wpl": np.ascontiguousarray(
            (wph_f - wph.astype(np.float32)).astype(E5)),
        "bfc_": f32("bfc") + b2 @ Wfc,
        "bo16": np.asarray(SW * bo_f, dtype=BF),
        "bp16": np.asarray(SP * bp_f, dtype=BF),
    }
    out["_has_bias"] = bool(np.any(bo_f != 0.0) or np.any(bp_f != 0.0)
                            or np.any(bq_ != 0.0) or np.any(bk_ != 0.0))
    return out


def _build(has_bias, wpl_j):
    import concourse.bass as bass
    import concourse.tile as tile
    from concourse import bacc, mybir
    from concourse.masks import make_identity
    from contextlib import ExitStack

    F = mybir.dt.float32
    BF = mybir.dt.bfloat16
    E4 = mybir.dt.float8e4
    E5 = mybir.dt.float8e5
    I32 = mybir.dt.int32
    AF = mybir.ActivationFunctionType
    OP = mybir.AluOpType
    DR = mybir.MatmulPerfMode.DoubleRow

    nc = bacc.Bacc(None, target_bir_lowering=False)

    x_d = nc.dram_tensor("x", [S, D], F, kind="ExternalInput")
    out_d = nc.dram_tensor("out", [S, D], F, kind="ExternalOutput")
    wm8_d = nc.dram_tensor("wm8", [D, D], E4, kind="ExternalInput")
    wv8_d = nc.dram_tensor("wv8", [D, D], E4, kind="ExternalInput")
    wo8_d = nc.dram_tensor("wo8", [D, D], E4, kind="ExternalInput")
    wfh_d = nc.dram_tensor("wfh", [D, H], E4, kind="ExternalInput")
    wfl_d = nc.dram_tensor("wfl", [D, H], E5, kind="ExternalInput")
    wph_d = nc.dram_tensor("wph", [H, D], E4, kind="ExternalInput")
    bfc_d = nc.dram_tensor("bfc_", [H], F, kind="ExternalInput")
    if wpl_j:
        wpl_d = nc.dram_tensor("wpl", [H, D], E5, kind="ExternalInput")
    if has_bias:
        bo_d = nc.dram_tensor("bo16", [D], BF, kind="ExternalInput")
        bp_d = nc.dram_tensor("bp16", [D], BF, kind="ExternalInput")
        rk_d = nc.dram_tensor("rk8", [D], E4, kind="ExternalInput")
        sb_d = nc.dram_tensor("sb", [P], F, kind="ExternalInput")

    inv_sqrt_d = 1.0 / float(np.sqrt(np.float32(D)))

    with tile.TileContext(nc) as tc, ExitStack() as ctx:
        singles = ctx.enter_context(tc.tile_pool(name="singles", bufs=1))

        ident16 = singles.tile([P, P], BF)
        make_identity(nc, ident16)
        warm = singles.tile([1, 1], F)
        nc.vector.memset(warm, 0.0)
        nln4_t = singles.tile([P, 1], F)
        nc.vector.memset(nln4_t, -LN4)
        invsw = singles.tile([1, 1], F)
        nc.vector.memset(invsw, 1.0 / SW)
        ones8 = singles.tile([P, 2, P], E4)
        nc.vector.memset(ones8, 1.0)
        bfc_col = singles.tile([P, HT], F)
        if has_bias:
            ones_row = singles.tile([1, P], BF)
            nc.vector.memset(ones_row, 1.0)
            bo_row = singles.tile([1, D], BF)
            bp_row = singles.tile([1, D], BF)
            rk_t = singles.tile([P, DT, 1], E4)
            sb_col = singles.tile([P, 1], F)

        # persistent activations
        perm = ctx.enter_context(tc.tile_pool(name="perm", bufs=1))
        x2_sb = perm.tile([P, 2 * TPC, D], F)       # rolling 2-chunk residual
        h2s = perm.tile([P, 2, DT, 2, CH], E4)      # rolling LN2 out hi/lo
        ms = perm.tile([P, HT, 2, CH], E4)          # gelu out hi/lo (1 chunk)

        wfhp = ctx.enter_context(tc.tile_pool(name="wfhp", bufs=1))
        wfh_t = wfhp.tile([P, DT, H], E4)
        wflp = ctx.enter_context(tc.tile_pool(name="wflp", bufs=1))
        wfl_t = wflp.tile([P, DT, H], E5)

        qkv_ctx = ExitStack()
        qkvp = qkv_ctx.enter_context(tc.tile_pool(name="qkv", bufs=1))
        hT8 = qkvp.tile([P, DT, S], E4)      # LN1 out, d-major (scores lhsT)
        g8 = qkvp.tile([P, DT, S], E4)       # h @ M, d-major (scores rhs)
        v8 = qkvp.tile([P, ST, D], E4)
        wo_t = qkvp.tile([P, DT, D], E4)
        wm8_t = qkvp.tile([P, DT, D], E4)

        e8_ctx = ExitStack()
        e8p = e8_ctx.enter_context(tc.tile_pool(name="e8p", bufs=2))
        e8_list = [None] * NCH

        wqkv_ctx = ExitStack()
        wqkv = wqkv_ctx.enter_context(tc.tile_pool(name="wqkv", bufs=1))
        wv_t = wqkv.tile([P, DT, D], E4)

        # ------------- Phase 1: LN1 -> hT8; v, k, q (all fp8) -------------
        with (
            tc.tile_pool(name="ph1", bufs=3) as ph1,
            tc.tile_pool(name="ps_tr", bufs=2, space="PSUM") as ps_trp,
            tc.tile_pool(name="ps_k", bufs=3, space="PSUM") as ps_kp,
            tc.tile_pool(name="ps_v", bufs=3, space="PSUM") as ps_vp,
        ):
            x_ts = [None] * ST
            h_ts = [None] * ST
            # trigger the exp-set act table load at t=0, off the critical path
            nc.scalar.activation(out=warm, in_=warm, func=AF.Exp,
                                 bias=0.0, scale=1.0)
            # qkv weights ride the otherwise-idle ACT DMA queue; x tiles
            # alternate SP/Pool so nothing queues behind a weight transfer
            nc.scalar.dma_start(
                wv_t[:], wv8_d.ap().rearrange("(t p) n -> p t n", p=P))
            nc.scalar.dma_start(
                wm8_t[:], wm8_d.ap().rearrange("(t p) n -> p t n", p=P))

            def ln1_group(g0, n):
                mvg = ph1.tile([P, TPC, 2], F, tag="mvg")
                for i in range(n):
                    st = g0 + i
                    x_t = ph1.tile([P, D], F, tag="xt", bufs=6)
                    q = nc.gpsimd if st % 2 else nc.sync
                    q.dma_start(out=x_t, in_=x_d.ap()[st * P:(st + 1) * P, :])
                    if st == 9:
                        nc.sync.dma_start(
                            wo_t[:],
                            wo8_d.ap().rearrange("(t p) n -> p t n", p=P))
                        nc.sync.dma_start(
                            bfc_col, bfc_d.ap().rearrange("(t p) -> p t", p=P))
                    x_ts[st] = x_t
                    stats = ph1.tile([P, 2, 6], F, tag="st")
                    nc.vector.bn_stats(out=stats[:, 0, :], in_=x_t[:, :512])
                    nc.vector.bn_stats(out=stats[:, 1, :], in_=x_t[:, 512:])
                    nc.vector.bn_aggr(out=mvg[:, i, :], in_=stats)
                # batched rsqrt(var+eps): quake bit-trick + 1 Newton, on Pool
                # so the DVE stats stream isn't serialized behind it
                rsg = ph1.tile([P, TPC], F, tag="rsg")
                nc.gpsimd.tensor_scalar(out=rsg[:, :n], in0=mvg[:, :n, 1],
                                        scalar1=EPS, scalar2=None, op0=OP.add)
                rig = ph1.tile([P, TPC], I32, tag="rig")
                nc.gpsimd.tensor_scalar(out=rig[:, :n],
                                        in0=rsg[:, :n].bitcast(I32),
                                        scalar1=1, scalar2=None,
                                        op0=OP.logical_shift_right)
                nc.gpsimd.tensor_scalar(out=rig[:, :n], in0=rig[:, :n],
                                        scalar1=-1,
                                        scalar2=None, op0=OP.bitwise_xor)
                nc.gpsimd.tensor_scalar(out=rig[:, :n], in0=rig[:, :n],
                                        scalar1=0x5f3759e0,
                                        scalar2=None, op0=OP.add)
                rng = ph1.tile([P, TPC], F, tag="rng")
                nc.gpsimd.tensor_tensor(out=rng[:, :n],
                                        in0=rig[:, :n].bitcast(F),
                                        in1=rig[:, :n].bitcast(F), op=OP.mult)
                nc.gpsimd.tensor_tensor(out=rng[:, :n], in0=rng[:, :n],
                                        in1=rsg[:, :n], op=OP.mult)
                nc.gpsimd.tensor_scalar(out=rng[:, :n], in0=rng[:, :n],
                                        scalar1=-0.5,
                                        scalar2=1.5, op0=OP.mult, op1=OP.add)
                nc.gpsimd.tensor_tensor(out=rsg[:, :n],
                                        in0=rig[:, :n].bitcast(F),
                                        in1=rng[:, :n], op=OP.mult)
                for i in range(n):
                    st = g0 + i
                    h_t = ph1.tile([P, D], BF, tag="ht", bufs=6)
                    nc.gpsimd.tensor_scalar(out=h_t, in0=x_ts[st],
                                            scalar1=mvg[:, i, 0:1],
                                            scalar2=rsg[:, i:i + 1],
                                            op0=OP.subtract, op1=OP.mult)
                    h_ts[st] = h_t

            def consume_trv(g0, n):
                for i in range(n):
                    sp = g0 + i
                    h_t = h_ts[sp]
                    ps_tr = ps_trp.tile([P, DT, P], BF, tag="tr")
                    for dt_ in range(DT):
                        nc.tensor.transpose(ps_tr[:, dt_, :],
                                            h_t[:, dt_ * P:(dt_ + 1) * P],
                                            ident16)
                    if sp % 2:
                        nc.scalar.activation(
                            out=hT8[:, :, sp * P:(sp + 1) * P],
                            in_=ps_tr, func=AF.Copy, bias=0.0, scale=1.0)
                    else:
                        nc.vector.tensor_copy(
                            out=hT8[:, :, sp * P:(sp + 1) * P], in_=ps_tr)
                for i in range(n):
                    sv = g0 + i
                    for lo, w, eng in ((0, 512, 0), (512, 256, 1)):
                        ps_v = ps_vp.tile([P, CH], F, tag="v")
                        for j in range(DT // 2):
                            nc.tensor.matmul(
                                ps_v[:, :w],
                                hT8[:, 2 * j:2 * j + 2, sv * P:(sv + 1) * P],
                                wv_t[:, 2 * j:2 * j + 2, lo:lo + w],
                                start=(j == 0), stop=(j == DT // 2 - 1),
                                perf_mode=DR)
                        # bv folds into bo' host-side (bo16)
                        if eng:
                            nc.scalar.activation(
                                out=v8[:, sv, lo:lo + w], in_=ps_v[:, :w],
                                func=AF.Identity, bias=0.0, scale=1.0 / SW)
                        else:
                            nc.vector.tensor_scalar(
                                out=v8[:, sv, lo:lo + w], in0=ps_v[:, :w],
                                scalar1=1.0 / SW, scalar2=None, op0=OP.mult)

            def g_chunk(gc, pool, tag):
                # g = h @ M for one 512-token chunk (the q of the folded
                # score form); copies alternate ACT/DVE
                for dtp in range(DT):
                    ps_q = pool.tile([P, CH], F, tag=tag, name="ps_g")
                    for j in range(DT // 2):
                        nc.tensor.matmul(
                            ps_q,
                            wm8_t[:, 2 * j:2 * j + 2, dtp * P:(dtp + 1) * P],
                            hT8[:, 2 * j:2 * j + 2, gc * CH:(gc + 1) * CH],
                            start=(j == 0), stop=(j == DT // 2 - 1),
                            perf_mode=DR)
                    if dtp % 2:
                        nc.scalar.activation(
                            out=g8[:, dtp, gc * CH:(gc + 1) * CH], in_=ps_q,
                            func=AF.Identity, bias=0.0, scale=1.0 / SW)
                    else:
                        nc.vector.tensor_scalar(
                            out=g8[:, dtp, gc * CH:(gc + 1) * CH], in0=ps_q,
                            scalar1=1.0 / SW, scalar2=None, op0=OP.mult)

            def consume_chunk(kc):
                for i in range(TPC):
                    sv = TPC * kc + i
                    for lo, w, eng in ((0, 512, 0), (512, 256, 1)):
                        ps_v = ps_vp.tile([P, CH], F, tag="v")
                        for j in range(DT // 2):
                            nc.tensor.matmul(
                                ps_v[:, :w],
                                hT8[:, 2 * j:2 * j + 2, sv * P:(sv + 1) * P],
                                wv_t[:, 2 * j:2 * j + 2, lo:lo + w],
                                start=(j == 0), stop=(j == DT // 2 - 1),
                                perf_mode=DR)
                        # bv folds into bo' host-side (bo16)
                        if eng:
                            nc.scalar.activation(
                                out=v8[:, sv, lo:lo + w], in_=ps_v[:, :w],
                                func=AF.Identity, bias=0.0, scale=1.0 / SW)
                        else:
                            nc.vector.tensor_scalar(
                                out=v8[:, sv, lo:lo + w], in0=ps_v[:, :w],
                                scalar1=1.0 / SW, scalar2=None, op0=OP.mult)
                if kc == 0:
                    e8_list[0] = e8p.tile([P, ST, CH], E4, tag="e8",
                                          name="e8_c0")
                    g_chunk(0, ps_kp, "k")
                # chunk-0 scores for this k-chunk's keys
                for st2 in range(TPC * kc, TPC * kc + TPC):
                    ps_s = ps_kp.tile([P, CH], F, tag="k", name="ps_s0")
                    for j in range(DT // 2):
                        nc.tensor.matmul(
                            ps_s,
                            hT8[:, 2 * j:2 * j + 2, st2 * P:(st2 + 1) * P],
                            g8[:, 2 * j:2 * j + 2, 0:CH],
                            start=(j == 0), stop=(j == DT // 2 - 1),
                            perf_mode=DR)
                    nc.scalar.activation(out=e8_list[0][:, st2, :], in_=ps_s,
                                         func=AF.Exp, scale=inv_sqrt_d,
                                         bias=nln4_t)

            groups = [(g0, 2) for g0 in range(0, ST, 2)]
            done_chunks = 0
            for gi in range(len(groups) + 1):
                if gi >= 1:
                    g0, n = groups[gi - 1]
                    consume_trv(g0, n)
                    full = (g0 + n) // TPC
                    while done_chunks < full:
                        consume_chunk(done_chunks)
                        done_chunks += 1
                if gi < len(groups):
                    ln1_group(*groups[gi])
            # g for chunk 1 (slot 1's scores weave early and need it);
            # chunks 2-3 are produced inside slots 2-3
            g_chunk(1, ps_kp, "k")
        wqkv_ctx.close()

        # MLP fc weights land during slot 1 (the DMA device is saturated with
        # x tiles + qkv weights during phase 1; here it idles)
        nc.sync.dma_start(wfh_t[:],
                          wfh_d.ap().rearrange("(t p) n -> p t n", p=P))
        nc.sync.dma_start(wfl_t[:],
                          wfl_d.ap().rearrange("(t p) n -> p t n", p=P))

        # MLP proj weights arrive into the space hT8/wq/wk/wv vacated
        wphp_ctx = ExitStack()
        wphp = wphp_ctx.enter_context(tc.tile_pool(name="wphp", bufs=1))
        wph_t = wphp.tile([P, HT, D], E4)
        nc.sync.dma_start(wph_t[:],
                          wph_d.ap().rearrange("(t p) n -> p t n", p=P))
        if wpl_j:
            wpl_t = wphp.tile([P, 2 * wpl_j, D], E5)
            nc.sync.dma_start(
                wpl_t[:],
                wpl_d.ap()[0:2 * wpl_j * P, :].rearrange(
                    "(t p) n -> p t n", p=P))
        if has_bias:
            nc.sync.dma_start(bo_row, bo_d.ap().unsqueeze(0))
            nc.sync.dma_start(bp_row, bp_d.ap().unsqueeze(0))
            nc.sync.dma_start(
                rk_t, rk_d.ap().rearrange("(t p) -> p t", p=P).unsqueeze(2))
            nc.sync.dma_start(sb_col, sb_d.ap().unsqueeze(1))

        def quake_rsqrt(pool, mvs, rss):
            # rsqrt(var+eps): quake bit-trick + 2 Newton steps, all on DVE
            vb = pool.tile([P, TPC], F, tag="vb")
            nc.vector.tensor_scalar(out=vb, in0=mvs[:, :, 1], scalar1=EPS,
                                    scalar2=None, op0=OP.add)
            ib = pool.tile([P, TPC], I32, tag="ib")
            nc.vector.tensor_scalar(out=ib, in0=vb[:].bitcast(I32),
                                    scalar1=1, scalar2=None,
                                    op0=OP.logical_shift_right)
            nc.vector.tensor_scalar(out=ib, in0=ib, scalar1=-1,
                                    scalar2=None, op0=OP.bitwise_xor)
            nc.vector.tensor_scalar(out=ib, in0=ib, scalar1=0x5f3759e0,
                                    scalar2=None, op0=OP.add)
            nc.vector.tensor_copy(out=rss, in_=ib[:].bitcast(F))
            nt = pool.tile([P, TPC], F, tag="nt")
            for _ in range(2):
                nc.vector.tensor_tensor(out=nt, in0=rss, in1=rss, op=OP.mult)
                nc.vector.tensor_tensor(out=nt, in0=nt, in1=vb, op=OP.mult)
                nc.vector.tensor_scalar(out=nt, in0=nt, scalar1=-0.5,
                                        scalar2=1.5, op0=OP.mult, op1=OP.add)
                nc.vector.tensor_tensor(out=rss, in0=rss, in1=nt, op=OP.mult)

        # ------------- Fused pipeline: slots c = 1..5 -------------
        # slot c: back(c-1) attnV/Z/o/LN2 + mlp(c-2) fc/proj + front(c) scores
        with (
            tc.tile_pool(name="pp", bufs=2) as pp,
            tc.tile_pool(name="ps_sc", bufs=2, space="PSUM") as ps_scp,
            tc.tile_pool(name="ps_av", bufs=2, space="PSUM") as ps_avp,
            tc.tile_pool(name="ps_fp", bufs=3, space="PSUM") as ps_fpp,
            tc.tile_pool(name="ps_rz", bufs=1, space="PSUM") as ps_rzp,
        ):
            for c in range(1, 6):
                b = c - 1 if c - 1 <= 3 else None      # back chunk
                m = c - 2 if 0 <= c - 2 <= 3 else None  # mlp chunk
                have_front = c <= 3

                if have_front:
                    e8_list[c] = e8p.tile([P, ST, CH], E4, tag="e8",
                                          name=f"e8_{c}")

                # ---- mlp(m) quanta ----
                fcq = []
                if m is not None:
                    hs = h2s[:, m % 2]

                    def fc_ht(ht, hs=hs):
                        ps = ps_fpp.tile([P, CH], F, tag="fp", name="u")
                        for j in range(DT // 2):
                            nc.tensor.matmul(
                                ps, wfh_t[:, 2 * j:2 * j + 2,
                                          ht * P:(ht + 1) * P],
                                hs[:, 2 * j:2 * j + 2, 0, :],
                                start=(j == 0), stop=False, perf_mode=DR)
                        for j in range(DT // 2):
                            nc.tensor.matmul(
                                ps, wfh_t[:, 2 * j:2 * j + 2,
                                          ht * P:(ht + 1) * P],
                                hs[:, 2 * j:2 * j + 2, 1, :],
                                start=False, stop=False, perf_mode=DR)
                        for j in range(DT // 2):
                            nc.tensor.matmul(
                                ps, wfl_t[:, 2 * j:2 * j + 2,
                                          ht * P:(ht + 1) * P],
                                hs[:, 2 * j:2 * j + 2, 0, :],
                                start=False, stop=(j == DT // 2 - 1),
                                perf_mode=DR)
                        m16 = pp.tile([P, CH], BF, tag="m16", bufs=2)
                        nc.scalar.activation(out=m16, in_=ps, func=AF.Gelu,
                                             bias=bfc_col[:, ht:ht + 1],
                                             scale=1.0 / SW)
                        nc.gpsimd.tensor_copy(out=ms[:, ht, 0, :], in_=m16)
                        nc.gpsimd.tensor_tensor(out=ms[:, ht, 1, :], in0=m16,
                                                in1=ms[:, ht, 0, :],
                                                op=OP.subtract)

                    fcq = [(lambda ht=ht: fc_ht(ht)) for ht in range(HT)]

                fci = [0]

                def emit_fc(n):
                    for _ in range(n):
                        if fci[0] < len(fcq):
                            fcq[fci[0]]()
                            fci[0] += 1

                def proj_su(su, seg, o_ts={}):
                    lo, w = (0, 512) if seg == 0 else (512, 256)
                    ps = ps_fpp.tile([P, CH], F, tag="fp", name="pj")
                    last = ('b' if has_bias else
                            ('l' if wpl_j else 'a'))
                    for arm in range(2):
                        for j in range(HT // 2):
                            isl = (last == 'a' and arm == 1
                                   and j == HT // 2 - 1)
                            nc.tensor.matmul(
                                ps[:, :w],
                                ms[:, 2 * j:2 * j + 2, arm,
                                   su * P:(su + 1) * P],
                                wph_t[:, 2 * j:2 * j + 2, lo:lo + w],
                                start=(arm == 0 and j == 0), stop=isl,
                                perf_mode=DR)
                    if wpl_j:
                        for j in range(wpl_j):
                            isl = (last == 'l' and j == wpl_j - 1)
                            nc.tensor.matmul(
                                ps[:, :w],
                                ms[:, 2 * j:2 * j + 2, 0,
                                   su * P:(su + 1) * P],
                                wpl_t[:, 2 * j:2 * j + 2, lo:lo + w],
                                start=False, stop=isl, perf_mode=DR)
                    if has_bias:
                        nc.tensor.matmul(ps[:, :w], ones_row,
                                         bp_row[:, lo:lo + w],
                                         start=False, stop=True)
                    st_sl = (m % 2) * TPC + su
                    if seg == 0:
                        o_ts[su] = pp.tile([P, D], F, tag="ot", bufs=2,
                                           name="o_t")
                    o_t = o_ts[su]
                    nc.scalar.activation(out=o_t[:, lo:lo + w],
                                         in_=ps[:, :w], func=AF.Identity,
                                         bias=0.0, scale=1.0 / SP)
                    nc.gpsimd.tensor_tensor(out=o_t[:, lo:lo + w],
                                            in0=o_t[:, lo:lo + w],
                                            in1=x2_sb[:, st_sl, lo:lo + w],
                                            op=OP.add)
                    # per-segment DMA shortens the end-of-kernel drain
                    st = m * TPC + su
                    nc.sync.dma_start(
                        out_d.ap()[st * P:(st + 1) * P, lo:lo + w],
                        o_t[:, lo:lo + w])

                # ---- back(b) helpers ----
                if b is not None:
                    e8b = e8_list[b]
                    xb_ts = []
                    for su in range(TPC):
                        st = b * TPC + su
                        x_t = pp.tile([P, D], F, tag="xb", bufs=4)
                        nc.sync.dma_start(x_t,
                                          x_d.ap()[st * P:(st + 1) * P, :])
                        xb_ts.append(x_t)
                    yT8 = pp.tile([P, DT, CH], E4, tag="yt", bufs=2)
                    mvs = pp.tile([P, TPC, 2], F, tag="mvs")
                    rss = pp.tile([P, TPC], F, tag="rss")
                    rz_cols = pp.tile([P, TPC], F, tag="rzc")

                    def back_attnv(dtp):
                        ps = ps_avp.tile([P, CH], F, tag="av", name="av")
                        for pr in range(ST // 2):
                            nc.tensor.matmul(
                                ps,
                                v8[:, 2 * pr:2 * pr + 2,
                                   dtp * P:(dtp + 1) * P],
                                e8b[:, 2 * pr:2 * pr + 2, :],
                                start=(pr == 0), stop=(pr == ST // 2 - 1),
                                perf_mode=DR)
                        if dtp % 2:
                            nc.scalar.activation(out=yT8[:, dtp, :], in_=ps,
                                                 func=AF.Copy, bias=0.0,
                                                 scale=1.0)
                        else:
                            nc.vector.tensor_copy(out=yT8[:, dtp, :], in_=ps)

                    def back_z_rz():
                        ps_z = ps_scp.tile([P, CH], F, tag="sc", name="z")
                        for pr in range(ST // 2):
                            nc.tensor.matmul(ps_z, ones8,
                                             e8b[:, 2 * pr:2 * pr + 2, :],
                                             start=(pr == 0),
                                             stop=(pr == ST // 2 - 1),
                                             perf_mode=DR)
                        if has_bias:
                            z_row = pp.tile([1, CH], BF, tag="zrow")
                            nc.vector.tensor_copy(out=z_row, in_=ps_z[0:1, :])
                        else:
                            z_row = None
                        rz = pp.tile([P, CH], F, tag="rz", bufs=1)
                        nc.vector.reciprocal(out=rz, in_=ps_z)
                        ps_rz = ps_rzp.tile([P, TPC], F, tag="rz")
                        for su in range(TPC):
                            # K=1 fp32 matmul: broadcast rz row -> per-token
                            # column, pre-divided by SW (invsw operand)
                            nc.tensor.matmul(
                                ps_rz[:, su:su + 1],
                                rz[0:1, su * P:(su + 1) * P],
                                invsw, start=True, stop=True)
                        nc.vector.tensor_copy(out=rz_cols, in_=ps_rz)
                        return z_row

                    def back_o(su, z_row):
                        st_sl = (b % 2) * TPC + su
                        for lo, w in ((0, 512), (512, 256)):
                            ps = ps_avp.tile([P, CH], F, tag="av", name="o")
                            for j in range(DT // 2):
                                nc.tensor.matmul(
                                    ps[:, :w],
                                    yT8[:, 2 * j:2 * j + 2,
                                        su * P:(su + 1) * P],
                                    wo_t[:, 2 * j:2 * j + 2, lo:lo + w],
                                    start=(j == 0),
                                    stop=(j == DT // 2 - 1 and
                                          not has_bias),
                                    perf_mode=DR)
                            if has_bias:
                                # bo rides as bo*Z*SW; the rz/SW scaling
                                # cancels it back to +bo
                                nc.tensor.matmul(ps[:, :w],
                                                 z_row[:, su * P:(su + 1) * P],
                                                 bo_row[:, lo:lo + w],
                                                 start=False, stop=True)
                            # whole x2 chain on DVE: mult, residual add, then
                            # stats follow in the same FIFO (no cross-engine
                            # sem latency)
                            nc.vector.tensor_scalar(
                                out=x2_sb[:, st_sl, lo:lo + w],
                                in0=ps[:, :w],
                                scalar1=rz_cols[:, su:su + 1],
                                scalar2=None, op0=OP.mult)
                            nc.vector.tensor_tensor(
                                out=x2_sb[:, st_sl, lo:lo + w],
                                in0=x2_sb[:, st_sl, lo:lo + w],
                                in1=xb_ts[su][:, lo:lo + w], op=OP.add)
                        stats = pp.tile([P, 3, 6], F, tag="st3")
                        for i in range(3):
                            nc.vector.bn_stats(
                                out=stats[:, i, :],
                                in_=x2_sb[:, st_sl, i * 256:(i + 1) * 256])
                        nc.vector.bn_aggr(out=mvs[:, su, :], in_=stats)

                    def back_ln2(su):
                        st_sl = (b % 2) * TPC + su
                        h2_t = pp.tile([P, D], BF, tag="h2", bufs=2)
                        nc.vector.tensor_scalar(out=h2_t,
                                                in0=x2_sb[:, st_sl, :],
                                                scalar1=mvs[:, su, 0:1],
                                                scalar2=rss[:, su:su + 1],
                                                op0=OP.subtract, op1=OP.mult)
                        ps_tr = ps_scp.tile([P, DT, P], BF, tag="sc",
                                            name="tr2")
                        for dt_ in range(DT):
                            nc.tensor.transpose(
                                ps_tr[:, dt_, :],
                                h2_t[:, dt_ * P:(dt_ + 1) * P], ident16)
                        hi = h2s[:, b % 2, :, 0, su * P:(su + 1) * P]
                        nc.scalar.activation(out=hi, in_=ps_tr, func=AF.Copy,
                                             bias=0.0, scale=1.0)
                        nc.vector.tensor_tensor(
                            out=h2s[:, b % 2, :, 1, su * P:(su + 1) * P],
                            in0=ps_tr, in1=hi, op=OP.subtract)

                def front_score(st2):
                    if has_bias:
                        # rank-1 key-bias shift bq.k = h.(g1 Wk bq) rides the
                        # exp bias operand (keys are the psum partitions)
                        ps_b = ps_rzp.tile([P, TPC], F, tag="rz",
                                           name="ps_b")
                        for j in range(DT // 2):
                            nc.tensor.matmul(
                                ps_b[:, 0:1],
                                hT8[:, 2 * j:2 * j + 2,
                                    st2 * P:(st2 + 1) * P],
                                rk_t[:, 2 * j:2 * j + 2, :],
                                start=(j == 0), stop=(j == DT // 2 - 1),
                                perf_mode=DR)
                        bcol = pp.tile([P, 1], F, tag="bcol")
                        nc.vector.tensor_scalar(
                            out=bcol, in0=ps_b[:, 0:1],
                            scalar1=inv_sqrt_d / SW, scalar2=sb_col,
                            op0=OP.mult, op1=OP.add)
                        ebias = bcol
                    else:
                        ebias = nln4_t
                    ps = ps_scp.tile([P, CH], F, tag="sc", name="s")
                    for j in range(DT // 2):
                        nc.tensor.matmul(
                            ps,
                            hT8[:, 2 * j:2 * j + 2, st2 * P:(st2 + 1) * P],
                            g8[:, 2 * j:2 * j + 2, c * CH:(c + 1) * CH],
                            start=(j == 0), stop=(j == DT // 2 - 1),
                            perf_mode=DR)
                    nc.scalar.activation(out=e8_list[c][:, st2, :], in_=ps,
                                         func=AF.Exp, scale=inv_sqrt_d,
                                         bias=ebias)

                # ---- slot emission ----
                sci = [0]

                def emit_sc(n):
                    for _ in range(n):
                        if have_front and sci[0] < ST:
                            front_score(sci[0])
                            sci[0] += 1

                # in MLP-less slots the scores weave early so their exps
                # (ACT) finish before the next slot's gelus queue behind them
                early_sc = (m is None)
                if m is not None:
                    # prefetch the gelu act-table while the previous slot's
                    # exps drain, so gelu(0) doesn't stall the fc stream
                    nc.scalar.activation(out=warm, in_=warm, func=AF.Gelu,
                                         bias=0.0, scale=1.0)
                if b is not None:
                    # stage 1: attnV woven with fc
                    for dtp in range(DT):
                        if early_sc:
                            emit_sc(2)
                        back_attnv(dtp)
                        emit_fc(1)
                    if have_front and m is not None:
                        # produce this chunk's g = h @ M (consumed by the
                        # stage-5 scores)
                        g_chunk(c, ps_scp, "sc")
                    emit_fc(2)
                    # stage 2: Z + rz, more fc
                    if early_sc:
                        emit_sc(2)
                    z_row = back_z_rz()
                    emit_fc(4)
                    # stage 3: o-proj + x2 + stats, more fc
                    for su in range(TPC):
                        if early_sc:
                            emit_sc(1)
                        back_o(su, z_row)
                        emit_fc(2)
                    # stage 4: LN2 apply + transposes + h2s
                    quake_rsqrt(pp, mvs, rss)
                    for su in range(TPC):
                        back_ln2(su)
                        emit_fc(1)
                emit_fc(HT)  # any leftovers (and the c=5 no-back slot)
                # stage 5: proj woven with next chunk's scores+exps
                if m is not None:
                    for su in range(TPC):
                        for seg in range(2):
                            proj_su(su, seg)
                            emit_sc(2)
                emit_sc(ST)
        wphp_ctx.close()
        e8_ctx.close()
        qkv_ctx.close()

    return nc


def _get_nc():
    key = _CACHE.get("key")
    if "nc" not in _CACHE or key != (_CACHE.get("has_bias"), WPL_J):
        has_bias = _CACHE.get("has_bias", False)
        nc = _build(has_bias, WPL_J)
        nc.compile()
        _CACHE["nc"] = nc
        _CACHE["key"] = (has_bias, WPL_J)
    return _CACHE["nc"]


TRACE = False


def kernel(**inputs):
    from concourse.bass_utils import run_bass_kernel_spmd

    x = np.asarray(inputs["x"], dtype=np.float32)
    base = _prep(inputs)
    _CACHE["has_bias"] = base.pop("_has_bias")
    nc = _get_nc()
    names = {"wm8", "wv8", "wo8", "wfh", "wfl", "wph", "bfc_"}
    if WPL_J:
        names.add("wpl")
    if _CACHE["has_bias"]:
        names |= {"bo16", "bp16", "rk8", "sb"}
    ship = {k: v for k, v in base.items() if k in names}
    in_maps = [dict(ship, x=np.ascontiguousarray(x[bb]))
               for bb in range(N_CORES)]
    res = run_bass_kernel_spmd(nc, in_maps, core_ids=list(range(N_CORES)),
                               trace=TRACE)
    _CACHE["last_res"] = res
    return np.stack([res.results[bb]["out"] for bb in range(N_CORES)], axis=0)


# revision 48
# speedup vs baseline: 1.2528x; 1.0149x over previous
"""Trainium2 Bass kernel for a dense transformer block (B=8, S=2048, D=768, H=3072).

Sharding: pure data-parallel over batch -- one batch element per NeuronCore.

All heavy matmuls are fp8-e4m3 MatmulPerfMode.DoubleRow (K=256/instruction at
0.5 cycles/row). Cost model: matmul time = out_free_size * cycles_per_row, so
total PE time ~ (#contraction passes) x (output width); arms on the MLP
matmuls are the dominant cost.

Numerics (rel_absmax gate 2e-2; emulated 1.5e-2, baseline was 6.1e-3):
  - weights are pre-scaled by powers of two (SW=16 for the D-side mats,
    SP=32 for Wproj) so their entries sit in e4m3's normal range instead of
    half-denormal; the inverse scales ride existing activation scale operands
    (q/k/v/gelu) or the rz-extraction matmul (o-proj) for free.
  - attention path (q/k/v/scores/exp/y/o) is plain e4m3.
  - fc keeps 3 arms: Wfh*(hi+lo) + Wfl*hi (h2 split hi/lo e4m3; Wfl e5m2
    residual). proj runs 2 arms: Wph*(mhi+mlo) -- the Wpl residual arm is
    dropped (WPL_J can partially restore it).
  - exp is computed as exp(s/sqrt(d) - 3ln2) = e/8 so the unnormalized
    attention accumulator stays below e4m3's 240 max; 1/Z (and the 1/SW
    unscale) is applied per-token on the x2 write via a K=1 fp32 matmul that
    extracts rz columns (replaces the old fp32 transposes).
  - when all matmul biases fold to zero (true for this reference: betas and
    biases are zeros) the bias matmuls are omitted entirely at build time.

Schedule: a 3-stage software pipeline over the 4 token chunks, fused across
the old attention/MLP phase boundary so the MLP's PE-dense stream fills the
stalls of the attention dependency chains:
  slot c emits: attnV/Z/o-proj/LN2 of chunk c-1, fc+proj of chunk c-2, and
  scores+exp of chunk c (last, so the ACT engine does gelus first and exps
  at the end -- Exp and Gelu live in different activation-table sets, so this
  ordering costs only 2 table loads per slot).
x2 and h2s are rolling 2-chunk buffers; ms single-chunk. PSUM: 3 banks for
scores/Z/LN2-transposes, 2 for attnV/o-proj, 2 for fc/proj, 1 for rz.
"""

import numpy as np

P = 128
S, D, H = 2048, 768, 3072
DT = D // P            # 6 d-tiles
HT = H // P            # 24 h-tiles
ST = S // P            # 16 token tiles
CH = 512               # chunk width (tokens)
NCH = S // CH          # 4 chunks
TPC = CH // P          # 4 token tiles per chunk
EPS = 1e-5
N_CORES = 8
LN4 = 2.0794415416798357   # 3*ln2; exp bias so e8 = exp(s)/8
SW = 16.0                  # scale for D-side weight mats (sigma ~0.036)
SP = 32.0                  # scale for Wproj (sigma ~0.018)
WPL_J = 0                  # 0..12: partial Wpl residual passes (accuracy knob)

WEIGHT_NAMES = [
    "ln1_g", "ln1_b", "ln2_g", "ln2_b",
    "Wq", "bq", "Wk", "bk", "Wv", "bv", "Wo", "bo",
    "Wfc", "bfc", "Wproj", "bproj",
]

_CACHE = {}


def _prep(inputs):
    """Host-side weight quantization + LN/bias folding (pure numpy)."""
    import ml_dtypes
    E4, E5, BF = ml_dtypes.float8_e4m3, ml_dtypes.float8_e5m2, ml_dtypes.bfloat16
    f32 = lambda k: np.asarray(inputs[k], dtype=np.float32)
    g1, b1 = f32("ln1_g"), f32("ln1_b")
    g2, b2 = f32("ln2_g"), f32("ln2_b")
    Wq, Wk, Wv, Wo = f32("Wq"), f32("Wk"), f32("Wv"), f32("Wo")
    Wfc, Wproj = f32("Wfc"), f32("Wproj")
    q8 = lambda a: np.ascontiguousarray(a.astype(E4))
    Wfc_g = g2[:, None] * Wfc * SW
    wfh = Wfc_g.astype(E4)
    wph_f = Wproj * SP
    wph = wph_f.astype(E4)
    bo_f = f32("bo") + (f32("bv") + b1 @ Wv) @ Wo
    bp_f = f32("bproj")
    bq_ = f32("bq") + b1 @ Wq
    bk_ = f32("bk") + b1 @ Wk
    # scores = (h g1 Wq)(h g1 Wk)^T = h M h^T with M precomputed host-side:
    # the whole k tensor never materializes on device.
    M = g1[:, None] * (Wq @ Wk.T) * g1[None, :]
    # per-query bias shift q.bk cancels in softmax; per-key shift bq.k is
    # the rank-1 term h.(g1 Wk bq) riding the exp bias; bq.bk is a constant
    rk = g1 * (Wk @ bq_)
    out = {
        "wm8": q8(SW * M),
        "wv8": q8(SW * g1[:, None] * Wv), "wo8": q8(SW * Wo),
        "rk8": q8(SW * rk),
        "sb": np.full((P,), float(bq_ @ bk_) / np.sqrt(np.float32(D)) - LN4,
                      dtype=np.float32),
        "wfh": np.ascontiguousarray(wfh),
        "wfl": np.ascontiguousarray(
            (Wfc_g - wfh.astype(np.float32)).astype(E5)),
        "wph": np.ascontiguousarray(wph),
        "wpl": np.ascontiguousarray(
            (wph_f - wph.astype(np.float32)).astype(E5)),
        "bfc_": f32("bfc") + b2 @ Wfc,
        "bo16": np.asarray(SW * bo_f, dtype=BF),
        "bp16": np.asarray(SP * bp_f, dtype=BF),
    }
    out["_has_bias"] = bool(np.any(bo_f != 0.0) or np.any(bp_f != 0.0)
                            or np.any(bq_ != 0.0) or np.any(bk_ != 0.0))
    return out


def _build(has_bias, wpl_j):
    import concourse.bass as bass
    import concourse.tile as tile
    from concourse import bacc, mybir
    from concourse.masks import make_identity
    from contextlib import ExitStack

    F = mybir.dt.float32
    BF = mybir.dt.bfloat16
    E4 = mybir.dt.float8e4
    E5 = mybir.dt.float8e5
    I32 = mybir.dt.int32
    AF = mybir.ActivationFunctionType
    OP = mybir.AluOpType
    DR = mybir.MatmulPerfMode.DoubleRow

    nc = bacc.Bacc(None, target_bir_lowering=False)

    x_d = nc.dram_tensor("x", [S, D], F, kind="ExternalInput")
    out_d = nc.dram_tensor("out", [S, D], F, kind="ExternalOutput")
    wm8_d = nc.dram_tensor("wm8", [D, D], E4, kind="ExternalInput")
    wv8_d = nc.dram_tensor("wv8", [D, D], E4, kind="ExternalInput")
    wo8_d = nc.dram_tensor("wo8", [D, D], E4, kind="ExternalInput")
    wfh_d = nc.dram_tensor("wfh", [D, H], E4, kind="ExternalInput")
    wfl_d = nc.dram_tensor("wfl", [D, H], E5, kind="ExternalInput")
    wph_d = nc.dram_tensor("wph", [H, D], E4, kind="ExternalInput")
    bfc_d = nc.dram_tensor("bfc_", [H], F, kind="ExternalInput")
    if wpl_j:
        wpl_d = nc.dram_tensor("wpl", [H, D], E5, kind="ExternalInput")
    if has_bias:
        bo_d = nc.dram_tensor("bo16", [D], BF, kind="ExternalInput")
        bp_d = nc.dram_tensor("bp16", [D], BF, kind="ExternalInput")
        rk_d = nc.dram_tensor("rk8", [D], E4, kind="ExternalInput")
        sb_d = nc.dram_tensor("sb", [P], F, kind="ExternalInput")

    inv_sqrt_d = 1.0 / float(np.sqrt(np.float32(D)))

    with tile.TileContext(nc) as tc, ExitStack() as ctx:
        singles = ctx.enter_context(tc.tile_pool(name="singles", bufs=1))

        ident16 = singles.tile([P, P], BF)
        make_identity(nc, ident16)
        warm = singles.tile([1, 1], F)
        nc.vector.memset(warm, 0.0)
        nln4_t = singles.tile([P, 1], F)
        nc.vector.memset(nln4_t, -LN4)
        invsw = singles.tile([1, 1], F)
        nc.vector.memset(invsw, 1.0 / SW)
        ones8 = singles.tile([P, 2, P], E4)
        nc.vector.memset(ones8, 1.0)
        bfc_col = singles.tile([P, HT], F)
        if has_bias:
            ones_row = singles.tile([1, P], BF)
            nc.vector.memset(ones_row, 1.0)
            bo_row = singles.tile([1, D], BF)
            bp_row = singles.tile([1, D], BF)
            rk_t = singles.tile([P, DT, 1], E4)
            sb_col = singles.tile([P, 1], F)

        # persistent activations
        perm = ctx.enter_context(tc.tile_pool(name="perm", bufs=1))
        x2_sb = perm.tile([P, 2 * TPC, D], F)       # rolling 2-chunk residual
        h2s = perm.tile([P, 2, DT, 2, CH], E4)      # rolling LN2 out hi/lo
        ms = perm.tile([P, HT, 2, CH], E4)          # gelu out hi/lo (1 chunk)

        wfhp = ctx.enter_context(tc.tile_pool(name="wfhp", bufs=1))
        wfh_t = wfhp.tile([P, DT, H], E4)
        wflp = ctx.enter_context(tc.tile_pool(name="wflp", bufs=1))
        wfl_t = wflp.tile([P, DT, H], E5)

        qkv_ctx = ExitStack()
        qkvp = qkv_ctx.enter_context(tc.tile_pool(name="qkv", bufs=1))
        hT8 = qkvp.tile([P, DT, S], E4)      # LN1 out, d-major (scores lhsT)
        g8 = qkvp.tile([P, DT, S], E4)       # h @ M, d-major (scores rhs)
        v8 = qkvp.tile([P, ST, D], E4)
        wo_t = qkvp.tile([P, DT, D], E4)
        wm8_t = qkvp.tile([P, DT, D], E4)

        e8_ctx = ExitStack()
        e8p = e8_ctx.enter_context(tc.tile_pool(name="e8p", bufs=2))
        e8_list = [None] * NCH

        wqkv_ctx = ExitStack()
        wqkv = wqkv_ctx.enter_context(tc.tile_pool(name="wqkv", bufs=1))
        wv_t = wqkv.tile([P, DT, D], E4)

        # ------------- Phase 1: LN1 -> hT8; v, k, q (all fp8) -------------
        with (
            tc.tile_pool(name="ph1", bufs=3) as ph1,
            tc.tile_pool(name="ps_tr", bufs=2, space="PSUM") as ps_trp,
            tc.tile_pool(name="ps_k", bufs=3, space="PSUM") as ps_kp,
            tc.tile_pool(name="ps_v", bufs=3, space="PSUM") as ps_vp,
        ):
            x_ts = [None] * ST
            h_ts = [None] * ST
            # trigger the exp-set act table load at t=0, off the critical path
            nc.scalar.activation(out=warm, in_=warm, func=AF.Exp,
                                 bias=0.0, scale=1.0)
            # qkv weights ride the otherwise-idle ACT DMA queue; x tiles
            # alternate SP/Pool so nothing queues behind a weight transfer
            nc.scalar.dma_start(
                wv_t[:], wv8_d.ap().rearrange("(t p) n -> p t n", p=P))
            nc.scalar.dma_start(
                wm8_t[:], wm8_d.ap().rearrange("(t p) n -> p t n", p=P))

            def ln1_group(g0, n):
                mvg = ph1.tile([P, TPC, 2], F, tag="mvg")
                for i in range(n):
                    st = g0 + i
                    x_t = ph1.tile([P, D], F, tag="xt", bufs=6)
                    q = nc.gpsimd if st % 2 else nc.sync
                    q.dma_start(out=x_t, in_=x_d.ap()[st * P:(st + 1) * P, :])
                    if st == 9:
                        nc.sync.dma_start(
                            wo_t[:],
                            wo8_d.ap().rearrange("(t p) n -> p t n", p=P))
                        nc.sync.dma_start(
                            bfc_col, bfc_d.ap().rearrange("(t p) -> p t", p=P))
                    x_ts[st] = x_t
                    stats = ph1.tile([P, 2, 6], F, tag="st")
                    nc.vector.bn_stats(out=stats[:, 0, :], in_=x_t[:, :512])
                    nc.vector.bn_stats(out=stats[:, 1, :], in_=x_t[:, 512:])
                    nc.vector.bn_aggr(out=mvg[:, i, :], in_=stats)
                # batched rsqrt(var+eps): quake bit-trick + 1 Newton, on Pool
                # so the DVE stats stream isn't serialized behind it
                rsg = ph1.tile([P, TPC], F, tag="rsg")
                nc.gpsimd.tensor_scalar(out=rsg[:, :n], in0=mvg[:, :n, 1],
                                        scalar1=EPS, scalar2=None, op0=OP.add)
                rig = ph1.tile([P, TPC], I32, tag="rig")
                nc.gpsimd.tensor_scalar(out=rig[:, :n],
                                        in0=rsg[:, :n].bitcast(I32),
                                        scalar1=1, scalar2=None,
                                        op0=OP.logical_shift_right)
                nc.gpsimd.tensor_scalar(out=rig[:, :n], in0=rig[:, :n],
                                        scalar1=-1,
                                        scalar2=None, op0=OP.bitwise_xor)
                nc.gpsimd.tensor_scalar(out=rig[:, :n], in0=rig[:, :n],
                                        scalar1=0x5f3759e0,
                                        scalar2=None, op0=OP.add)
                rng = ph1.tile([P, TPC], F, tag="rng")
                nc.gpsimd.tensor_tensor(out=rng[:, :n],
                                        in0=rig[:, :n].bitcast(F),
                                        in1=rig[:, :n].bitcast(F), op=OP.mult)
                nc.gpsimd.tensor_tensor(out=rng[:, :n], in0=rng[:, :n],
                                        in1=rsg[:, :n], op=OP.mult)
                nc.gpsimd.tensor_scalar(out=rng[:, :n], in0=rng[:, :n],
                                        scalar1=-0.5,
                                        scalar2=1.5, op0=OP.mult, op1=OP.add)
                nc.gpsimd.tensor_tensor(out=rsg[:, :n],
                                        in0=rig[:, :n].bitcast(F),
                                        in1=rng[:, :n], op=OP.mult)
                for i in range(n):
                    st = g0 + i
                    h_t = ph1.tile([P, D], BF, tag="ht", bufs=6)
                    nc.gpsimd.tensor_scalar(out=h_t, in0=x_ts[st],
                                            scalar1=mvg[:, i, 0:1],
                                            scalar2=rsg[:, i:i + 1],
                                            op0=OP.subtract, op1=OP.mult)
                    h_ts[st] = h_t

            def consume_trv(g0, n):
                for i in range(n):
                    sp = g0 + i
                    h_t = h_ts[sp]
                    ps_tr = ps_trp.tile([P, DT, P], BF, tag="tr")
                    for dt_ in range(DT):
                        nc.tensor.transpose(ps_tr[:, dt_, :],
                                            h_t[:, dt_ * P:(dt_ + 1) * P],
                                            ident16)
                    if sp % 3:
                        nc.scalar.activation(
                            out=hT8[:, :, sp * P:(sp + 1) * P],
                            in_=ps_tr, func=AF.Copy, bias=0.0, scale=1.0)
                    else:
                        nc.vector.tensor_copy(
                            out=hT8[:, :, sp * P:(sp + 1) * P], in_=ps_tr)
                for i in range(n):
                    sv = g0 + i
                    for lo, w, eng in ((0, 512, 0), (512, 256, 1)):
                        ps_v = ps_vp.tile([P, CH], F, tag="v")
                        for j in range(DT // 2):
                            nc.tensor.matmul(
                                ps_v[:, :w],
                                hT8[:, 2 * j:2 * j + 2, sv * P:(sv + 1) * P],
                                wv_t[:, 2 * j:2 * j + 2, lo:lo + w],
                                start=(j == 0), stop=(j == DT // 2 - 1),
                                perf_mode=DR)
                        # bv folds into bo' host-side (bo16)
                        if eng:
                            nc.scalar.activation(
                                out=v8[:, sv, lo:lo + w], in_=ps_v[:, :w],
                                func=AF.Identity, bias=0.0, scale=1.0 / SW)
                        else:
                            nc.vector.tensor_scalar(
                                out=v8[:, sv, lo:lo + w], in0=ps_v[:, :w],
                                scalar1=1.0 / SW, scalar2=None, op0=OP.mult)

            def g_chunk(gc, pool, tag):
                # g = h @ M for one 512-token chunk (the q of the folded
                # score form); copies alternate ACT/DVE
                for dtp in range(DT):
                    ps_q = pool.tile([P, CH], F, tag=tag, name="ps_g")
                    for j in range(DT // 2):
                        nc.tensor.matmul(
                            ps_q,
                            wm8_t[:, 2 * j:2 * j + 2, dtp * P:(dtp + 1) * P],
                            hT8[:, 2 * j:2 * j + 2, gc * CH:(gc + 1) * CH],
                            start=(j == 0), stop=(j == DT // 2 - 1),
                            perf_mode=DR)
                    if dtp % 2:
                        nc.scalar.activation(
                            out=g8[:, dtp, gc * CH:(gc + 1) * CH], in_=ps_q,
                            func=AF.Identity, bias=0.0, scale=1.0 / SW)
                    else:
                        nc.vector.tensor_scalar(
                            out=g8[:, dtp, gc * CH:(gc + 1) * CH], in0=ps_q,
                            scalar1=1.0 / SW, scalar2=None, op0=OP.mult)

            def consume_chunk(kc):
                for i in range(TPC):
                    sv = TPC * kc + i
                    for lo, w, eng in ((0, 512, 0), (512, 256, 1)):
                        ps_v = ps_vp.tile([P, CH], F, tag="v")
                        for j in range(DT // 2):
                            nc.tensor.matmul(
                                ps_v[:, :w],
                                hT8[:, 2 * j:2 * j + 2, sv * P:(sv + 1) * P],
                                wv_t[:, 2 * j:2 * j + 2, lo:lo + w],
                                start=(j == 0), stop=(j == DT // 2 - 1),
                                perf_mode=DR)
                        # bv folds into bo' host-side (bo16)
                        if eng:
                            nc.scalar.activation(
                                out=v8[:, sv, lo:lo + w], in_=ps_v[:, :w],
                                func=AF.Identity, bias=0.0, scale=1.0 / SW)
                        else:
                            nc.vector.tensor_scalar(
                                out=v8[:, sv, lo:lo + w], in0=ps_v[:, :w],
                                scalar1=1.0 / SW, scalar2=None, op0=OP.mult)
                if kc == 0:
                    e8_list[0] = e8p.tile([P, ST, CH], E4, tag="e8",
                                          name="e8_c0")
                    g_chunk(0, ps_kp, "k")
                # chunk-0 scores for this k-chunk's keys
                for st2 in range(TPC * kc, TPC * kc + TPC):
                    ps_s = ps_kp.tile([P, CH], F, tag="k", name="ps_s0")
                    for j in range(DT // 2):
                        nc.tensor.matmul(
                            ps_s,
                            hT8[:, 2 * j:2 * j + 2, st2 * P:(st2 + 1) * P],
                            g8[:, 2 * j:2 * j + 2, 0:CH],
                            start=(j == 0), stop=(j == DT // 2 - 1),
                            perf_mode=DR)
                    nc.scalar.activation(out=e8_list[0][:, st2, :], in_=ps_s,
                                         func=AF.Exp, scale=inv_sqrt_d,
                                         bias=nln4_t)

            groups = [(g0, 2) for g0 in range(0, ST, 2)]
            done_chunks = 0
            for gi in range(len(groups) + 1):
                if gi >= 1:
                    g0, n = groups[gi - 1]
                    consume_trv(g0, n)
                    full = (g0 + n) // TPC
                    while done_chunks < full:
                        consume_chunk(done_chunks)
                        done_chunks += 1
                if gi < len(groups):
                    ln1_group(*groups[gi])
            # g for chunk 1 (slot 1's scores weave early and need it);
            # chunks 2-3 are produced inside slots 2-3
            g_chunk(1, ps_kp, "k")
        wqkv_ctx.close()

        # MLP fc weights land during slot 1 (the DMA device is saturated with
        # x tiles + qkv weights during phase 1; here it idles)
        nc.sync.dma_start(wfh_t[:],
                          wfh_d.ap().rearrange("(t p) n -> p t n", p=P))
        nc.sync.dma_start(wfl_t[:],
                          wfl_d.ap().rearrange("(t p) n -> p t n", p=P))

        # MLP proj weights arrive into the space hT8/wq/wk/wv vacated
        wphp_ctx = ExitStack()
        wphp = wphp_ctx.enter_context(tc.tile_pool(name="wphp", bufs=1))
        wph_t = wphp.tile([P, HT, D], E4)
        nc.sync.dma_start(wph_t[:],
                          wph_d.ap().rearrange("(t p) n -> p t n", p=P))
        if wpl_j:
            wpl_t = wphp.tile([P, 2 * wpl_j, D], E5)
            nc.sync.dma_start(
                wpl_t[:],
                wpl_d.ap()[0:2 * wpl_j * P, :].rearrange(
                    "(t p) n -> p t n", p=P))
        if has_bias:
            nc.sync.dma_start(bo_row, bo_d.ap().unsqueeze(0))
            nc.sync.dma_start(bp_row, bp_d.ap().unsqueeze(0))
            nc.sync.dma_start(
                rk_t, rk_d.ap().rearrange("(t p) -> p t", p=P).unsqueeze(2))
            nc.sync.dma_start(sb_col, sb_d.ap().unsqueeze(1))

        def quake_rsqrt(pool, mvs, rss):
            # rsqrt(var+eps): quake bit-trick + 2 Newton steps, all on DVE
            vb = pool.tile([P, TPC], F, tag="vb")
            nc.vector.tensor_scalar(out=vb, in0=mvs[:, :, 1], scalar1=EPS,
                                    scalar2=None, op0=OP.add)
            ib = pool.tile([P, TPC], I32, tag="ib")
            nc.vector.tensor_scalar(out=ib, in0=vb[:].bitcast(I32),
                                    scalar1=1, scalar2=None,
                                    op0=OP.logical_shift_right)
            nc.vector.tensor_scalar(out=ib, in0=ib, scalar1=-1,
                                    scalar2=None, op0=OP.bitwise_xor)
            nc.vector.tensor_scalar(out=ib, in0=ib, scalar1=0x5f3759e0,
                                    scalar2=None, op0=OP.add)
            nc.vector.tensor_copy(out=rss, in_=ib[:].bitcast(F))
            nt = pool.tile([P, TPC], F, tag="nt")
            for _ in range(2):
                nc.vector.tensor_tensor(out=nt, in0=rss, in1=rss, op=OP.mult)
                nc.vector.tensor_tensor(out=nt, in0=nt, in1=vb, op=OP.mult)
                nc.vector.tensor_scalar(out=nt, in0=nt, scalar1=-0.5,
                                        scalar2=1.5, op0=OP.mult, op1=OP.add)
                nc.vector.tensor_tensor(out=rss, in0=rss, in1=nt, op=OP.mult)

        # ------------- Fused pipeline: slots c = 1..5 -------------
        # slot c: back(c-1) attnV/Z/o/LN2 + mlp(c-2) fc/proj + front(c) scores
        with (
            tc.tile_pool(name="pp", bufs=2) as pp,
            tc.tile_pool(name="ps_sc", bufs=2, space="PSUM") as ps_scp,
            tc.tile_pool(name="ps_av", bufs=2, space="PSUM") as ps_avp,
            tc.tile_pool(name="ps_fp", bufs=3, space="PSUM") as ps_fpp,
            tc.tile_pool(name="ps_rz", bufs=1, space="PSUM") as ps_rzp,
        ):
            for c in range(1, 6):
                b = c - 1 if c - 1 <= 3 else None      # back chunk
                m = c - 2 if 0 <= c - 2 <= 3 else None  # mlp chunk
                have_front = c <= 3

                if have_front:
                    e8_list[c] = e8p.tile([P, ST, CH], E4, tag="e8",
                                          name=f"e8_{c}")

                # ---- mlp(m) quanta ----
                fcq = []
                if m is not None:
                    hs = h2s[:, m % 2]

                    def fc_ht(ht, hs=hs):
                        ps = ps_fpp.tile([P, CH], F, tag="fp", name="u")
                        for j in range(DT // 2):
                            nc.tensor.matmul(
                                ps, wfh_t[:, 2 * j:2 * j + 2,
                                          ht * P:(ht + 1) * P],
                                hs[:, 2 * j:2 * j + 2, 0, :],
                                start=(j == 0), stop=False, perf_mode=DR)
                        for j in range(DT // 2):
                            nc.tensor.matmul(
                                ps, wfh_t[:, 2 * j:2 * j + 2,
                                          ht * P:(ht + 1) * P],
                                hs[:, 2 * j:2 * j + 2, 1, :],
                                start=False, stop=False, perf_mode=DR)
                        for j in range(DT // 2):
                            nc.tensor.matmul(
                                ps, wfl_t[:, 2 * j:2 * j + 2,
                                          ht * P:(ht + 1) * P],
                                hs[:, 2 * j:2 * j + 2, 0, :],
                                start=False, stop=(j == DT // 2 - 1),
                                perf_mode=DR)
                        m16 = pp.tile([P, CH], BF, tag="m16", bufs=2)
                        nc.scalar.activation(out=m16, in_=ps, func=AF.Gelu,
                                             bias=bfc_col[:, ht:ht + 1],
                                             scale=1.0 / SW)
                        nc.gpsimd.tensor_copy(out=ms[:, ht, 0, :], in_=m16)
                        nc.gpsimd.tensor_tensor(out=ms[:, ht, 1, :], in0=m16,
                                                in1=ms[:, ht, 0, :],
                                                op=OP.subtract)

                    fcq = [(lambda ht=ht: fc_ht(ht)) for ht in range(HT)]

                fci = [0]

                def emit_fc(n):
                    for _ in range(n):
                        if fci[0] < len(fcq):
                            fcq[fci[0]]()
                            fci[0] += 1

                def proj_su(su, seg, o_ts={}):
                    lo, w = (0, 512) if seg == 0 else (512, 256)
                    ps = ps_fpp.tile([P, CH], F, tag="fp", name="pj")
                    last = ('b' if has_bias else
                            ('l' if wpl_j else 'a'))
                    for arm in range(2):
                        for j in range(HT // 2):
                            isl = (last == 'a' and arm == 1
                                   and j == HT // 2 - 1)
                            nc.tensor.matmul(
                                ps[:, :w],
                                ms[:, 2 * j:2 * j + 2, arm,
                                   su * P:(su + 1) * P],
                                wph_t[:, 2 * j:2 * j + 2, lo:lo + w],
                                start=(arm == 0 and j == 0), stop=isl,
                                perf_mode=DR)
                    if wpl_j:
                        for j in range(wpl_j):
                            isl = (last == 'l' and j == wpl_j - 1)
                            nc.tensor.matmul(
                                ps[:, :w],
                                ms[:, 2 * j:2 * j + 2, 0,
                                   su * P:(su + 1) * P],
                                wpl_t[:, 2 * j:2 * j + 2, lo:lo + w],
                                start=False, stop=isl, perf_mode=DR)
                    if has_bias:
                        nc.tensor.matmul(ps[:, :w], ones_row,
                                         bp_row[:, lo:lo + w],
                                         start=False, stop=True)
                    st_sl = (m % 2) * TPC + su
                    if seg == 0:
                        o_ts[su] = pp.tile([P, D], F, tag="ot", bufs=2,
                                           name="o_t")
                    o_t = o_ts[su]
                    nc.scalar.activation(out=o_t[:, lo:lo + w],
                                         in_=ps[:, :w], func=AF.Identity,
                                         bias=0.0, scale=1.0 / SP)
                    nc.gpsimd.tensor_tensor(out=o_t[:, lo:lo + w],
                                            in0=o_t[:, lo:lo + w],
                                            in1=x2_sb[:, st_sl, lo:lo + w],
                                            op=OP.add)
                    # per-segment DMA shortens the end-of-kernel drain
                    st = m * TPC + su
                    nc.sync.dma_start(
                        out_d.ap()[st * P:(st + 1) * P, lo:lo + w],
                        o_t[:, lo:lo + w])

                # ---- back(b) helpers ----
                if b is not None:
                    e8b = e8_list[b]
                    xb_ts = []
                    for su in range(TPC):
                        st = b * TPC + su
                        x_t = pp.tile([P, D], F, tag="xb", bufs=4)
                        nc.sync.dma_start(x_t,
                                          x_d.ap()[st * P:(st + 1) * P, :])
                        xb_ts.append(x_t)
                    yT8 = pp.tile([P, DT, CH], E4, tag="yt", bufs=2)
                    mvs = pp.tile([P, TPC, 2], F, tag="mvs")
                    rss = pp.tile([P, TPC], F, tag="rss")
                    rz_cols = pp.tile([P, TPC], F, tag="rzc")

                    def back_attnv(dtp):
                        ps = ps_avp.tile([P, CH], F, tag="av", name="av")
                        for pr in range(ST // 2):
                            nc.tensor.matmul(
                                ps,
                                v8[:, 2 * pr:2 * pr + 2,
                                   dtp * P:(dtp + 1) * P],
                                e8b[:, 2 * pr:2 * pr + 2, :],
                                start=(pr == 0), stop=(pr == ST // 2 - 1),
                                perf_mode=DR)
                        if dtp % 2 or m is None:
                            nc.scalar.activation(out=yT8[:, dtp, :], in_=ps,
                                                 func=AF.Copy, bias=0.0,
                                                 scale=1.0)
                        else:
                            nc.vector.tensor_copy(out=yT8[:, dtp, :], in_=ps)

                    def back_z_rz():
                        ps_z = ps_scp.tile([P, CH], F, tag="sc", name="z")
                        for pr in range(ST // 2):
                            nc.tensor.matmul(ps_z, ones8,
                                             e8b[:, 2 * pr:2 * pr + 2, :],
                                             start=(pr == 0),
                                             stop=(pr == ST // 2 - 1),
                                             perf_mode=DR)
                        if has_bias:
                            z_row = pp.tile([1, CH], BF, tag="zrow")
                            nc.vector.tensor_copy(out=z_row, in_=ps_z[0:1, :])
                        else:
                            z_row = None
                        rz = pp.tile([P, CH], F, tag="rz", bufs=1)
                        nc.vector.reciprocal(out=rz, in_=ps_z)
                        ps_rz = ps_rzp.tile([P, TPC], F, tag="rz")
                        for su in range(TPC):
                            # K=1 fp32 matmul: broadcast rz row -> per-token
                            # column, pre-divided by SW (invsw operand)
                            nc.tensor.matmul(
                                ps_rz[:, su:su + 1],
                                rz[0:1, su * P:(su + 1) * P],
                                invsw, start=True, stop=True)
                        nc.vector.tensor_copy(out=rz_cols, in_=ps_rz)
                        return z_row

                    def back_o(su, z_row):
                        st_sl = (b % 2) * TPC + su
                        for lo, w in ((0, 512), (512, 256)):
                            ps = ps_avp.tile([P, CH], F, tag="av", name="o")
                            for j in range(DT // 2):
                                nc.tensor.matmul(
                                    ps[:, :w],
                                    yT8[:, 2 * j:2 * j + 2,
                                        su * P:(su + 1) * P],
                                    wo_t[:, 2 * j:2 * j + 2, lo:lo + w],
                                    start=(j == 0),
                                    stop=(j == DT // 2 - 1 and
                                          not has_bias),
                                    perf_mode=DR)
                            if has_bias:
                                # bo rides as bo*Z*SW; the rz/SW scaling
                                # cancels it back to +bo
                                nc.tensor.matmul(ps[:, :w],
                                                 z_row[:, su * P:(su + 1) * P],
                                                 bo_row[:, lo:lo + w],
                                                 start=False, stop=True)
                            nc.vector.tensor_scalar(
                                out=x2_sb[:, st_sl, lo:lo + w],
                                in0=ps[:, :w],
                                scalar1=rz_cols[:, su:su + 1],
                                scalar2=None, op0=OP.mult)
                            nc.gpsimd.tensor_tensor(
                                out=x2_sb[:, st_sl, lo:lo + w],
                                in0=x2_sb[:, st_sl, lo:lo + w],
                                in1=xb_ts[su][:, lo:lo + w], op=OP.add)
                        stats = pp.tile([P, 3, 6], F, tag="st3")
                        for i in range(3):
                            nc.vector.bn_stats(
                                out=stats[:, i, :],
                                in_=x2_sb[:, st_sl, i * 256:(i + 1) * 256])
                        nc.vector.bn_aggr(out=mvs[:, su, :], in_=stats)

                    def back_ln2(su):
                        st_sl = (b % 2) * TPC + su
                        h2_t = pp.tile([P, D], BF, tag="h2", bufs=2)
                        nc.gpsimd.tensor_scalar(out=h2_t,
                                                in0=x2_sb[:, st_sl, :],
                                                scalar1=mvs[:, su, 0:1],
                                                scalar2=rss[:, su:su + 1],
                                                op0=OP.subtract, op1=OP.mult)
                        ps_tr = ps_scp.tile([P, DT, P], BF, tag="sc",
                                            name="tr2")
                        for dt_ in range(DT):
                            nc.tensor.transpose(
                                ps_tr[:, dt_, :],
                                h2_t[:, dt_ * P:(dt_ + 1) * P], ident16)
                        hi = h2s[:, b % 2, :, 0, su * P:(su + 1) * P]
                        nc.scalar.activation(out=hi, in_=ps_tr, func=AF.Copy,
                                             bias=0.0, scale=1.0)
                        nc.vector.tensor_tensor(
                            out=h2s[:, b % 2, :, 1, su * P:(su + 1) * P],
                            in0=ps_tr, in1=hi, op=OP.subtract)

                def front_score(st2):
                    if has_bias:
                        # rank-1 key-bias shift bq.k = h.(g1 Wk bq) rides the
                        # exp bias operand (keys are the psum partitions)
                        ps_b = ps_rzp.tile([P, TPC], F, tag="rz",
                                           name="ps_b")
                        for j in range(DT // 2):
                            nc.tensor.matmul(
                                ps_b[:, 0:1],
                                hT8[:, 2 * j:2 * j + 2,
                                    st2 * P:(st2 + 1) * P],
                                rk_t[:, 2 * j:2 * j + 2, :],
                                start=(j == 0), stop=(j == DT // 2 - 1),
                                perf_mode=DR)
                        bcol = pp.tile([P, 1], F, tag="bcol")
                        nc.vector.tensor_scalar(
                            out=bcol, in0=ps_b[:, 0:1],
                            scalar1=inv_sqrt_d / SW, scalar2=sb_col,
                            op0=OP.mult, op1=OP.add)
                        ebias = bcol
                    else:
                        ebias = nln4_t
                    ps = ps_scp.tile([P, CH], F, tag="sc", name="s")
                    for j in range(DT // 2):
                        nc.tensor.matmul(
                            ps,
                            hT8[:, 2 * j:2 * j + 2, st2 * P:(st2 + 1) * P],
                            g8[:, 2 * j:2 * j + 2, c * CH:(c + 1) * CH],
                            start=(j == 0), stop=(j == DT // 2 - 1),
                            perf_mode=DR)
                    nc.scalar.activation(out=e8_list[c][:, st2, :], in_=ps,
                                         func=AF.Exp, scale=inv_sqrt_d,
                                         bias=ebias)

                # ---- slot emission ----
                sci = [0]

                def emit_sc(n):
                    for _ in range(n):
                        if have_front and sci[0] < ST:
                            front_score(sci[0])
                            sci[0] += 1

                # in MLP-less slots the scores weave early so their exps
                # (ACT) finish before the next slot's gelus queue behind them
                early_sc = (m is None)
                if m is not None:
                    # prefetch the gelu act-table while the previous slot's
                    # exps drain, so gelu(0) doesn't stall the fc stream
                    nc.scalar.activation(out=warm, in_=warm, func=AF.Gelu,
                                         bias=0.0, scale=1.0)
                if b is not None:
                    # stage 1: attnV woven with fc
                    for dtp in range(DT):
                        if early_sc:
                            emit_sc(2)
                        back_attnv(dtp)
                        emit_fc(1)
                    if have_front and m is not None:
                        # produce this chunk's g = h @ M (consumed by the
                        # stage-5 scores)
                        g_chunk(c, ps_scp, "sc")
                    emit_fc(2)
                    # stage 2: Z + rz, more fc
                    if early_sc:
                        emit_sc(2)
                    z_row = back_z_rz()
                    emit_fc(4)
                    # stage 3: o-proj + x2 + stats, more fc
                    for su in range(TPC):
                        if early_sc:
                            emit_sc(1)
                        back_o(su, z_row)
                        emit_fc(2)
                    # stage 4: LN2 apply + transposes + h2s
                    quake_rsqrt(pp, mvs, rss)
                    for su in range(TPC):
                        back_ln2(su)
                        emit_fc(1)
                emit_fc(HT)  # any leftovers (and the c=5 no-back slot)
                # stage 5: proj woven with next chunk's scores+exps
                if m is not None:
                    for su in range(TPC):
                        for seg in range(2):
                            proj_su(su, seg)
                            emit_sc(2)
                emit_sc(ST)
        wphp_ctx.close()
        e8_ctx.close()
        qkv_ctx.close()

    return nc


def _get_nc():
    key = _CACHE.get("key")
    if "nc" not in _CACHE or key != (_CACHE.get("has_bias"), WPL_J):
        has_bias = _CACHE.get("has_bias", False)
        nc = _build(has_bias, WPL_J)
        nc.compile()
        _CACHE["nc"] = nc
        _CACHE["key"] = (has_bias, WPL_J)
    return _CACHE["nc"]


TRACE = False


def kernel(**inputs):
    from concourse.bass_utils import run_bass_kernel_spmd

    x = np.asarray(inputs["x"], dtype=np.float32)
    base = _prep(inputs)
    _CACHE["has_bias"] = base.pop("_has_bias")
    nc = _get_nc()
    names = {"wm8", "wv8", "wo8", "wfh", "wfl", "wph", "bfc_"}
    if WPL_J:
        names.add("wpl")
    if _CACHE["has_bias"]:
        names |= {"bo16", "bp16", "rk8", "sb"}
    ship = {k: v for k, v in base.items() if k in names}
    in_maps = [dict(ship, x=np.ascontiguousarray(x[bb]))
               for bb in range(N_CORES)]
    res = run_bass_kernel_spmd(nc, in_maps, core_ids=list(range(N_CORES)),
                               trace=TRACE)
    _CACHE["last_res"] = res
    return np.stack([res.results[bb]["out"] for bb in range(N_CORES)], axis=0)
